# revision 1
# baseline (speedup 1.0000x reference)
"""Distributed GAT (nn_AdjGAT) kernel for 8 TRN2 NeuronCores.

Math: the reference's per-edge softmax logit depends ONLY on the source
node (scores[h,v,k] = attn[h, adj[v,k]]), so
    softmax_k -> w[n_k] / sum_j w[n_j],  w = exp(attn)
    head[h,v] = (sum_k w[h,n_k] * t[h,n_k]) / (sum_k w[h,n_k])
    out = relu(mean_h(head + b[h]))
Each core computes per-node rows [w*t_0..w*t_3 | w_0..w_3] (bf16, padded
to 640 elems = 1280B), AllGathers the row table, then per 128-node chunk:
one dma_gather of 2048 neighbor rows + block-diagonal-ones matmuls that
sum each node's K=16 rows (numerator AND denominator in one pass), then
normalize/relu.  Padding slots point at a hardwired zero row (w=0).
"""

import math
from contextlib import ExitStack

import numpy as np

import concourse.bass as bass
import concourse.bacc as bacc
import concourse.mybir as mybir
from concourse import library_config
from concourse.bass_utils import run_bass_kernel_spmd

F32 = mybir.dt.float32
BF16 = mybir.dt.bfloat16
I16 = mybir.dt.int16
I32 = mybir.dt.int32

V, D, K, O, H = 20000, 256, 16, 128, 4
NCORES = 8


class Cfg:
    def __init__(self, V=V, D=D, K=K, O=O, H=H, ncores=NCORES):
        assert V % ncores == 0
        self.V, self.D, self.K, self.O, self.H, self.ncores = V, D, K, O, H, ncores
        self.VP = V // ncores
        self.ZPAD = 12
        self.VPAD = self.VP + self.ZPAD
        self.VT = self.VPAD * ncores
        self.RW = H * O + 128          # 640 bf16 elems: [s (512) | w (4) | pad]
        self.NCHUNK = math.ceil(self.VP / 128)
        self.VBLK = 512 if self.VP >= 512 else 128 * math.ceil(self.VP / 128)
        self.NBLK = math.ceil(self.VP / self.VBLK)
        self.NJ = self.VBLK // 128
        self.VPF = self.NBLK * self.VBLK
        self.DC = D // 128
        assert D % 128 == 0 and O == 128 and K == 16 and self.DC >= 1
        assert self.VT < 32000


def build_graph(cfg: Cfg, skip_ag=False, only=None, debug=False):
    nc = bacc.Bacc()
    VP, VPAD, VT, RW, H_, O_, DC = (
        cfg.VP, cfg.VPAD, cfg.VT, cfg.RW, cfg.H, cfg.O, cfg.DC)
    NCH, NBLK, VBLK, NJ, VPF = cfg.NCHUNK, cfg.NBLK, cfg.VBLK, cfg.NJ, cfg.VPF
    SW = H_ * O_
    NE = 128 * 16
    NIT = NBLK * H_

    xT = nc.declare_dram_parameter("xT", [cfg.D, VP], F32, isOutput=False)
    Wp = nc.declare_dram_parameter("W", [H_, cfg.D, O_], F32, isOutput=False)
    ap_ = nc.declare_dram_parameter("a", [H_, O_], F32, isOutput=False)
    bp = nc.declare_dram_parameter("b", [H_, O_], F32, isOutput=False)
    eyep = nc.declare_dram_parameter("eye", [128, 128], F32, isOutput=False)
    blkp = nc.declare_dram_parameter("blk", [128, 248], F32, isOutput=False)
    idxp = nc.declare_dram_parameter("idx", [128, NCH * 16], I32, isOutput=False)
    out_ext = nc.declare_dram_parameter("out", [VP, O_], F32, isOutput=True)
    if debug:
        dbg_t0 = nc.declare_dram_parameter("dbg_t0", [128, RW], BF16, isOutput=True)
        dbg_t1 = nc.declare_dram_parameter("dbg_t1", [128, RW], BF16, isOutput=True)
        dbg_gt = nc.declare_dram_parameter("dbg_gt", [128, 16, RW], BF16,
                                           isOutput=True)

    tbl_loc = nc.dram_tensor("tbl_loc", [VPAD, RW], BF16)
    tbl = nc.dram_tensor("tbl", [VT, RW], BF16, addr_space="Shared")

    ctx = ExitStack()
    sb = lambda name, shape, dt: ctx.enter_context(nc.sbuf_tensor(name, shape, dt))
    xT_sb = sb("xT_sb", [128, DC, VPF], BF16)
    W_sb = sb("W_sb", [128, H_, DC, O_], BF16)
    a_sb = sb("a_sb", [128, H_], BF16)
    b_sb = sb("b_sb", [H_, O_], BF16)
    eye_sb = sb("eye_sb", [128, 128], BF16)
    blk_sb = sb("blk_sb", [128, 248], BF16)
    idx_sb = sb("idx_sb", [128, NCH * 16], I32)
    ones1 = sb("ones1", [1, 128], BF16)
    one11 = sb("one11", [1, 1], BF16)
    ones4 = sb("ones4", [H_, O_], BF16)
    zero_sb = sb("zero_sb", [cfg.ZPAD, RW], BF16)
    tB_sb = [sb(f"tB_sb{i}", [128, VBLK], BF16) for i in range(2)]
    sB_sb = [sb(f"sB_sb{i}", [128, VBLK], BF16) for i in range(2)]
    w_row = [sb(f"w_row{i}", [1, VBLK], BF16) for i in range(2)]
    stage = [sb(f"stage{i}", [128, NJ, RW], BF16) for i in range(2)]
    binit_sb = sb("binit_sb", [128, O_], F32)
    gtile = [sb(f"gtile{i}", [128, 16, RW], BF16) for i in range(2)]
    den_sb = [sb(f"den{i}", [128, H_], F32) for i in range(2)]
    rcp_sb = [sb(f"rcp{i}", [128, H_], F32) for i in range(2)]
    acc_sb = [sb(f"acc{i}", [128, O_], F32) for i in range(2)]
    ostage = [sb(f"ostage{i}", [128, O_], F32) for i in range(2)]

    # phase-1 PSUM (freed before phase-2 allocs; full 2KB banks each)
    ph1 = ExitStack()
    psa = lambda name, shape, dt: ph1.enter_context(nc.psum_tensor(name, shape, dt))
    ps_tB = [psa(f"ps_tB{i}", [128, 512], F32) for i in range(2)]
    ps_wbc = [psa(f"ps_wbc{i}", [128, 512], F32) for i in range(2)]
    ps_misc = [psa(f"ps_misc{i}", [128, 512], F32) for i in range(2)]
    ps_C = [psa(f"ps_C{i}", [128, 1024], BF16) for i in range(2)]
    ph1.close()
    ph2 = ExitStack()
    ps_A = [ph2.enter_context(nc.psum_tensor(f"ps_A{i}", [128, 512], F32))
            for i in range(2)]
    ps_B2 = [ph2.enter_context(nc.psum_tensor(f"ps_B2{i}", [128, 512], F32))
             for i in range(2)]
    ph2.close()

    blk_ndma = []
    for bN in range(NBLK):
        lo, hi = bN * VBLK, min(VP, bN * VBLK + VBLK)
        blk_ndma.append((1 if (hi - lo) // 128 else 0) + (1 if (hi - lo) % 128 else 0))
    cum_stdma = np.cumsum(blk_ndma).tolist()

    sctx = ExitStack()
    sem = lambda n: sctx.enter_context(nc.semaphore(n))
    (s_in, s_idx, s_dvi, s_bini, s_bcp, s_tb, s_tbsb, s_attn, s_w, s_sb,
     s_pw, s_pc, s_st, s_zr, s_cc, s_mm, s_ep, s_rel) = [sem(n) for n in (
        "s_in", "s_idx", "s_dvi", "s_bini", "s_bcp", "s_tb", "s_tbsb",
        "s_attn", "s_w", "s_sb", "s_pw", "s_pc", "s_st", "s_zr",
        "s_cc", "s_mm", "s_ep", "s_rel")]
    s_g = [sem("s_g0"), sem("s_g1")]
    s_sd = [sem("s_sd0"), sem("s_sd1")]
    s_o = [sem("s_o0"), sem("s_o1")]
    # cumulative stage-DMA counts per parity (for safe per-parity waits)
    cum_par = []
    c0 = c1 = 0
    for bN in range(NBLK):
        if bN % 2 == 0:
            c0 += blk_ndma[bN]
        else:
            c1 += blk_ndma[bN]
        cum_par.append((c0, c1))
    with nc.Block() as block:
        @block.gpsimd
        def _(g):
            g.dma_start(out=xT_sb[:, :, 0:VP], in_=bass.AP(
                xT, 0, [[VP, 128], [128 * VP, DC], [1, VP]])).then_inc(s_in, 16)
            g.dma_start(out=W_sb[:, :, :, :], in_=bass.AP(
                Wp, 0, [[O_, 128], [cfg.D * O_, H_], [128 * O_, DC], [1, O_]]
            )).then_inc(s_in, 16)
            with nc.allow_non_contiguous_dma(reason="tiny transposed a load"):
                g.dma_start(out=a_sb[:, :], in_=bass.AP(
                    ap_, 0, [[1, 128], [O_, H_]])).then_inc(s_in, 16)
            g.dma_start(out=b_sb[:, :], in_=bass.AP(
                bp, 0, [[O_, H_], [1, O_]])).then_inc(s_in, 16)
            g.dma_start(out=eye_sb[:, :], in_=eyep[:, :]).then_inc(s_in, 16)
            g.dma_start(out=blk_sb[:, :], in_=blkp[:, :]).then_inc(s_in, 16)
            g.wait_ge(s_dvi, 1)
            g.dma_start(out=bass.AP(tbl_loc, VP * RW, [[RW, cfg.ZPAD], [1, RW]]),
                        in_=zero_sb[:, :]).then_inc(s_zr, 16)
            g.wait_ge(s_zr, 16)
            if only != "p2":
                g.wait_ge(s_sd[0], 16 * cum_par[-1][0])
                if cum_par[-1][1]:
                    g.wait_ge(s_sd[1], 16 * cum_par[-1][1])
            if skip_ag:
                g.dma_start(out=bass.AP(tbl, 0, [[RW, VPAD], [1, RW]]),
                            in_=tbl_loc[:, :]).then_inc(s_cc, 16)
                g.wait_ge(s_cc, 16)
            else:
                g.collective_compute(
                    "AllGather", mybir.AluOpType.bypass,
                    replica_groups=[list(range(cfg.ncores))],
                    ins=[tbl_loc[:, :]], outs=[tbl[:, :]],
                ).then_inc(s_cc)
                g.wait_ge(s_cc, 1)
            g.wait_ge(s_idx, 16)
            if debug:
                g.dma_start(out=dbg_t0[:, :], in_=bass.AP(
                    tbl, 0, [[RW, 128], [1, RW]])).then_inc(s_zr, 16)
                g.dma_start(out=dbg_t1[:, :], in_=bass.AP(
                    tbl, VPAD * RW, [[RW, 128], [1, RW]])).then_inc(s_zr, 16)
            for c in (range(NCH) if only not in ("p1", "p2mmn") else []):
                if c >= 2 and only != "p2g":
                    g.wait_ge(s_mm, c - 1)
                for j in range(16):
                    g.indirect_dma_start(
                        out=gtile[c % 2][:, j, :],
                        out_offset=None,
                        in_=tbl[:, :],
                        in_offset=bass.IndirectOffsetOnAxis(
                            ap=idx_sb[:, c * 16 + j:c * 16 + j + 1], axis=0),
                    ).then_inc(s_g[c % 2], 16)

        @block.tensor
        def _(pe):
            pe.wait_ge(s_in, 96)
            pe.wait_ge(s_dvi, 1)
            pe.matmul(ps_misc[0][:, 0:O_], ones4[:, :], b_sb[:, :],
                      start=True, stop=True).then_inc(s_bini, 1)
            for it in (range(NIT) if only != "p2" else []):
                bN, h = divmod(it, H_)
                vlo = bN * VBLK
                if it >= 2:
                    pe.wait_ge(s_tbsb, it - 1)   # ps_tB[it%2] free (ACT copied)
                mm = pe.matmul(ps_tB[it % 2][:, 0:VBLK], W_sb[:, h, 0, :],
                               xT_sb[:, 0, vlo:vlo + VBLK],
                               start=True, stop=(DC == 1))
                for c in range(1, DC):
                    mm = pe.matmul(ps_tB[it % 2][:, 0:VBLK], W_sb[:, h, c, :],
                                   xT_sb[:, c, vlo:vlo + VBLK],
                                   start=False, stop=(c == DC - 1))
                mm.then_inc(s_tb, 1)
                pe.wait_ge(s_tbsb, it + 1)       # tB_sb[it%2] ready
                if it == 0:
                    pe.wait_ge(s_bcp, 1)         # binit copied out of ps_misc[0]
                if it >= 2:
                    pe.wait_ge(s_st, it - 1)     # ps_misc/ps_C[it%2] free (DVE)
                pe.matmul(ps_misc[it % 2][0:1, 0:VBLK], a_sb[:, h:h + 1],
                          tB_sb[it % 2][:, :], start=True, stop=True
                          ).then_inc(s_attn, 1)
                pe.wait_ge(s_w, it + 1)          # w_row[it%2] ready
                if it >= 2:
                    pe.wait_ge(s_sb, it - 1)     # ps_wbc[it%2] free (DVE)
                pe.matmul(ps_wbc[it % 2][:, 0:VBLK], ones1[:, :],
                          w_row[it % 2][:, :], start=True, stop=True)
                for j in range(NJ):
                    e = pe.matmul(ps_misc[it % 2][:, j:j + 1],
                                  w_row[it % 2][0:1, j * 128:(j + 1) * 128],
                                  one11[:, :], start=True, stop=True)
                e.then_inc(s_pw, 1)
                pe.wait_ge(s_sb, it + 1)         # sB_sb[it%2] ready
                for j in range(NJ):
                    e = pe.transpose(ps_C[it % 2][:, j * 128:(j + 1) * 128],
                                     sB_sb[it % 2][:, j * 128:(j + 1) * 128],
                                     eye_sb[:, :])
                e.then_inc(s_pc, 1)
            for c in (range(NCH) if only not in ("p1", "p2g") else []):
                if only != "p2mmn":
                    pe.wait_ge(s_g[c % 2], 256 * (c // 2 + 1))
                if c >= 2 and only not in ("p2mm", "p2mmn"):
                    pe.wait_ge(s_ep, c - 1)      # ps_A/ps_B2[c%2] free (DVE)
                for gi in range(16):
                    pe.matmul(ps_A[c % 2][:, 0:SW],
                              blk_sb[:, 120 - 8 * gi:248 - 8 * gi],
                              gtile[c % 2][:, gi, 0:SW],
                              start=(gi == 0), stop=(gi == 15))
                for gi in range(16):
                    e = pe.matmul(ps_B2[c % 2][:, 0:H_],
                                  blk_sb[:, 120 - 8 * gi:248 - 8 * gi],
                                  gtile[c % 2][:, gi, SW:SW + H_],
                                  start=(gi == 0), stop=(gi == 15))
                e.then_inc(s_mm, 1)

        @block.scalar
        def _(s):
            for it in (range(NIT) if only != "p2" else []):
                s.wait_ge(s_tb, it + 1)
                if it >= 2:
                    s.wait_ge(s_sb, it - 1)      # tB_sb[it%2] free (DVE read)
                s.activation(tB_sb[it % 2][:, :], ps_tB[it % 2][:, 0:VBLK],
                             mybir.ActivationFunctionType.Copy).then_inc(s_tbsb, 1)
                s.wait_ge(s_attn, it + 1)
                if it >= 2:
                    s.wait_ge(s_pw, it - 1)      # w_row[it%2] free (PE read)
                s.activation(w_row[it % 2][:, :], ps_misc[it % 2][0:1, 0:VBLK],
                             mybir.ActivationFunctionType.Exp).then_inc(s_w, 1)
            for c in (range(NCH) if only not in ("p1", "p2mm", "p2g", "p2mmn") else []):
                s.wait_ge(s_ep, c + 1)
                if c >= 2:
                    s.wait_ge(s_o[c % 2], 16 * (c // 2))  # ostage free
                s.activation(ostage[c % 2][:, :], acc_sb[c % 2][:, :],
                             mybir.ActivationFunctionType.Relu,
                             scale=1.0 / H_).then_inc(s_rel, 1)

        @block.vector
        def _(v):
            v.memset(ones1[:, :], 1.0)
            v.memset(one11[:, :], 1.0)
            v.memset(ones4[:, :], 1.0)
            v.memset(stage[0][:, :, SW + H_:RW], 0.0)
            v.memset(stage[1][:, :, SW + H_:RW], 0.0)
            if VPF > VP:
                v.memset(xT_sb[:, :, VP:VPF], 0.0)
            v.memset(zero_sb[:, :], 0.0).then_inc(s_dvi, 1)
            v.wait_ge(s_bini, 1)
            v.tensor_copy(binit_sb[:, :], ps_misc[0][:, 0:O_]).then_inc(s_bcp, 1)
            for it in (range(NIT) if only != "p2" else []):
                bN, h = divmod(it, H_)
                v.wait_ge(s_pw, it + 1)          # ps_wbc ready (+ tB_sb ready)
                if it >= 2:
                    v.wait_ge(s_pc, it - 1)      # sB_sb[it%2] free (PE read)
                v.tensor_tensor(sB_sb[it % 2][:, :], tB_sb[it % 2][:, :],
                                ps_wbc[it % 2][:, 0:VBLK],
                                mybir.AluOpType.mult).then_inc(s_sb, 1)
                v.wait_ge(s_pc, it + 1)          # ps_C ready
                if bN >= 2 and h == 0:
                    v.wait_ge(s_sd[bN % 2], 16 * cum_par[bN - 2][bN % 2])
                v.tensor_copy(
                    stage[bN % 2][:, :, h * O_:(h + 1) * O_],
                    bass.AP(ps_C[it % 2], 0, [[1024, 128], [128, NJ], [1, 128]]))
                v.tensor_copy(
                    stage[bN % 2][:, :, SW + h:SW + h + 1],
                    bass.AP(ps_misc[it % 2], 0, [[512, 128], [1, NJ], [1, 1]])
                ).then_inc(s_st, 1)
            for c in (range(NCH) if only not in ("p1", "p2mm", "p2g", "p2mmn") else []):
                v.wait_ge(s_mm, c + 1)
                if c >= 2:
                    v.wait_ge(s_rel, c - 1)      # acc_sb[c%2] free (ACT read)
                v.tensor_scalar_max(den_sb[c % 2][:, :], ps_B2[c % 2][:, 0:H_],
                                    1e-30)
                v.drain()
                v.reciprocal(rcp_sb[c % 2][:, :], den_sb[c % 2][:, :])
                v.drain()
                v.scalar_tensor_tensor(
                    acc_sb[c % 2][:, :], ps_A[c % 2][:, 0:O_],
                    rcp_sb[c % 2][:, 0:1], binit_sb[:, :],
                    mybir.AluOpType.mult, mybir.AluOpType.add)
                for h in range(1, H_):
                    v.drain()
                    e = v.scalar_tensor_tensor(
                        acc_sb[c % 2][:, :], ps_A[c % 2][:, h * O_:(h + 1) * O_],
                        rcp_sb[c % 2][:, h:h + 1], acc_sb[c % 2][:, :],
                        mybir.AluOpType.mult, mybir.AluOpType.add)
                e.then_inc(s_ep, 1)

        @block.sync
        def _(sy):
            sy.dma_start(out=idx_sb[:, :], in_=idxp[:, :]).then_inc(s_idx, 16)
            for bN in (range(NBLK) if only != "p2" else []):
                sy.wait_ge(s_st, H_ * (bN + 1))
                lo, hi = bN * VBLK, min(VP, bN * VBLK + VBLK)
                full_j, rem = (hi - lo) // 128, (hi - lo) % 128
                if full_j:
                    sy.dma_start(
                        out=bass.AP(tbl_loc, lo * RW,
                                    [[RW, 128], [128 * RW, full_j], [1, RW]]),
                        in_=stage[bN % 2][:, 0:full_j, :]).then_inc(s_sd[bN % 2], 16)
                if rem:
                    sy.dma_start(
                        out=bass.AP(tbl_loc, (lo + full_j * 128) * RW,
                                    [[RW, rem], [1, RW]]),
                        in_=stage[bN % 2][0:rem, full_j, :]).then_inc(s_sd[bN % 2], 16)
            if debug:
                sy.wait_ge(s_g[0], 16)
                sy.dma_start(out=dbg_gt[:, :, :],
                             in_=gtile[0][:, :, :]).then_inc(s_zr, 16)
            for c in (range(NCH) if only not in ("p1", "p2mm", "p2g", "p2mmn") else []):
                sy.wait_ge(s_rel, c + 1)
                lo = c * 128
                rows = min(128, VP - lo)
                sy.dma_start(out=bass.AP(out_ext, lo * O_, [[O_, rows], [1, O_]]),
                             in_=ostage[c % 2][0:rows, :]).then_inc(s_o[c % 2], 16)
            if only not in ("p1", "p2mm", "p2g", "p2mmn"):
                sy.wait_ge(s_o[0], 16 * ((NCH + 1) // 2))
                if NCH > 1:
                    sy.wait_ge(s_o[1], 16 * (NCH // 2))

    sctx.close()
    ctx.close()
    nc.compile()
    return nc


def prep_core_inputs(cfg: Cfg, x, W, a, b, adj_lst, r):
    """Host-side shard/layout prep for core r (index/layout only, no math)."""
    VP, VPAD, NCH = cfg.VP, cfg.VPAD, cfg.NCHUNK
    xs = np.ascontiguousarray(x[r * VP:(r + 1) * VP].T)
    adj = adj_lst[r * VP:(r + 1) * VP]
    rows = np.where(adj == cfg.V, VP,
                    (adj // VP) * VPAD + (adj % VP)).astype(np.int32)
    rows_p = np.full((NCH * 128, cfg.K), VP, np.int32)
    rows_p[:VP] = rows
    # gtile[p, g, :] = tbl[idx[p, c*16+g]]; edge (node 8g+i, k) at p=16i+k
    lin = rows_p.reshape(NCH, 16, 128)           # [c, g, p] (p = i*16 + k)
    idx = np.ascontiguousarray(
        lin.transpose(2, 0, 1).reshape(128, NCH * 16).astype(np.int32))
    blk = np.zeros((128, 248), np.float32)
    blk[np.arange(128), 120 + np.arange(128) // 16] = 1.0
    return {
        "xT": xs.astype(np.float32), "W": np.asarray(W, np.float32),
        "a": np.asarray(a, np.float32), "b": np.asarray(b, np.float32),
        "eye": np.eye(128, dtype=np.float32), "blk": blk, "idx": idx,
    }


_GRAPH_CACHE = {}


def kernel(x, W, a, b, adj_lst, mask_index, _cfg=None, _trace=False):
    cfg = _cfg or Cfg()
    x = np.asarray(x)
    adj_lst = np.asarray(adj_lst)
    assert int(mask_index) == cfg.V
    key = (cfg.V, cfg.ncores)
    if key not in _GRAPH_CACHE:
        _GRAPH_CACHE[key] = build_graph(cfg)
    nc = _GRAPH_CACHE[key]
    in_maps = [prep_core_inputs(cfg, x, W, a, b, adj_lst, r)
               for r in range(cfg.ncores)]
    res = run_bass_kernel_spmd(nc, in_maps, list(range(cfg.ncores)),
                               trace=_trace)
    out = np.concatenate([res.results[r]["out"] for r in range(cfg.ncores)], 0)
    kernel._last_exec_ns = res.exec_time_ns
    return out



# revision 9
# speedup vs baseline: 1.1043x; 1.1043x over previous
"""Distributed GAT (nn_AdjGAT) kernel for 8 TRN2 NeuronCores.

Math: the reference's per-edge softmax logit depends ONLY on the source
node (scores[h,v,k] = attn[h, adj[v,k]]), so
    softmax_k -> w[n_k] / sum_j w[n_j],  w = exp(attn)
    head[h,v] = (sum_k w[h,n_k] * t[h,n_k]) / (sum_k w[h,n_k])
    out = relu(mean_h(head + b[h]))
Each core computes per-node rows [w*t_0..w*t_3 | w_0..w_3] (bf16, padded
to 640 elems = 1280B), AllGathers the row table, then per 128-node chunk:
one dma_gather of 2048 neighbor rows + block-diagonal-ones matmuls that
sum each node's K=16 rows (numerator AND denominator in one pass), then
normalize/relu.  Padding slots point at a hardwired zero row (w=0).
"""

import math
from contextlib import ExitStack

import numpy as np

import concourse.bass as bass
import concourse.bacc as bacc
import concourse.mybir as mybir
from concourse import library_config
from concourse.bass_utils import run_bass_kernel_spmd

F32 = mybir.dt.float32
BF16 = mybir.dt.bfloat16
I16 = mybir.dt.int16
I32 = mybir.dt.int32

V, D, K, O, H = 20000, 256, 16, 128, 4
NCORES = 8


class Cfg:
    def __init__(self, V=V, D=D, K=K, O=O, H=H, ncores=NCORES):
        assert V % ncores == 0
        self.V, self.D, self.K, self.O, self.H, self.ncores = V, D, K, O, H, ncores
        self.VP = V // ncores
        self.ZPAD = 12
        self.VPAD = self.VP + self.ZPAD
        self.VT = self.VPAD * ncores
        self.RW = H * O + 128          # 640 bf16 elems: [s (512) | w (4) | pad]
        self.NCHUNK = math.ceil(self.VP / 128)
        self.VBLK = 512 if self.VP >= 512 else 128 * math.ceil(self.VP / 128)
        self.NBLK = math.ceil(self.VP / self.VBLK)
        self.NJ = self.VBLK // 128
        self.VPF = self.NBLK * self.VBLK
        self.DC = D // 128
        assert D % 128 == 0 and O == 128 and K == 16 and self.DC >= 1
        assert self.VT < 32000


def build_graph(cfg: Cfg, skip_ag=False, only=None, debug=False):
    nc = bacc.Bacc()
    VP, VPAD, VT, RW, H_, O_, DC = (
        cfg.VP, cfg.VPAD, cfg.VT, cfg.RW, cfg.H, cfg.O, cfg.DC)
    NCH, NBLK, VBLK, NJ, VPF = cfg.NCHUNK, cfg.NBLK, cfg.VBLK, cfg.NJ, cfg.VPF
    SW = H_ * O_
    NE = 128 * 16
    NIT = NBLK * H_

    xT = nc.declare_dram_parameter("xT", [cfg.D, VP], F32, isOutput=False)
    Wp = nc.declare_dram_parameter("W", [H_, cfg.D, O_], F32, isOutput=False)
    ap_ = nc.declare_dram_parameter("a", [H_, O_], F32, isOutput=False)
    bp = nc.declare_dram_parameter("b", [H_, O_], F32, isOutput=False)
    eyep = nc.declare_dram_parameter("eye", [128, 128], F32, isOutput=False)
    blkp = nc.declare_dram_parameter("blk", [128, 248], F32, isOutput=False)
    idxp = nc.declare_dram_parameter("idx", [128, NCH * 128], I16, isOutput=False)
    out_ext = nc.declare_dram_parameter("out", [VP, O_], F32, isOutput=True)
    if debug:
        dbg_t0 = nc.declare_dram_parameter("dbg_t0", [128, RW], BF16, isOutput=True)
        dbg_t1 = nc.declare_dram_parameter("dbg_t1", [128, RW], BF16, isOutput=True)
        dbg_gt = nc.declare_dram_parameter("dbg_gt", [128, 16, RW], BF16,
                                           isOutput=True)

    tbl_loc = nc.dram_tensor("tbl_loc", [VPAD, RW], BF16)
    tbl = nc.dram_tensor("tbl", [VT, RW], BF16, addr_space="Shared")

    ctx = ExitStack()
    sb = lambda name, shape, dt: ctx.enter_context(nc.sbuf_tensor(name, shape, dt))
    xT_sb = sb("xT_sb", [128, DC, VPF], BF16)
    W_sb = sb("W_sb", [128, H_, DC, O_], BF16)
    a_sb = sb("a_sb", [128, H_], BF16)
    b_sb = sb("b_sb", [H_, O_], BF16)
    eye_sb = sb("eye_sb", [128, 128], BF16)
    blk_sb = sb("blk_sb", [128, 248], BF16)
    idx_sb = sb("idx_sb", [128, NCH * 128], I16)
    ones1 = sb("ones1", [1, 128], BF16)
    one11 = sb("one11", [1, 1], BF16)
    ones4 = sb("ones4", [H_, O_], BF16)
    zero_sb = sb("zero_sb", [cfg.ZPAD, RW], BF16)
    tB_sb = [sb(f"tB_sb{i}", [128, VBLK], BF16) for i in range(2)]
    sB_sb = [sb(f"sB_sb{i}", [128, VBLK], BF16) for i in range(2)]
    w_row = [sb(f"w_row{i}", [1, VBLK], BF16) for i in range(2)]
    stage = [sb(f"stage{i}", [128, NJ, RW], BF16) for i in range(2)]
    binit_sb = sb("binit_sb", [128, O_], F32)
    gtile = [sb(f"gtile{i}", [128, 16, RW], BF16) for i in range(2)]
    den_sb = [sb(f"den{i}", [128, H_], F32) for i in range(2)]
    rcp_sb = [sb(f"rcp{i}", [128, H_], F32) for i in range(2)]
    acc_sb = [sb(f"acc{i}", [128, O_], F32) for i in range(2)]
    ostage = [sb(f"ostage{i}", [128, O_], F32) for i in range(2)]

    # phase-1 PSUM (freed before phase-2 allocs; full 2KB banks each)
    ph1 = ExitStack()
    psa = lambda name, shape, dt: ph1.enter_context(nc.psum_tensor(name, shape, dt))
    ps_tB = [psa(f"ps_tB{i}", [128, 512], F32) for i in range(2)]
    ps_wbc = [psa(f"ps_wbc{i}", [128, 512], F32) for i in range(2)]
    ps_misc = [psa(f"ps_misc{i}", [128, 512], F32) for i in range(2)]
    ps_C = [psa(f"ps_C{i}", [128, 1024], BF16) for i in range(2)]
    ph1.close()
    ph2 = ExitStack()
    ps_A = [ph2.enter_context(nc.psum_tensor(f"ps_A{i}", [128, 512], F32))
            for i in range(2)]
    ps_B2 = [ph2.enter_context(nc.psum_tensor(f"ps_B2{i}", [128, 512], F32))
             for i in range(2)]
    ph2.close()

    blk_ndma = []
    for bN in range(NBLK):
        lo, hi = bN * VBLK, min(VP, bN * VBLK + VBLK)
        blk_ndma.append((1 if (hi - lo) // 128 else 0) + (1 if (hi - lo) % 128 else 0))
    cum_stdma = np.cumsum(blk_ndma).tolist()

    sctx = ExitStack()
    sem = lambda n: sctx.enter_context(nc.semaphore(n))
    (s_in, s_idx, s_dvi, s_bini, s_bcp, s_tb, s_tbsb, s_attn, s_w, s_sb,
     s_pw, s_pc, s_st, s_zr, s_cc, s_mm, s_ep, s_rel) = [sem(n) for n in (
        "s_in", "s_idx", "s_dvi", "s_bini", "s_bcp", "s_tb", "s_tbsb",
        "s_attn", "s_w", "s_sb", "s_pw", "s_pc", "s_st", "s_zr",
        "s_cc", "s_mm", "s_ep", "s_rel")]
    s_g = [sem("s_g0"), sem("s_g1")]
    s_sd = [sem("s_sd0"), sem("s_sd1")]
    s_o = [sem("s_o0"), sem("s_o1")]
    # cumulative stage-DMA counts per parity (for safe per-parity waits)
    cum_par = []
    c0 = c1 = 0
    for bN in range(NBLK):
        if bN % 2 == 0:
            c0 += blk_ndma[bN]
        else:
            c1 += blk_ndma[bN]
        cum_par.append((c0, c1))
    with nc.Block() as block:
        @block.gpsimd
        def _(g):
            g.dma_start(out=xT_sb[:, :, 0:VP], in_=bass.AP(
                xT, 0, [[VP, 128], [128 * VP, DC], [1, VP]])).then_inc(s_in, 16)
            g.dma_start(out=W_sb[:, :, :, :], in_=bass.AP(
                Wp, 0, [[O_, 128], [cfg.D * O_, H_], [128 * O_, DC], [1, O_]]
            )).then_inc(s_in, 16)
            with nc.allow_non_contiguous_dma(reason="tiny transposed a load"):
                g.dma_start(out=a_sb[:, :], in_=bass.AP(
                    ap_, 0, [[1, 128], [O_, H_]])).then_inc(s_in, 16)
            g.dma_start(out=b_sb[:, :], in_=bass.AP(
                bp, 0, [[O_, H_], [1, O_]])).then_inc(s_in, 16)
            g.dma_start(out=eye_sb[:, :], in_=eyep[:, :]).then_inc(s_in, 16)
            g.dma_start(out=blk_sb[:, :], in_=blkp[:, :]).then_inc(s_in, 16)
            g.wait_ge(s_dvi, 1)
            g.dma_start(out=bass.AP(tbl_loc, VP * RW, [[RW, cfg.ZPAD], [1, RW]]),
                        in_=zero_sb[:, :]).then_inc(s_zr, 16)
            g.wait_ge(s_zr, 16)
            if only != "p2":
                g.wait_ge(s_sd[0], 16 * cum_par[-1][0])
                if cum_par[-1][1]:
                    g.wait_ge(s_sd[1], 16 * cum_par[-1][1])
            if skip_ag:
                g.dma_start(out=bass.AP(tbl, 0, [[RW, VPAD], [1, RW]]),
                            in_=tbl_loc[:, :]).then_inc(s_cc, 16)
                g.wait_ge(s_cc, 16)
            else:
                g.collective_compute(
                    "AllGather", mybir.AluOpType.bypass,
                    replica_groups=[list(range(cfg.ncores))],
                    ins=[tbl_loc[:, :]], outs=[tbl[:, :]],
                ).then_inc(s_cc)
                g.wait_ge(s_cc, 1)
            g.wait_ge(s_idx, 16)
            if debug:
                g.dma_start(out=dbg_t0[:, :], in_=bass.AP(
                    tbl, 0, [[RW, 128], [1, RW]])).then_inc(s_zr, 16)
                g.dma_start(out=dbg_t1[:, :], in_=bass.AP(
                    tbl, VPAD * RW, [[RW, 128], [1, RW]])).then_inc(s_zr, 16)
            for c in (range(NCH) if only not in ("p1", "p2mmn") else []):
                if c >= 2 and only != "p2g":
                    g.wait_ge(s_mm, c - 1)
                for hf in range(2):
                    g.dma_gather(
                        out_ap=gtile[c % 2][:, hf * 8:hf * 8 + 8, :],
                        in_ap=tbl[:, :],
                        idxs_ap=idx_sb[:, c * 128 + hf * 64:c * 128 + hf * 64 + 64],
                        num_idxs=NE // 2,
                        num_idxs_reg=NE // 2,
                        elem_size=RW,
                    ).then_inc(s_g[c % 2], 16)

        @block.tensor
        def _(pe):
            pe.wait_ge(s_in, 96)
            pe.wait_ge(s_dvi, 1)
            pe.matmul(ps_misc[0][:, 0:O_], ones4[:, :], b_sb[:, :],
                      start=True, stop=True).then_inc(s_bini, 1)
            for it in (range(NIT) if only != "p2" else []):
                bN, h = divmod(it, H_)
                vlo = bN * VBLK
                if it >= 2:
                    pe.wait_ge(s_tbsb, it - 1)   # ps_tB[it%2] free (ACT copied)
                mm = pe.matmul(ps_tB[it % 2][:, 0:VBLK], W_sb[:, h, 0, :],
                               xT_sb[:, 0, vlo:vlo + VBLK],
                               start=True, stop=(DC == 1))
                for c in range(1, DC):
                    mm = pe.matmul(ps_tB[it % 2][:, 0:VBLK], W_sb[:, h, c, :],
                                   xT_sb[:, c, vlo:vlo + VBLK],
                                   start=False, stop=(c == DC - 1))
                mm.then_inc(s_tb, 1)
                pe.wait_ge(s_tbsb, it + 1)       # tB_sb[it%2] ready
                if it == 0:
                    pe.wait_ge(s_bcp, 1)         # binit copied out of ps_misc[0]
                if it >= 2:
                    pe.wait_ge(s_st, it - 1)     # ps_misc/ps_C[it%2] free (DVE)
                pe.matmul(ps_misc[it % 2][0:1, 0:VBLK], a_sb[:, h:h + 1],
                          tB_sb[it % 2][:, :], start=True, stop=True
                          ).then_inc(s_attn, 1)
                pe.wait_ge(s_w, it + 1)          # w_row[it%2] ready
                if it >= 2:
                    pe.wait_ge(s_sb, it - 1)     # ps_wbc[it%2] free (DVE)
                pe.matmul(ps_wbc[it % 2][:, 0:VBLK], ones1[:, :],
                          w_row[it % 2][:, :], start=True, stop=True)
                for j in range(NJ):
                    e = pe.matmul(ps_misc[it % 2][:, j:j + 1],
                                  w_row[it % 2][0:1, j * 128:(j + 1) * 128],
                                  one11[:, :], start=True, stop=True)
                e.then_inc(s_pw, 1)
                pe.wait_ge(s_sb, it + 1)         # sB_sb[it%2] ready
                for j in range(NJ):
                    e = pe.transpose(ps_C[it % 2][:, j * 128:(j + 1) * 128],
                                     sB_sb[it % 2][:, j * 128:(j + 1) * 128],
                                     eye_sb[:, :])
                e.then_inc(s_pc, 1)
            for c in (range(NCH) if only not in ("p1", "p2g") else []):
                if only != "p2mmn":
                    pe.wait_ge(s_g[c % 2], 32 * (c // 2 + 1))
                if c >= 2 and only not in ("p2mm", "p2mmn"):
                    pe.wait_ge(s_ep, c - 1)      # ps_A/ps_B2[c%2] free (DVE)
                for gi in range(16):
                    pe.matmul(ps_A[c % 2][:, 0:SW],
                              blk_sb[:, 120 - 8 * gi:248 - 8 * gi],
                              gtile[c % 2][:, gi, 0:SW],
                              start=(gi == 0), stop=(gi == 15))
                for gi in range(16):
                    e = pe.matmul(ps_B2[c % 2][:, 0:H_],
                                  blk_sb[:, 120 - 8 * gi:248 - 8 * gi],
                                  gtile[c % 2][:, gi, SW:SW + H_],
                                  start=(gi == 0), stop=(gi == 15))
                e.then_inc(s_mm, 1)

        @block.scalar
        def _(s):
            for it in (range(NIT) if only != "p2" else []):
                s.wait_ge(s_tb, it + 1)
                if it >= 2:
                    s.wait_ge(s_sb, it - 1)      # tB_sb[it%2] free (DVE read)
                s.activation(tB_sb[it % 2][:, :], ps_tB[it % 2][:, 0:VBLK],
                             mybir.ActivationFunctionType.Copy).then_inc(s_tbsb, 1)
                s.wait_ge(s_attn, it + 1)
                if it >= 2:
                    s.wait_ge(s_pw, it - 1)      # w_row[it%2] free (PE read)
                s.activation(w_row[it % 2][:, :], ps_misc[it % 2][0:1, 0:VBLK],
                             mybir.ActivationFunctionType.Exp).then_inc(s_w, 1)
            for c in (range(NCH) if only not in ("p1", "p2mm", "p2g", "p2mmn") else []):
                s.wait_ge(s_ep, c + 1)
                if c >= 2:
                    s.wait_ge(s_o[c % 2], 16 * (c // 2))  # ostage free
                s.activation(ostage[c % 2][:, :], acc_sb[c % 2][:, :],
                             mybir.ActivationFunctionType.Relu,
                             scale=1.0 / H_).then_inc(s_rel, 1)

        @block.vector
        def _(v):
            v.memset(ones1[:, :], 1.0)
            v.memset(one11[:, :], 1.0)
            v.memset(ones4[:, :], 1.0)
            v.memset(stage[0][:, :, SW + H_:RW], 0.0)
            v.memset(stage[1][:, :, SW + H_:RW], 0.0)
            if VPF > VP:
                v.memset(xT_sb[:, :, VP:VPF], 0.0)
            v.memset(zero_sb[:, :], 0.0).then_inc(s_dvi, 1)
            v.wait_ge(s_bini, 1)
            v.tensor_copy(binit_sb[:, :], ps_misc[0][:, 0:O_]).then_inc(s_bcp, 1)
            for it in (range(NIT) if only != "p2" else []):
                bN, h = divmod(it, H_)
                v.wait_ge(s_pw, it + 1)          # ps_wbc ready (+ tB_sb ready)
                if it >= 2:
                    v.wait_ge(s_pc, it - 1)      # sB_sb[it%2] free (PE read)
                v.tensor_tensor(sB_sb[it % 2][:, :], tB_sb[it % 2][:, :],
                                ps_wbc[it % 2][:, 0:VBLK],
                                mybir.AluOpType.mult).then_inc(s_sb, 1)
                v.wait_ge(s_pc, it + 1)          # ps_C ready
                if bN >= 2 and h == 0:
                    v.wait_ge(s_sd[bN % 2], 16 * cum_par[bN - 2][bN % 2])
                v.tensor_copy(
                    stage[bN % 2][:, :, h * O_:(h + 1) * O_],
                    bass.AP(ps_C[it % 2], 0, [[1024, 128], [128, NJ], [1, 128]]))
                v.tensor_copy(
                    stage[bN % 2][:, :, SW + h:SW + h + 1],
                    bass.AP(ps_misc[it % 2], 0, [[512, 128], [1, NJ], [1, 1]])
                ).then_inc(s_st, 1)
            for c in (range(NCH) if only not in ("p1", "p2mm", "p2g", "p2mmn") else []):
                v.wait_ge(s_mm, c + 1)
                if c >= 2:
                    v.wait_ge(s_rel, c - 1)      # acc_sb[c%2] free (ACT read)
                v.tensor_scalar_max(den_sb[c % 2][:, :], ps_B2[c % 2][:, 0:H_],
                                    1e-30)
                v.drain()
                v.reciprocal(rcp_sb[c % 2][:, :], den_sb[c % 2][:, :])
                v.drain()
                v.scalar_tensor_tensor(
                    acc_sb[c % 2][:, :], ps_A[c % 2][:, 0:O_],
                    rcp_sb[c % 2][:, 0:1], binit_sb[:, :],
                    mybir.AluOpType.mult, mybir.AluOpType.add)
                for h in range(1, H_):
                    v.drain()
                    e = v.scalar_tensor_tensor(
                        acc_sb[c % 2][:, :], ps_A[c % 2][:, h * O_:(h + 1) * O_],
                        rcp_sb[c % 2][:, h:h + 1], acc_sb[c % 2][:, :],
                        mybir.AluOpType.mult, mybir.AluOpType.add)
                e.then_inc(s_ep, 1)

        @block.sync
        def _(sy):
            sy.dma_start(out=idx_sb[:, :], in_=idxp[:, :]).then_inc(s_idx, 16)
            for bN in (range(NBLK) if only != "p2" else []):
                sy.wait_ge(s_st, H_ * (bN + 1))
                lo, hi = bN * VBLK, min(VP, bN * VBLK + VBLK)
                full_j, rem = (hi - lo) // 128, (hi - lo) % 128
                if full_j:
                    sy.dma_start(
                        out=bass.AP(tbl_loc, lo * RW,
                                    [[RW, 128], [128 * RW, full_j], [1, RW]]),
                        in_=stage[bN % 2][:, 0:full_j, :]).then_inc(s_sd[bN % 2], 16)
                if rem:
                    sy.dma_start(
                        out=bass.AP(tbl_loc, (lo + full_j * 128) * RW,
                                    [[RW, rem], [1, RW]]),
                        in_=stage[bN % 2][0:rem, full_j, :]).then_inc(s_sd[bN % 2], 16)
            if debug:
                sy.wait_ge(s_g[0], 16)
                sy.dma_start(out=dbg_gt[:, :, :],
                             in_=gtile[0][:, :, :]).then_inc(s_zr, 16)
            for c in (range(NCH) if only not in ("p1", "p2mm", "p2g", "p2mmn") else []):
                sy.wait_ge(s_rel, c + 1)
                lo = c * 128
                rows = min(128, VP - lo)
                sy.dma_start(out=bass.AP(out_ext, lo * O_, [[O_, rows], [1, O_]]),
                             in_=ostage[c % 2][0:rows, :]).then_inc(s_o[c % 2], 16)
            if only not in ("p1", "p2mm", "p2g", "p2mmn"):
                sy.wait_ge(s_o[0], 16 * ((NCH + 1) // 2))
                if NCH > 1:
                    sy.wait_ge(s_o[1], 16 * (NCH // 2))

    sctx.close()
    ctx.close()
    nc.compile()
    return nc


def prep_core_inputs(cfg: Cfg, x, W, a, b, adj_lst, r):
    """Host-side shard/layout prep for core r (index/layout only, no math)."""
    VP, VPAD, NCH = cfg.VP, cfg.VPAD, cfg.NCHUNK
    xs = np.ascontiguousarray(x[r * VP:(r + 1) * VP].T)
    adj = adj_lst[r * VP:(r + 1) * VP]
    rows = np.where(adj == cfg.V, VP,
                    (adj // VP) * VPAD + (adj % VP)).astype(np.int32)
    rows_p = np.full((NCH * 128, cfg.K), VP, np.int32)
    rows_p[:VP] = rows
    # gtile[p, g, :] = tbl[flat[g*128+p]]; edge (node 8g+i, k) at p=16i+k.
    # dma_gather flat order i reads idxs_sb[i%16, i//16] (16-part wrap,
    # replicated to all 8 gpsimd cores).
    lin = rows_p.reshape(NCH, 16, 128)           # [c, g, p] (p = i*16 + k)
    F = lin.reshape(NCH, 2, 64, 16)              # [c, half, s, q]: i = s*16+q
    B = F.transpose(0, 1, 3, 2)                  # [c, half, q, s]
    idx = np.ascontiguousarray(
        np.tile(B, (1, 1, 8, 1)).transpose(2, 0, 1, 3).reshape(128, NCH * 128)
    ).astype(np.int16)
    blk = np.zeros((128, 248), np.float32)
    blk[np.arange(128), 120 + np.arange(128) // 16] = 1.0
    return {
        "xT": xs.astype(np.float32), "W": np.asarray(W, np.float32),
        "a": np.asarray(a, np.float32), "b": np.asarray(b, np.float32),
        "eye": np.eye(128, dtype=np.float32), "blk": blk, "idx": idx,
    }


_GRAPH_CACHE = {}


def kernel(x, W, a, b, adj_lst, mask_index, _cfg=None, _trace=False):
    cfg = _cfg or Cfg()
    x = np.asarray(x)
    adj_lst = np.asarray(adj_lst)
    assert int(mask_index) == cfg.V
    key = (cfg.V, cfg.ncores)
    if key not in _GRAPH_CACHE:
        _GRAPH_CACHE[key] = build_graph(cfg)
    nc = _GRAPH_CACHE[key]
    in_maps = [prep_core_inputs(cfg, x, W, a, b, adj_lst, r)
               for r in range(cfg.ncores)]
    res = run_bass_kernel_spmd(nc, in_maps, list(range(cfg.ncores)),
                               trace=_trace)
    out = np.concatenate([res.results[r]["out"] for r in range(cfg.ncores)], 0)
    kernel._last_exec_ns = res.exec_time_ns
    return out



# revision 40
# speedup vs baseline: 1.1986x; 1.0854x over previous
"""Distributed GAT (nn_AdjGAT) kernel for 8 TRN2 NeuronCores.

Math: the reference's per-edge softmax logit depends ONLY on the source
node (scores[h,v,k] = attn[h, adj[v,k]]), so
    softmax_k -> w[n_k] / sum_j w[n_j],  w = exp(attn)
    head[h,v] = (sum_k w[h,n_k] * t[h,n_k]) / (sum_k w[h,n_k])
    out = relu(mean_h(head + b[h]))
Phase 1 (per core): build node-major table rows [w*t_0..w*t_3 | w_0..w_3]
(bf16, padded to 640 elems = 1280B).  attn is computed independently of t
via a3 = W_h @ a_h (device-precomputed), attn = a3^T x per 512-node block
for all 4 heads at once.  AllGather the row table.
Phase 2: per 128-node chunk, two 1024-row dma_gather ops fetch the 2048
neighbor rows (SWDGE descriptor generation on GpSimd is the bottleneck,
~8ns/row), then block-diagonal-ones matmuls sum each node's K=16 rows
(numerator AND denominator), then normalize/relu.  Padding slots point at
a hardwired zero row (w=0).
"""

import math
from contextlib import ExitStack

import numpy as np

import concourse.bass as bass
import concourse.bacc as bacc
import concourse.mybir as mybir
from concourse import library_config
from concourse.bass_utils import run_bass_kernel_spmd

F32 = mybir.dt.float32
BF16 = mybir.dt.bfloat16
I16 = mybir.dt.int16
I32 = mybir.dt.int32

V, D, K, O, H = 20000, 256, 16, 128, 4
NCORES = 8


class Cfg:
    def __init__(self, V=V, D=D, K=K, O=O, H=H, ncores=NCORES):
        assert V % ncores == 0
        self.V, self.D, self.K, self.O, self.H, self.ncores = V, D, K, O, H, ncores
        self.VP = V // ncores
        self.ZPAD = 12
        self.VPAD = self.VP + self.ZPAD
        self.VT = self.VPAD * ncores
        self.RW = H * O + 128          # 640 bf16 elems: [s (512) | w (4) | pad]
        self.NCHUNK = math.ceil(self.VP / 128)
        self.VBLK = 512 if self.VP >= 512 else 128 * math.ceil(self.VP / 128)
        self.NBLK = math.ceil(self.VP / self.VBLK)
        self.NJ = self.VBLK // 128
        self.VPF = self.NBLK * self.VBLK
        self.DC = D // 128
        assert D % 128 == 0 and O == 128 and K == 16 and self.DC >= 1
        assert self.VT < 32000


def build_graph(cfg: Cfg, skip_ag=False):
    nc = bacc.Bacc()
    VP, VPAD, VT, RW, H_, O_, DC = (
        cfg.VP, cfg.VPAD, cfg.VT, cfg.RW, cfg.H, cfg.O, cfg.DC)
    NCH, NBLK, VBLK, NJ, VPF = cfg.NCHUNK, cfg.NBLK, cfg.VBLK, cfg.NJ, cfg.VPF
    SW = H_ * O_
    NE = 128 * 16
    NIT = NBLK * H_

    xT = nc.declare_dram_parameter("xT", [cfg.D, VP], F32, isOutput=False)
    Wp = nc.declare_dram_parameter("W", [H_, cfg.D, O_], F32, isOutput=False)
    WTp = nc.declare_dram_parameter("WT", [O_, H_, DC, 128], F32, isOutput=False)
    aTp = nc.declare_dram_parameter("aT", [O_, H_], F32, isOutput=False)
    bp = nc.declare_dram_parameter("b", [H_, O_], F32, isOutput=False)
    eyep = nc.declare_dram_parameter("eye", [128, 128], F32, isOutput=False)
    selp = nc.declare_dram_parameter("sel", [H_, H_ * 128], F32, isOutput=False)
    blkp = nc.declare_dram_parameter("blk", [128, 248], F32, isOutput=False)
    idxp = nc.declare_dram_parameter("idx", [128, NCH * 128], I16, isOutput=False)
    out_ext = nc.declare_dram_parameter("out", [VP, O_], F32, isOutput=True)

    tbl_loc = nc.dram_tensor("tbl_loc", [VPAD, RW], BF16)
    tbl = nc.dram_tensor("tbl", [VT, RW], BF16, addr_space="Shared")

    ctx = ExitStack()
    sb = lambda name, shape, dt: ctx.enter_context(nc.sbuf_tensor(name, shape, dt))
    xT_sb = sb("xT_sb", [128, DC, VPF], BF16)
    W_sb = sb("W_sb", [128, H_, DC, O_], BF16)
    WT_sb = sb("WT_sb", [128, H_, DC, 128], BF16)
    aT_sb = sb("aT_sb", [128, H_], BF16)
    a3_sb = sb("a3_sb", [128, DC * H_], BF16)
    b_sb = sb("b_sb", [H_, O_], BF16)
    eye_sb = sb("eye_sb", [128, 128], BF16)
    blk_sb = sb("blk_sb", [128, 248], BF16)
    idx_sb = sb("idx_sb", [128, NCH * 128], I16)
    ones4 = sb("ones4", [H_, O_], BF16)
    zero_sb = sb("zero_sb", [cfg.ZPAD, RW], BF16)
    w4 = [sb(f"w4_{i}", [H_, VBLK], BF16) for i in range(2)]
    sel_sb = sb("sel_sb", [H_, H_ * 128], BF16)
    tB_sb = [sb(f"tB_sb{i}", [128, VBLK], BF16) for i in range(2)]
    sB_sb = [sb(f"sB_sb{i}", [128, VBLK], BF16) for i in range(2)]
    stage = [sb(f"stage{i}", [128, NJ, RW], BF16) for i in range(2)]
    binit_sb = sb("binit_sb", [128, O_], F32)
    gtile = [sb(f"gtile{i}", [128, 16, RW], BF16) for i in range(2)]
    den_sb = [sb(f"den{i}", [128, H_], F32) for i in range(2)]
    rcp_sb = [sb(f"rcp{i}", [128, H_], F32) for i in range(2)]
    acc_sb = [sb(f"acc{i}", [128, O_], F32) for i in range(2)]
    ostage = [sb(f"ostage{i}", [128, O_], F32) for i in range(2)]

    # phase-1 PSUM (freed before phase-2 allocs; full 2KB banks each)
    ph1 = ExitStack()
    psa = lambda name, shape, dt: ph1.enter_context(nc.psum_tensor(name, shape, dt))
    ps_tB = [psa(f"ps_tB{i}", [128, 512], F32) for i in range(2)]
    ps_wbc = [psa(f"ps_wbc{i}", [128, 512], F32) for i in range(2)]
    ps_misc = [psa(f"ps_misc{i}", [128, 512], F32) for i in range(2)]
    ps_C = [psa(f"ps_C{i}", [128, 1024], BF16) for i in range(2)]
    ph1.close()
    ph2 = ExitStack()
    ps_A = [ph2.enter_context(nc.psum_tensor(f"ps_A{i}", [128, 512], F32))
            for i in range(2)]
    ps_B2 = [ph2.enter_context(nc.psum_tensor(f"ps_B2{i}", [128, 512], F32))
             for i in range(2)]
    ph2.close()

    blk_ndma = []
    for bN in range(NBLK):
        lo, hi = bN * VBLK, min(VP, bN * VBLK + VBLK)
        blk_ndma.append((1 if (hi - lo) // 128 else 0) + (1 if (hi - lo) % 128 else 0))

    sctx = ExitStack()
    sem = lambda n: sctx.enter_context(nc.semaphore(n))
    (s_ld, s_ld2, s_idx, s_dvi, s_bini, s_bcp, s_a3, s_a3c, s_tb, s_tbsb,
     s_attn, s_w, s_pw, s_sb, s_pc, s_w4t, s_st, s_zr, s_cc, s_mm, s_ep,
     s_rel) = [sem(n) for n in (
        "s_ld", "s_ld2", "s_idx", "s_dvi", "s_bini", "s_bcp", "s_a3",
        "s_a3c", "s_tb", "s_tbsb", "s_attn", "s_w", "s_pw", "s_sb", "s_pc",
        "s_w4t", "s_st", "s_zr", "s_cc", "s_mm", "s_ep", "s_rel")]
    s_xb = [sem(f"s_xb{i}") for i in range(NBLK)]
    s_g = [sem("s_g0"), sem("s_g1")]
    s_sd = [sem("s_sd0"), sem("s_sd1")]
    s_o = [sem("s_o0"), sem("s_o1")]
    # cumulative stage-DMA counts per parity (for safe per-parity waits)
    cum_par = []
    c0 = c1 = 0
    for bN in range(NBLK):
        if bN % 2 == 0:
            c0 += blk_ndma[bN]
        else:
            c1 += blk_ndma[bN]
        cum_par.append((c0, c1))
    with nc.Block() as block:
        @block.gpsimd
        def _(g):
            g.dma_start(out=W_sb[:, :, :, :], in_=bass.AP(
                Wp, 0, [[O_, 128], [cfg.D * O_, H_], [128 * O_, DC], [1, O_]]
            )).then_inc(s_ld, 16)
            g.dma_start(out=WT_sb[:, :, :, :], in_=bass.AP(
                WTp, 0, [[H_ * DC * 128, 128], [DC * 128, H_], [128, DC], [1, 128]]
            )).then_inc(s_ld, 16)
            g.dma_start(out=aT_sb[:, :], in_=bass.AP(
                aTp, 0, [[H_, 128], [1, H_]])).then_inc(s_ld, 16)
            g.dma_start(out=b_sb[:, :], in_=bass.AP(
                bp, 0, [[O_, H_], [1, O_]])).then_inc(s_ld, 16)
            g.dma_start(out=eye_sb[:, :], in_=eyep[:, :]).then_inc(s_ld, 16)
            g.dma_start(out=sel_sb[:, :], in_=selp[:, :]).then_inc(s_ld, 16)
            for bN in range(NBLK):
                lo, hi = bN * VBLK, min(VP, bN * VBLK + VBLK)
                g.dma_start(out=xT_sb[:, :, lo:hi], in_=bass.AP(
                    xT, lo, [[VP, 128], [128 * VP, DC], [1, hi - lo]])
                ).then_inc(s_xb[bN], 16)
            g.dma_start(out=blk_sb[:, :], in_=blkp[:, :]).then_inc(s_ld2, 16)
            g.wait_ge(s_dvi, 1)
            g.dma_start(out=bass.AP(tbl_loc, VP * RW, [[RW, cfg.ZPAD], [1, RW]]),
                        in_=zero_sb[:, :]).then_inc(s_zr, 16)
            g.wait_ge(s_zr, 16)
            g.wait_ge(s_sd[0], 16 * cum_par[-1][0])
            if cum_par[-1][1]:
                g.wait_ge(s_sd[1], 16 * cum_par[-1][1])
            if skip_ag:
                g.dma_start(out=bass.AP(tbl, 0, [[RW, VPAD], [1, RW]]),
                            in_=tbl_loc[:, :]).then_inc(s_cc, 16)
                g.wait_ge(s_cc, 16)
            else:
                g.collective_compute(
                    "AllGather", mybir.AluOpType.bypass,
                    replica_groups=[list(range(cfg.ncores))],
                    ins=[tbl_loc[:, :]], outs=[tbl[:, :]],
                ).then_inc(s_cc)
                g.wait_ge(s_cc, 1)
            g.wait_ge(s_idx, 16)
            for c in range(NCH):
                if c >= 2:
                    g.wait_ge(s_mm, c - 1)
                for hf in range(2):
                    g.dma_gather(
                        out_ap=gtile[c % 2][:, hf * 8:hf * 8 + 8, :],
                        in_ap=tbl[:, :],
                        idxs_ap=idx_sb[:, c * 128 + hf * 64:c * 128 + hf * 64 + 64],
                        num_idxs=NE // 2,
                        num_idxs_reg=NE // 2,
                        elem_size=RW,
                    ).then_inc(s_g[c % 2], 16)

        @block.tensor
        def _(pe):
            def transp(t):
                pe.wait_ge(s_sb, t + 1)          # sB_sb[t%2] ready (DVE done)
                if t >= 2:
                    pe.wait_ge(s_st, t - 1)      # ps_C[t%2] s-cols free (DVE)
                for j in range(NJ):
                    e = pe.transpose(ps_C[t % 2][:, j * 128:(j + 1) * 128],
                                     sB_sb[t % 2][:, j * 128:(j + 1) * 128],
                                     eye_sb[:, :])
                e.then_inc(s_pc, 1)

            pe.wait_ge(s_ld, 96)
            pe.wait_ge(s_dvi, 1)
            pe.matmul(ps_misc[0][:, 0:O_], ones4[:, :], b_sb[:, :],
                      start=True, stop=True).then_inc(s_bini, 1)
            for c in range(DC):
                for h in range(H_):
                    e = pe.matmul(ps_misc[1][:, c * H_ + h:c * H_ + h + 1],
                                  WT_sb[:, h, c, :], aT_sb[:, h:h + 1],
                                  start=True, stop=True)
            e.then_inc(s_a3, 1)
            for it in range(NIT):
                bN, h = divmod(it, H_)
                vlo = bN * VBLK
                if it >= 1:
                    transp(it - 1)
                if h == 0:
                    pe.wait_ge(s_xb[bN], 16)
                    if bN == 0:
                        pe.wait_ge(s_bcp, 1)     # ps_misc[0] binit copied out
                        pe.wait_ge(s_a3c, 1)     # a3_sb ready (+ps_misc[1] free)
                    if bN >= 2:
                        pe.wait_ge(s_w, bN - 1)  # ps_misc[bN%2] free (exp done)
                    if bN >= 1:
                        pe.wait_ge(s_st, 4 * bN)  # pw cols of prev block copied
                    for c in range(DC):
                        mm = pe.matmul(ps_misc[bN % 2][0:4, 0:VBLK],
                                       a3_sb[:, c * H_:(c + 1) * H_],
                                       xT_sb[:, c, vlo:vlo + VBLK],
                                       start=(c == 0), stop=(c == DC - 1))
                    mm.then_inc(s_attn, 1)
                if it >= 2:
                    pe.wait_ge(s_tbsb, it - 1)   # ps_tB[it%2] free (ACT copied)
                mm = pe.matmul(ps_tB[it % 2][:, 0:VBLK], W_sb[:, h, 0, :],
                               xT_sb[:, 0, vlo:vlo + VBLK],
                               start=True, stop=(DC == 1))
                for c in range(1, DC):
                    mm = pe.matmul(ps_tB[it % 2][:, 0:VBLK], W_sb[:, h, c, :],
                                   xT_sb[:, c, vlo:vlo + VBLK],
                                   start=False, stop=(c == DC - 1))
                mm.then_inc(s_tb, 1)
                pe.wait_ge(s_w, bN + 1)          # w4[bN%2] ready (exp done)
                if it >= 2:
                    pe.wait_ge(s_sb, it - 1)     # ps_wbc[it%2] free (DVE)
                    pe.wait_ge(s_st, it - 1)     # ps_misc[it%2] pw cols copied
                pe.matmul(ps_wbc[it % 2][:, 0:VBLK],
                          sel_sb[:, h * 128:(h + 1) * 128],
                          w4[bN % 2][:, :], start=True, stop=True)
                for j in range(NJ):
                    e = pe.matmul(
                        ps_misc[it % 2][:, j:j + 1],
                        w4[bN % 2][:, j * 128:(j + 1) * 128],
                        sel_sb[:, h * 128:h * 128 + 1], start=True, stop=True)
                e.then_inc(s_pw, 1)
            transp(NIT - 1)
            pe.wait_ge(s_ld2, 16)
            for c in range(NCH):
                pe.wait_ge(s_g[c % 2], 32 * (c // 2 + 1))
                if c >= 2:
                    pe.wait_ge(s_ep, c - 1)      # ps_A/ps_B2[c%2] free (DVE)
                for gi in range(16):
                    pe.matmul(ps_A[c % 2][:, 0:SW],
                              blk_sb[:, 120 - 8 * gi:248 - 8 * gi],
                              gtile[c % 2][:, gi, 0:SW],
                              start=(gi == 0), stop=(gi == 15))
                for gi in range(16):
                    e = pe.matmul(ps_B2[c % 2][:, 0:H_],
                                  blk_sb[:, 120 - 8 * gi:248 - 8 * gi],
                                  gtile[c % 2][:, gi, SW:SW + H_],
                                  start=(gi == 0), stop=(gi == 15))
                e.then_inc(s_mm, 1)

        @block.scalar
        def _(s):
            s.wait_ge(s_a3, 1)
            s.activation(a3_sb[:, :], ps_misc[1][:, 0:DC * H_],
                         mybir.ActivationFunctionType.Copy).then_inc(s_a3c, 1)
            for it in range(NIT):
                bN, h = divmod(it, H_)
                if h == 0:
                    s.wait_ge(s_attn, bN + 1)
                    if bN >= 2:
                        s.wait_ge(s_pw, 4 * bN - 4)  # w_all free (wbc/pw read)
                    s.activation(w4[bN % 2][:, :],
                                 ps_misc[bN % 2][0:4, 0:VBLK],
                                 mybir.ActivationFunctionType.Exp
                                 ).then_inc(s_w, 1)
                s.wait_ge(s_tb, it + 1)
                if it >= 2:
                    s.wait_ge(s_sb, it - 1)      # tB_sb[it%2] free (DVE read)
                s.activation(tB_sb[it % 2][:, :], ps_tB[it % 2][:, 0:VBLK],
                             mybir.ActivationFunctionType.Copy).then_inc(s_tbsb, 1)
            for c in range(NCH):
                s.wait_ge(s_ep, c + 1)
                if c >= 2:
                    s.wait_ge(s_o[c % 2], 16 * (c // 2))  # ostage free
                s.activation(ostage[c % 2][:, :], acc_sb[c % 2][:, :],
                             mybir.ActivationFunctionType.Relu,
                             scale=1.0 / H_).then_inc(s_rel, 1)

        @block.vector
        def _(v):
            def copies(t):
                b, h = divmod(t, H_)
                v.wait_ge(s_pc, t + 1)           # ps_C[t%2] s-cols ready
                if h == 0 and b >= 2:
                    v.wait_ge(s_sd[b % 2], 16 * cum_par[b - 2][b % 2])
                v.tensor_copy(
                    stage[b % 2][:, :, h * O_:(h + 1) * O_],
                    bass.AP(ps_C[t % 2], 0, [[1024, 128], [128, NJ], [1, 128]]))
                v.tensor_copy(
                    stage[b % 2][:, :, SW + h:SW + h + 1],
                    bass.AP(ps_misc[t % 2], 0, [[512, 128], [1, NJ], [1, 1]])
                ).then_inc(s_st, 1)

            v.memset(ones4[:, :], 1.0)
            v.memset(stage[0][:, :, SW + H_:RW], 0.0)
            v.memset(stage[1][:, :, SW + H_:RW], 0.0)
            if VPF > VP:
                v.memset(xT_sb[:, :, VP:VPF], 0.0)
            v.memset(zero_sb[:, :], 0.0).then_inc(s_dvi, 1)
            v.wait_ge(s_bini, 1)
            v.tensor_copy(binit_sb[:, :], ps_misc[0][:, 0:O_]).then_inc(s_bcp, 1)
            for it in range(NIT):
                if it >= 1:
                    copies(it - 1)
                v.wait_ge(s_tbsb, it + 1)        # tB_sb[it%2] ready
                v.wait_ge(s_pw, it + 1)          # ps_wbc[it%2] ready
                if it >= 2:
                    v.wait_ge(s_pc, it - 1)      # sB_sb[it%2] free (PE read)
                v.tensor_tensor(sB_sb[it % 2][:, :], tB_sb[it % 2][:, :],
                                ps_wbc[it % 2][:, 0:VBLK],
                                mybir.AluOpType.mult).then_inc(s_sb, 1)
            copies(NIT - 1)
            for c in range(NCH):
                v.wait_ge(s_mm, c + 1)
                if c >= 2:
                    v.wait_ge(s_rel, c - 1)      # acc_sb[c%2] free (ACT read)
                v.tensor_scalar_max(den_sb[c % 2][:, :], ps_B2[c % 2][:, 0:H_],
                                    1e-30)
                v.drain()
                v.reciprocal(rcp_sb[c % 2][:, :], den_sb[c % 2][:, :])
                v.drain()
                v.scalar_tensor_tensor(
                    acc_sb[c % 2][:, :], ps_A[c % 2][:, 0:O_],
                    rcp_sb[c % 2][:, 0:1], binit_sb[:, :],
                    mybir.AluOpType.mult, mybir.AluOpType.add)
                for h in range(1, H_):
                    v.drain()
                    e = v.scalar_tensor_tensor(
                        acc_sb[c % 2][:, :], ps_A[c % 2][:, h * O_:(h + 1) * O_],
                        rcp_sb[c % 2][:, h:h + 1], acc_sb[c % 2][:, :],
                        mybir.AluOpType.mult, mybir.AluOpType.add)
                e.then_inc(s_ep, 1)

        @block.sync
        def _(sy):
            sy.dma_start(out=idx_sb[:, :], in_=idxp[:, :]).then_inc(s_idx, 16)
            for bN in range(NBLK):
                sy.wait_ge(s_st, H_ * (bN + 1))
                lo, hi = bN * VBLK, min(VP, bN * VBLK + VBLK)
                full_j, rem = (hi - lo) // 128, (hi - lo) % 128
                if full_j:
                    sy.dma_start(
                        out=bass.AP(tbl_loc, lo * RW,
                                    [[RW, 128], [128 * RW, full_j], [1, RW]]),
                        in_=stage[bN % 2][:, 0:full_j, :]).then_inc(s_sd[bN % 2], 16)
                if rem:
                    sy.dma_start(
                        out=bass.AP(tbl_loc, (lo + full_j * 128) * RW,
                                    [[RW, rem], [1, RW]]),
                        in_=stage[bN % 2][0:rem, full_j, :]).then_inc(s_sd[bN % 2], 16)
            for c in range(NCH):
                sy.wait_ge(s_rel, c + 1)
                lo = c * 128
                rows = min(128, VP - lo)
                sy.dma_start(out=bass.AP(out_ext, lo * O_, [[O_, rows], [1, O_]]),
                             in_=ostage[c % 2][0:rows, :]).then_inc(s_o[c % 2], 16)
            sy.wait_ge(s_o[0], 16 * ((NCH + 1) // 2))
            if NCH > 1:
                sy.wait_ge(s_o[1], 16 * (NCH // 2))

    sctx.close()
    ctx.close()
    nc.compile()
    return nc


def prep_core_inputs(cfg: Cfg, x, W, a, b, adj_lst, r):
    """Host-side shard/layout prep for core r (index/layout only, no math)."""
    VP, VPAD, NCH = cfg.VP, cfg.VPAD, cfg.NCHUNK
    xs = np.ascontiguousarray(x[r * VP:(r + 1) * VP].T)
    adj = adj_lst[r * VP:(r + 1) * VP]
    rows = np.where(adj == cfg.V, VP,
                    (adj // VP) * VPAD + (adj % VP)).astype(np.int32)
    rows_p = np.full((NCH * 128, cfg.K), VP, np.int32)
    rows_p[:VP] = rows
    # gtile[p, g, :] = tbl[flat[g*128+p]]; edge (node 8g+i, k) at p=16i+k.
    # dma_gather flat order i reads idxs_sb[i%16, i//16] (16-part wrap,
    # replicated to all 8 gpsimd cores); two 1024-idx ops per chunk.
    lin = rows_p.reshape(NCH, 16, 128)           # [c, g, p] (p = i*16 + k)
    F = lin.reshape(NCH, 2, 64, 16)              # [c, half, s, q]: i = s*16+q
    B = F.transpose(0, 1, 3, 2)                  # [c, half, q, s]
    idx = np.ascontiguousarray(
        np.tile(B, (1, 1, 8, 1)).transpose(2, 0, 1, 3).reshape(128, NCH * 128)
    ).astype(np.int16)
    blkm = np.zeros((128, 248), np.float32)
    blkm[np.arange(128), 120 + np.arange(128) // 16] = 1.0
    sel = np.zeros((cfg.H, cfg.H * 128), np.float32)
    for h in range(cfg.H):
        sel[h, h * 128:(h + 1) * 128] = 1.0
    W_ = np.asarray(W, np.float32)
    WT = np.ascontiguousarray(
        W_.transpose(2, 0, 1).reshape(cfg.O, cfg.H, cfg.DC, 128))
    return {
        "xT": xs.astype(np.float32), "W": W_, "WT": WT,
        "aT": np.ascontiguousarray(np.asarray(a, np.float32).T),
        "b": np.asarray(b, np.float32),
        "eye": np.eye(128, dtype=np.float32), "blk": blkm, "idx": idx,
        "sel": sel,
    }


_GRAPH_CACHE = {}


def kernel(x, W, a, b, adj_lst, mask_index, _cfg=None, _trace=False):
    cfg = _cfg or Cfg()
    x = np.asarray(x)
    adj_lst = np.asarray(adj_lst)
    assert int(mask_index) == cfg.V
    key = (cfg.V, cfg.ncores)
    if key not in _GRAPH_CACHE:
        _GRAPH_CACHE[key] = build_graph(cfg)
    nc = _GRAPH_CACHE[key]
    in_maps = [prep_core_inputs(cfg, x, W, a, b, adj_lst, r)
               for r in range(cfg.ncores)]
    res = run_bass_kernel_spmd(nc, in_maps, list(range(cfg.ncores)),
                               trace=_trace)
    out = np.concatenate([res.results[r]["out"] for r in range(cfg.ncores)], 0)
    kernel._last_exec_ns = res.exec_time_ns
    return out


# revision 51
# speedup vs baseline: 1.2093x; 1.0089x over previous
"""Distributed GAT (nn_AdjGAT) kernel for 8 TRN2 NeuronCores.

Math: the reference's per-edge softmax logit depends ONLY on the source
node (scores[h,v,k] = attn[h, adj[v,k]]), so
    softmax_k -> w[n_k] / sum_j w[n_j],  w = exp(attn)
    head[h,v] = (sum_k w[h,n_k] * t[h,n_k]) / (sum_k w[h,n_k])
    out = relu(mean_h(head + b[h]))
Phase 1 (per core): build node-major table rows [w*t_0..w*t_3 | w_0..w_3]
(bf16, padded to 640 elems = 1280B).  attn is computed independently of t
via a3 = W_h @ a_h (device-precomputed), attn = a3^T x per 512-node block
for all 4 heads at once.  AllGather the row table.
Phase 2: per 128-node chunk, two 1024-row dma_gather ops fetch the 2048
neighbor rows (SWDGE descriptor generation on GpSimd is the bottleneck,
~8ns/row), then block-diagonal-ones matmuls sum each node's K=16 rows
(numerator AND denominator), then normalize/relu.  Padding slots point at
a hardwired zero row (w=0).
"""

import math
from contextlib import ExitStack

import numpy as np

import concourse.bass as bass
import concourse.bacc as bacc
import concourse.mybir as mybir
from concourse import library_config
from concourse.bass_utils import run_bass_kernel_spmd

F32 = mybir.dt.float32
BF16 = mybir.dt.bfloat16
I16 = mybir.dt.int16
I32 = mybir.dt.int32

V, D, K, O, H = 20000, 256, 16, 128, 4
NCORES = 8


class Cfg:
    def __init__(self, V=V, D=D, K=K, O=O, H=H, ncores=NCORES):
        assert V % ncores == 0
        self.V, self.D, self.K, self.O, self.H, self.ncores = V, D, K, O, H, ncores
        self.VP = V // ncores
        self.ZPAD = 12
        self.VPAD = self.VP + self.ZPAD
        self.VT = self.VPAD * ncores
        self.RW = H * O + 128          # 640 bf16 elems: [s (512) | w (4) | pad]
        self.NCHUNK = math.ceil(self.VP / 128)
        self.VBLK = 512 if self.VP >= 512 else 128 * math.ceil(self.VP / 128)
        self.NBLK = math.ceil(self.VP / self.VBLK)
        self.NJ = self.VBLK // 128
        self.VPF = self.NBLK * self.VBLK
        self.DC = D // 128
        assert D % 128 == 0 and O == 128 and K == 16 and self.DC >= 1
        assert self.VT < 32000


def build_graph(cfg: Cfg, skip_ag=False):
    nc = bacc.Bacc()
    VP, VPAD, VT, RW, H_, O_, DC = (
        cfg.VP, cfg.VPAD, cfg.VT, cfg.RW, cfg.H, cfg.O, cfg.DC)
    NCH, NBLK, VBLK, NJ, VPF = cfg.NCHUNK, cfg.NBLK, cfg.VBLK, cfg.NJ, cfg.VPF
    SW = H_ * O_
    NE = 128 * 16
    NIT = NBLK * H_

    xT = nc.declare_dram_parameter("xT", [cfg.D, VP], F32, isOutput=False)
    Wp = nc.declare_dram_parameter("W", [H_, cfg.D, O_], F32, isOutput=False)
    WTp = nc.declare_dram_parameter("WT", [O_, H_, DC, 128], F32, isOutput=False)
    aTp = nc.declare_dram_parameter("aT", [O_, H_], F32, isOutput=False)
    bp = nc.declare_dram_parameter("b", [H_, O_], F32, isOutput=False)
    eyep = nc.declare_dram_parameter("eye", [128, 128], F32, isOutput=False)
    selp = nc.declare_dram_parameter("sel", [H_, H_ * 128], F32, isOutput=False)
    blkp = nc.declare_dram_parameter("blk", [128, 248], F32, isOutput=False)
    idxp = nc.declare_dram_parameter("idx", [128, NCH * 128], I16, isOutput=False)
    out_ext = nc.declare_dram_parameter("out", [VP, O_], F32, isOutput=True)

    tbl_loc = nc.dram_tensor("tbl_loc", [VPAD, RW], BF16)
    tbl = nc.dram_tensor("tbl", [VT, RW], BF16, addr_space="Shared")

    ctx = ExitStack()
    sb = lambda name, shape, dt: ctx.enter_context(nc.sbuf_tensor(name, shape, dt))
    xT_sb = sb("xT_sb", [128, DC, VPF], BF16)
    W_sb = sb("W_sb", [128, H_, DC, O_], BF16)
    WT_sb = sb("WT_sb", [128, H_, DC, 128], BF16)
    aT_sb = sb("aT_sb", [128, H_], BF16)
    a3_sb = sb("a3_sb", [128, DC * H_], BF16)
    b_sb = sb("b_sb", [H_, O_], BF16)
    eye_sb = sb("eye_sb", [128, 128], BF16)
    blk_sb = sb("blk_sb", [128, 248], BF16)
    idx_sb = sb("idx_sb", [128, NCH * 128], I16)
    ones4 = sb("ones4", [H_, O_], BF16)
    zero_sb = sb("zero_sb", [cfg.ZPAD, RW], BF16)
    w4 = [sb(f"w4_{i}", [H_, VBLK], BF16) for i in range(2)]
    sel_sb = sb("sel_sb", [H_, H_ * 128], BF16)
    tB_sb = [sb(f"tB_sb{i}", [128, VBLK], BF16) for i in range(2)]
    sB_sb = [sb(f"sB_sb{i}", [128, VBLK], BF16) for i in range(2)]
    stage = [sb(f"stage{i}", [128, NJ, RW], BF16) for i in range(2)]
    binit_sb = sb("binit_sb", [128, O_], F32)
    gtile = [sb(f"gtile{i}", [128, 16, RW], BF16) for i in range(2)]
    den_sb = [sb(f"den{i}", [128, H_], F32) for i in range(2)]
    rcp_sb = [sb(f"rcp{i}", [128, H_], F32) for i in range(2)]
    acc_sb = [sb(f"acc{i}", [128, O_], F32) for i in range(2)]
    ostage = [sb(f"ostage{i}", [128, O_], F32) for i in range(2)]

    # phase-1 PSUM (freed before phase-2 allocs; full 2KB banks each)
    ph1 = ExitStack()
    psa = lambda name, shape, dt: ph1.enter_context(nc.psum_tensor(name, shape, dt))
    ps_tB = [psa(f"ps_tB{i}", [128, 512], F32) for i in range(2)]
    ps_wbc = [psa(f"ps_wbc{i}", [128, 512], F32) for i in range(2)]
    ps_misc = [psa(f"ps_misc{i}", [128, 512], F32) for i in range(2)]
    ps_C = [psa(f"ps_C{i}", [128, 1024], BF16) for i in range(2)]
    ph1.close()
    ph2 = ExitStack()
    ps_A = [ph2.enter_context(nc.psum_tensor(f"ps_A{i}", [128, 512], F32))
            for i in range(2)]
    ps_B2 = [ph2.enter_context(nc.psum_tensor(f"ps_B2{i}", [128, 512], F32))
             for i in range(2)]
    ph2.close()

    blk_ndma = []
    for bN in range(NBLK):
        lo, hi = bN * VBLK, min(VP, bN * VBLK + VBLK)
        blk_ndma.append((1 if (hi - lo) // 128 else 0) + (1 if (hi - lo) % 128 else 0))

    sctx = ExitStack()
    sem = lambda n: sctx.enter_context(nc.semaphore(n))
    (s_ld, s_ld2, s_idx, s_dvi, s_bini, s_bcp, s_a3, s_a3c, s_tb, s_tbsb,
     s_attn, s_w, s_pw, s_sb, s_pc, s_w4t, s_st, s_zr, s_cc, s_mm, s_ep,
     s_rel) = [sem(n) for n in (
        "s_ld", "s_ld2", "s_idx", "s_dvi", "s_bini", "s_bcp", "s_a3",
        "s_a3c", "s_tb", "s_tbsb", "s_attn", "s_w", "s_pw", "s_sb", "s_pc",
        "s_w4t", "s_st", "s_zr", "s_cc", "s_mm", "s_ep", "s_rel")]
    s_xb = [sem(f"s_xb{i}") for i in range(NBLK)]
    s_ldb, s_lda, s_lds, s_ldwt, s_lde = [sem(n) for n in (
        "s_ldb", "s_lda", "s_lds", "s_ldwt", "s_lde")]
    s_g = [sem("s_g0"), sem("s_g1")]
    s_sd = [sem("s_sd0"), sem("s_sd1")]
    s_o = [sem("s_o0"), sem("s_o1")]
    # cumulative stage-DMA counts per parity (for safe per-parity waits)
    cum_par = []
    c0 = c1 = 0
    for bN in range(NBLK):
        if bN % 2 == 0:
            c0 += blk_ndma[bN]
        else:
            c1 += blk_ndma[bN]
        cum_par.append((c0, c1))
    with nc.Block() as block:
        @block.gpsimd
        def _(g):
            g.dma_start(out=b_sb[:, :], in_=bass.AP(
                bp, 0, [[O_, H_], [1, O_]])).then_inc(s_ldb, 16)
            g.dma_start(out=aT_sb[:, :], in_=bass.AP(
                aTp, 0, [[H_, 128], [1, H_]])).then_inc(s_lda, 16)
            g.dma_start(out=sel_sb[:, :], in_=selp[:, :]).then_inc(s_lds, 16)
            g.dma_start(out=W_sb[:, :, :, :], in_=bass.AP(
                Wp, 0, [[O_, 128], [cfg.D * O_, H_], [128 * O_, DC], [1, O_]]
            )).then_inc(s_ld, 16)
            lo, hi = 0, min(VP, VBLK)
            g.dma_start(out=xT_sb[:, :, lo:hi], in_=bass.AP(
                xT, lo, [[VP, 128], [128 * VP, DC], [1, hi - lo]])
            ).then_inc(s_xb[0], 16)
            g.dma_start(out=WT_sb[:, :, :, :], in_=bass.AP(
                WTp, 0, [[H_ * DC * 128, 128], [DC * 128, H_], [128, DC], [1, 128]]
            )).then_inc(s_ldwt, 16)
            g.dma_start(out=eye_sb[:, :], in_=eyep[:, :]).then_inc(s_lde, 16)
            for bN in range(1, NBLK):
                lo, hi = bN * VBLK, min(VP, bN * VBLK + VBLK)
                g.dma_start(out=xT_sb[:, :, lo:hi], in_=bass.AP(
                    xT, lo, [[VP, 128], [128 * VP, DC], [1, hi - lo]])
                ).then_inc(s_xb[bN], 16)
            g.dma_start(out=blk_sb[:, :], in_=blkp[:, :]).then_inc(s_ld2, 16)
            g.wait_ge(s_dvi, 1)
            g.dma_start(out=bass.AP(tbl_loc, VP * RW, [[RW, cfg.ZPAD], [1, RW]]),
                        in_=zero_sb[:, :]).then_inc(s_zr, 16)
            g.wait_ge(s_zr, 16)
            g.wait_ge(s_sd[0], 16 * cum_par[-1][0])
            if cum_par[-1][1]:
                g.wait_ge(s_sd[1], 16 * cum_par[-1][1])
            if skip_ag:
                g.dma_start(out=bass.AP(tbl, 0, [[RW, VPAD], [1, RW]]),
                            in_=tbl_loc[:, :]).then_inc(s_cc, 16)
                g.wait_ge(s_cc, 16)
            else:
                g.collective_compute(
                    "AllGather", mybir.AluOpType.bypass,
                    replica_groups=[list(range(cfg.ncores))],
                    ins=[tbl_loc[:, :]], outs=[tbl[:, :]],
                ).then_inc(s_cc)
                g.wait_ge(s_cc, 1)
            g.wait_ge(s_idx, 16)
            for c in range(NCH):
                if c >= 2:
                    g.wait_ge(s_mm, c - 1)
                for hf in range(2):
                    g.dma_gather(
                        out_ap=gtile[c % 2][:, hf * 8:hf * 8 + 8, :],
                        in_ap=tbl[:, :],
                        idxs_ap=idx_sb[:, c * 128 + hf * 64:c * 128 + hf * 64 + 64],
                        num_idxs=NE // 2,
                        num_idxs_reg=NE // 2,
                        elem_size=RW,
                    ).then_inc(s_g[c % 2], 16)

        @block.tensor
        def _(pe):
            def transp(t):
                pe.wait_ge(s_sb, t + 1)          # sB_sb[t%2] ready (DVE done)
                if t >= 2:
                    pe.wait_ge(s_st, t - 1)      # ps_C[t%2] s-cols free (DVE)
                for j in range(NJ):
                    e = pe.transpose(ps_C[t % 2][:, j * 128:(j + 1) * 128],
                                     sB_sb[t % 2][:, j * 128:(j + 1) * 128],
                                     eye_sb[:, :])
                e.then_inc(s_pc, 1)

            pe.wait_ge(s_ldb, 16)
            pe.wait_ge(s_dvi, 1)
            pe.matmul(ps_misc[0][:, 0:O_], ones4[:, :], b_sb[:, :],
                      start=True, stop=True).then_inc(s_bini, 1)
            pe.wait_ge(s_ldwt, 16)
            pe.wait_ge(s_lda, 16)
            for c in range(DC):
                for h in range(H_):
                    e = pe.matmul(ps_misc[1][:, c * H_ + h:c * H_ + h + 1],
                                  WT_sb[:, h, c, :], aT_sb[:, h:h + 1],
                                  start=True, stop=True)
            e.then_inc(s_a3, 1)
            pe.wait_ge(s_ld, 16)                 # W_sb
            pe.wait_ge(s_lds, 16)                # sel_sb
            pe.wait_ge(s_lde, 16)                # eye_sb
            for it in range(NIT):
                bN, h = divmod(it, H_)
                vlo = bN * VBLK
                if it >= 1:
                    transp(it - 1)
                if h == 0:
                    pe.wait_ge(s_xb[bN], 16)
                    if bN == 0:
                        pe.wait_ge(s_bcp, 1)     # ps_misc[0] binit copied out
                        pe.wait_ge(s_a3c, 1)     # a3_sb ready (+ps_misc[1] free)
                    if bN >= 2:
                        pe.wait_ge(s_w, bN - 1)  # ps_misc[bN%2] free (exp done)
                    if bN >= 1:
                        pe.wait_ge(s_st, 4 * bN)  # pw cols of prev block copied
                    for c in range(DC):
                        mm = pe.matmul(ps_misc[bN % 2][0:4, 0:VBLK],
                                       a3_sb[:, c * H_:(c + 1) * H_],
                                       xT_sb[:, c, vlo:vlo + VBLK],
                                       start=(c == 0), stop=(c == DC - 1))
                    mm.then_inc(s_attn, 1)
                if it >= 2:
                    pe.wait_ge(s_tbsb, it - 1)   # ps_tB[it%2] free (ACT copied)
                mm = pe.matmul(ps_tB[it % 2][:, 0:VBLK], W_sb[:, h, 0, :],
                               xT_sb[:, 0, vlo:vlo + VBLK],
                               start=True, stop=(DC == 1))
                for c in range(1, DC):
                    mm = pe.matmul(ps_tB[it % 2][:, 0:VBLK], W_sb[:, h, c, :],
                                   xT_sb[:, c, vlo:vlo + VBLK],
                                   start=False, stop=(c == DC - 1))
                mm.then_inc(s_tb, 1)
                pe.wait_ge(s_w, bN + 1)          # w4[bN%2] ready (exp done)
                if it >= 2:
                    pe.wait_ge(s_sb, it - 1)     # ps_wbc[it%2] free (DVE)
                    pe.wait_ge(s_st, it - 1)     # ps_misc[it%2] pw cols copied
                pe.matmul(ps_wbc[it % 2][:, 0:VBLK],
                          sel_sb[:, h * 128:(h + 1) * 128],
                          w4[bN % 2][:, :], start=True, stop=True)
                for j in range(NJ):
                    e = pe.matmul(
                        ps_misc[it % 2][:, j:j + 1],
                        w4[bN % 2][:, j * 128:(j + 1) * 128],
                        sel_sb[:, h * 128:h * 128 + 1], start=True, stop=True)
                e.then_inc(s_pw, 1)
            transp(NIT - 1)
            pe.wait_ge(s_ld2, 16)
            for c in range(NCH):
                pe.wait_ge(s_g[c % 2], 32 * (c // 2 + 1))
                if c >= 2:
                    pe.wait_ge(s_ep, c - 1)      # ps_A/ps_B2[c%2] free (DVE)
                for gi in range(16):
                    pe.matmul(ps_A[c % 2][:, 0:SW],
                              blk_sb[:, 120 - 8 * gi:248 - 8 * gi],
                              gtile[c % 2][:, gi, 0:SW],
                              start=(gi == 0), stop=(gi == 15))
                for gi in range(16):
                    e = pe.matmul(ps_B2[c % 2][:, 0:H_],
                                  blk_sb[:, 120 - 8 * gi:248 - 8 * gi],
                                  gtile[c % 2][:, gi, SW:SW + H_],
                                  start=(gi == 0), stop=(gi == 15))
                e.then_inc(s_mm, 1)

        @block.scalar
        def _(s):
            s.wait_ge(s_a3, 1)
            s.activation(a3_sb[:, :], ps_misc[1][:, 0:DC * H_],
                         mybir.ActivationFunctionType.Copy).then_inc(s_a3c, 1)
            for it in range(NIT):
                bN, h = divmod(it, H_)
                if h == 0:
                    s.wait_ge(s_attn, bN + 1)
                    if bN >= 2:
                        s.wait_ge(s_pw, 4 * bN - 4)  # w4[bN%2] free (PE read)
                    s.activation(w4[bN % 2][:, :],
                                 ps_misc[bN % 2][0:4, 0:VBLK],
                                 mybir.ActivationFunctionType.Exp
                                 ).then_inc(s_w, 1)
                s.wait_ge(s_tb, it + 1)
                if it >= 2:
                    s.wait_ge(s_sb, it - 1)      # tB_sb[it%2] free (DVE read)
                s.activation(tB_sb[it % 2][:, :], ps_tB[it % 2][:, 0:VBLK],
                             mybir.ActivationFunctionType.Copy).then_inc(s_tbsb, 1)
            for c in range(NCH):
                s.wait_ge(s_ep, c + 1)
                if c >= 2:
                    s.wait_ge(s_o[c % 2], 16 * (c // 2))  # ostage free
                s.activation(ostage[c % 2][:, :], acc_sb[c % 2][:, :],
                             mybir.ActivationFunctionType.Relu,
                             scale=1.0 / H_).then_inc(s_rel, 1)

        @block.vector
        def _(v):
            def copies(t):
                b, h = divmod(t, H_)
                v.wait_ge(s_pc, t + 1)           # ps_C[t%2] s-cols ready
                if h == 0 and b >= 2:
                    v.wait_ge(s_sd[b % 2], 16 * cum_par[b - 2][b % 2])
                v.tensor_copy(
                    stage[b % 2][:, :, h * O_:(h + 1) * O_],
                    bass.AP(ps_C[t % 2], 0, [[1024, 128], [128, NJ], [1, 128]]))
                v.tensor_copy(
                    stage[b % 2][:, :, SW + h:SW + h + 1],
                    bass.AP(ps_misc[t % 2], 0, [[512, 128], [1, NJ], [1, 1]])
                ).then_inc(s_st, 1)

            v.memset(ones4[:, :], 1.0)
            v.memset(stage[0][:, :, SW + H_:RW], 0.0)
            v.memset(stage[1][:, :, SW + H_:RW], 0.0)
            if VPF > VP:
                v.memset(xT_sb[:, :, VP:VPF], 0.0)
            v.memset(zero_sb[:, :], 0.0).then_inc(s_dvi, 1)
            v.wait_ge(s_bini, 1)
            v.tensor_copy(binit_sb[:, :], ps_misc[0][:, 0:O_]).then_inc(s_bcp, 1)
            for it in range(NIT):
                if it >= 1:
                    copies(it - 1)
                v.wait_ge(s_tbsb, it + 1)        # tB_sb[it%2] ready
                v.wait_ge(s_pw, it + 1)          # ps_wbc[it%2] ready
                if it >= 2:
                    v.wait_ge(s_pc, it - 1)      # sB_sb[it%2] free (PE read)
                v.tensor_tensor(sB_sb[it % 2][:, :], tB_sb[it % 2][:, :],
                                ps_wbc[it % 2][:, 0:VBLK],
                                mybir.AluOpType.mult).then_inc(s_sb, 1)
            copies(NIT - 1)
            for c in range(NCH):
                v.wait_ge(s_mm, c + 1)
                if c >= 2:
                    v.wait_ge(s_rel, c - 1)      # acc_sb[c%2] free (ACT read)
                v.tensor_scalar_max(den_sb[c % 2][:, :], ps_B2[c % 2][:, 0:H_],
                                    1e-30)
                v.drain()
                v.reciprocal(rcp_sb[c % 2][:, :], den_sb[c % 2][:, :])
                v.drain()
                v.scalar_tensor_tensor(
                    acc_sb[c % 2][:, :], ps_A[c % 2][:, 0:O_],
                    rcp_sb[c % 2][:, 0:1], binit_sb[:, :],
                    mybir.AluOpType.mult, mybir.AluOpType.add)
                for h in range(1, H_):
                    v.drain()
                    e = v.scalar_tensor_tensor(
                        acc_sb[c % 2][:, :], ps_A[c % 2][:, h * O_:(h + 1) * O_],
                        rcp_sb[c % 2][:, h:h + 1], acc_sb[c % 2][:, :],
                        mybir.AluOpType.mult, mybir.AluOpType.add)
                e.then_inc(s_ep, 1)

        @block.sync
        def _(sy):
            sy.dma_start(out=idx_sb[:, :], in_=idxp[:, :]).then_inc(s_idx, 16)
            for bN in range(NBLK):
                sy.wait_ge(s_st, H_ * (bN + 1))
                lo, hi = bN * VBLK, min(VP, bN * VBLK + VBLK)
                full_j, rem = (hi - lo) // 128, (hi - lo) % 128
                if full_j:
                    sy.dma_start(
                        out=bass.AP(tbl_loc, lo * RW,
                                    [[RW, 128], [128 * RW, full_j], [1, RW]]),
                        in_=stage[bN % 2][:, 0:full_j, :]).then_inc(s_sd[bN % 2], 16)
                if rem:
                    sy.dma_start(
                        out=bass.AP(tbl_loc, (lo + full_j * 128) * RW,
                                    [[RW, rem], [1, RW]]),
                        in_=stage[bN % 2][0:rem, full_j, :]).then_inc(s_sd[bN % 2], 16)
            for c in range(NCH):
                sy.wait_ge(s_rel, c + 1)
                lo = c * 128
                rows = min(128, VP - lo)
                sy.dma_start(out=bass.AP(out_ext, lo * O_, [[O_, rows], [1, O_]]),
                             in_=ostage[c % 2][0:rows, :]).then_inc(s_o[c % 2], 16)
            sy.wait_ge(s_o[0], 16 * ((NCH + 1) // 2))
            if NCH > 1:
                sy.wait_ge(s_o[1], 16 * (NCH // 2))

    sctx.close()
    ctx.close()
    nc.compile()
    return nc


def prep_core_inputs(cfg: Cfg, x, W, a, b, adj_lst, r):
    """Host-side shard/layout prep for core r (index/layout only, no math)."""
    VP, VPAD, NCH = cfg.VP, cfg.VPAD, cfg.NCHUNK
    xs = np.ascontiguousarray(x[r * VP:(r + 1) * VP].T)
    adj = adj_lst[r * VP:(r + 1) * VP]
    rows = np.where(adj == cfg.V, VP,
                    (adj // VP) * VPAD + (adj % VP)).astype(np.int32)
    rows_p = np.full((NCH * 128, cfg.K), VP, np.int32)
    rows_p[:VP] = rows
    # gtile[p, g, :] = tbl[flat[g*128+p]]; edge (node 8g+i, k) at p=16i+k.
    # dma_gather flat order i reads idxs_sb[i%16, i//16] (16-part wrap,
    # replicated to all 8 gpsimd cores); two 1024-idx ops per chunk.
    lin = rows_p.reshape(NCH, 16, 128)           # [c, g, p] (p = i*16 + k)
    F = lin.reshape(NCH, 2, 64, 16)              # [c, half, s, q]: i = s*16+q
    B = F.transpose(0, 1, 3, 2)                  # [c, half, q, s]
    idx = np.ascontiguousarray(
        np.tile(B, (1, 1, 8, 1)).transpose(2, 0, 1, 3).reshape(128, NCH * 128)
    ).astype(np.int16)
    blkm = np.zeros((128, 248), np.float32)
    blkm[np.arange(128), 120 + np.arange(128) // 16] = 1.0
    sel = np.zeros((cfg.H, cfg.H * 128), np.float32)
    for h in range(cfg.H):
        sel[h, h * 128:(h + 1) * 128] = 1.0
    W_ = np.asarray(W, np.float32)
    WT = np.ascontiguousarray(
        W_.transpose(2, 0, 1).reshape(cfg.O, cfg.H, cfg.DC, 128))
    return {
        "xT": xs.astype(np.float32), "W": W_, "WT": WT,
        "aT": np.ascontiguousarray(np.asarray(a, np.float32).T),
        "b": np.asarray(b, np.float32),
        "eye": np.eye(128, dtype=np.float32), "blk": blkm, "idx": idx,
        "sel": sel,
    }


_GRAPH_CACHE = {}


def kernel(x, W, a, b, adj_lst, mask_index, _cfg=None, _trace=False):
    cfg = _cfg or Cfg()
    x = np.asarray(x)
    adj_lst = np.asarray(adj_lst)
    assert int(mask_index) == cfg.V
    key = (cfg.V, cfg.ncores)
    if key not in _GRAPH_CACHE:
        _GRAPH_CACHE[key] = build_graph(cfg)
    nc = _GRAPH_CACHE[key]
    in_maps = [prep_core_inputs(cfg, x, W, a, b, adj_lst, r)
               for r in range(cfg.ncores)]
    res = run_bass_kernel_spmd(nc, in_maps, list(range(cfg.ncores)),
                               trace=_trace)
    out = np.concatenate([res.results[r]["out"] for r in range(cfg.ncores)], 0)
    kernel._last_exec_ns = res.exec_time_ns
    return out


# revision 55
# speedup vs baseline: 1.2461x; 1.0304x over previous
"""Distributed GAT (nn_AdjGAT) kernel for 8 TRN2 NeuronCores.

Math: the reference's per-edge softmax logit depends ONLY on the source
node (scores[h,v,k] = attn[h, adj[v,k]]), so
    softmax_k -> w[n_k] / sum_j w[n_j],  w = exp(attn)
    head[h,v] = (sum_k w[h,n_k] * t[h,n_k]) / (sum_k w[h,n_k])
    out = relu(mean_h(head + b[h]))
Phase 1 (per core): build node-major table rows [w*t_0..w*t_3 | w_0..w_3]
(bf16, padded to 640 elems = 1280B).  attn is computed independently of t
via a3 = W_h @ a_h (device-precomputed), attn = a3^T x per 512-node block
for all 4 heads at once.  AllGather the row table.
Phase 2: per 128-node chunk, two 1024-row dma_gather ops fetch the 2048
neighbor rows (SWDGE descriptor generation on GpSimd is the bottleneck,
~8ns/row), then block-diagonal-ones matmuls sum each node's K=16 rows
(numerator AND denominator), then normalize/relu.  Padding slots point at
a hardwired zero row (w=0).
"""

import math
from contextlib import ExitStack

import numpy as np

import concourse.bass as bass
import concourse.bacc as bacc
import concourse.mybir as mybir
from concourse import library_config
from concourse.bass_utils import run_bass_kernel_spmd

F32 = mybir.dt.float32
BF16 = mybir.dt.bfloat16
I16 = mybir.dt.int16
I32 = mybir.dt.int32

V, D, K, O, H = 20000, 256, 16, 128, 4
NCORES = 8


class Cfg:
    def __init__(self, V=V, D=D, K=K, O=O, H=H, ncores=NCORES):
        assert V % ncores == 0
        self.V, self.D, self.K, self.O, self.H, self.ncores = V, D, K, O, H, ncores
        self.VP = V // ncores
        self.ZPAD = 12
        self.VPAD = self.VP + self.ZPAD
        self.VT = self.VPAD * ncores
        self.RW = H * O + 128          # 640 bf16 elems: [s (512) | w (4) | pad]
        self.NCHUNK = math.ceil(self.VP / 128)
        self.VBLK = 512 if self.VP >= 512 else 128 * math.ceil(self.VP / 128)
        self.NBLK = math.ceil(self.VP / self.VBLK)
        self.NJ = self.VBLK // 128
        self.VPF = self.NBLK * self.VBLK
        self.DC = D // 128
        assert D % 128 == 0 and O == 128 and K == 16 and self.DC >= 1
        assert self.VT < 32000


def build_graph(cfg: Cfg, skip_ag=False):
    nc = bacc.Bacc()
    VP, VPAD, VT, RW, H_, O_, DC = (
        cfg.VP, cfg.VPAD, cfg.VT, cfg.RW, cfg.H, cfg.O, cfg.DC)
    NCH, NBLK, VBLK, NJ, VPF = cfg.NCHUNK, cfg.NBLK, cfg.VBLK, cfg.NJ, cfg.VPF
    SW = H_ * O_
    NE = 128 * 16
    NIT = NBLK * H_

    xT = nc.declare_dram_parameter("xT", [cfg.D, VP], F32, isOutput=False)
    Wp = nc.declare_dram_parameter("W", [H_, cfg.D, O_], F32, isOutput=False)
    WTp = nc.declare_dram_parameter("WT", [O_, H_, DC, 128], F32, isOutput=False)
    aTp = nc.declare_dram_parameter("aT", [O_, H_], F32, isOutput=False)
    bp = nc.declare_dram_parameter("b", [H_, O_], F32, isOutput=False)
    eyep = nc.declare_dram_parameter("eye", [128, 128], F32, isOutput=False)
    selp = nc.declare_dram_parameter("sel", [H_, H_ * 128], F32, isOutput=False)
    blkp = nc.declare_dram_parameter("blk", [128, 248], F32, isOutput=False)
    idxp = nc.declare_dram_parameter("idx", [128, NCH * 128], I16, isOutput=False)
    out_ext = nc.declare_dram_parameter("out", [VP, O_], F32, isOutput=True)

    tbl_loc = nc.dram_tensor("tbl_loc", [VPAD, RW], BF16)
    tbl = nc.dram_tensor("tbl", [VT, RW], BF16, addr_space="Shared")

    ctx = ExitStack()
    sb = lambda name, shape, dt: ctx.enter_context(nc.sbuf_tensor(name, shape, dt))
    xT_sb = sb("xT_sb", [128, DC, VPF], BF16)
    W_sb = sb("W_sb", [128, H_, DC, O_], BF16)
    WT_sb = sb("WT_sb", [128, H_, DC, 128], BF16)
    aT_sb = sb("aT_sb", [128, H_], BF16)
    a3_sb = sb("a3_sb", [128, DC * H_], BF16)
    b_sb = sb("b_sb", [H_, O_], BF16)
    eye_sb = sb("eye_sb", [128, 128], BF16)
    blk_sb = sb("blk_sb", [128, 248], BF16)
    idx_sb = sb("idx_sb", [128, NCH * 128], I16)
    ones4 = sb("ones4", [H_, O_], BF16)
    zero_sb = sb("zero_sb", [cfg.ZPAD, RW], BF16)
    w4 = [sb(f"w4_{i}", [H_, VBLK], BF16) for i in range(2)]
    sel_sb = sb("sel_sb", [H_, H_ * 128], BF16)
    tB_sb = [sb(f"tB_sb{i}", [128, VBLK], BF16) for i in range(2)]
    sB_sb = [sb(f"sB_sb{i}", [128, VBLK], BF16) for i in range(2)]
    stage = [sb(f"stage{i}", [128, NJ, RW], BF16) for i in range(2)]
    binit_sb = sb("binit_sb", [128, O_], F32)
    gtile = [sb(f"gtile{i}", [128, 16, RW], BF16) for i in range(2)]
    den_sb = [sb(f"den{i}", [128, H_], F32) for i in range(2)]
    rcp_sb = [sb(f"rcp{i}", [128, H_], F32) for i in range(2)]
    acc_sb = [sb(f"acc{i}", [128, O_], F32) for i in range(2)]
    ostage = [sb(f"ostage{i}", [128, O_], F32) for i in range(2)]

    # phase-1 PSUM (freed before phase-2 allocs; full 2KB banks each)
    ph1 = ExitStack()
    psa = lambda name, shape, dt: ph1.enter_context(nc.psum_tensor(name, shape, dt))
    ps_tB = [psa(f"ps_tB{i}", [128, 512], F32) for i in range(2)]
    ps_wbc = [psa(f"ps_wbc{i}", [128, 512], F32) for i in range(2)]
    ps_misc = [psa(f"ps_misc{i}", [128, 512], F32) for i in range(2)]
    ps_C = [psa(f"ps_C{i}", [128, 1024], BF16) for i in range(2)]
    ph1.close()
    ph2 = ExitStack()
    ps_A = [ph2.enter_context(nc.psum_tensor(f"ps_A{i}", [128, 512], F32))
            for i in range(2)]
    ps_B2 = [ph2.enter_context(nc.psum_tensor(f"ps_B2{i}", [128, 512], F32))
             for i in range(2)]
    ph2.close()

    blk_ndma = []
    for bN in range(NBLK):
        lo, hi = bN * VBLK, min(VP, bN * VBLK + VBLK)
        blk_ndma.append((1 if (hi - lo) // 128 else 0) + (1 if (hi - lo) % 128 else 0))

    sctx = ExitStack()
    sem = lambda n: sctx.enter_context(nc.semaphore(n))
    (s_ld, s_ld2, s_idx, s_dvi, s_bini, s_bcp, s_a3, s_a3c, s_tb, s_tbsb,
     s_attn, s_w, s_pw, s_sb, s_pc, s_w4t, s_st, s_zr, s_cc, s_mm, s_ep,
     s_rel) = [sem(n) for n in (
        "s_ld", "s_ld2", "s_idx", "s_dvi", "s_bini", "s_bcp", "s_a3",
        "s_a3c", "s_tb", "s_tbsb", "s_attn", "s_w", "s_pw", "s_sb", "s_pc",
        "s_w4t", "s_st", "s_zr", "s_cc", "s_mm", "s_ep", "s_rel")]
    s_xb = [sem(f"s_xb{i}") for i in range(NBLK)]
    s_ldb, s_lda, s_lds, s_ldwt, s_lde = [sem(n) for n in (
        "s_ldb", "s_lda", "s_lds", "s_ldwt", "s_lde")]
    s_g = [sem("s_g0"), sem("s_g1")]
    s_sd = [sem("s_sd0"), sem("s_sd1")]
    s_o = [sem("s_o0"), sem("s_o1")]
    # cumulative stage-DMA counts per parity (for safe per-parity waits)
    cum_par = []
    c0 = c1 = 0
    for bN in range(NBLK):
        if bN % 2 == 0:
            c0 += blk_ndma[bN]
        else:
            c1 += blk_ndma[bN]
        cum_par.append((c0, c1))
    with nc.Block() as block:
        @block.gpsimd
        def _(g):
            g.dma_start(out=b_sb[:, :], in_=bass.AP(
                bp, 0, [[O_, H_], [1, O_]])).then_inc(s_ldb, 16)
            g.dma_start(out=aT_sb[:, :], in_=bass.AP(
                aTp, 0, [[H_, 128], [1, H_]])).then_inc(s_lda, 16)
            g.dma_start(out=sel_sb[:, :], in_=selp[:, :]).then_inc(s_lds, 16)
            g.dma_start(out=W_sb[:, :, :, :], in_=bass.AP(
                Wp, 0, [[O_, 128], [cfg.D * O_, H_], [128 * O_, DC], [1, O_]]
            )).then_inc(s_ld, 16)
            lo, hi = 0, min(VP, VBLK)
            g.dma_start(out=xT_sb[:, :, lo:hi], in_=bass.AP(
                xT, lo, [[VP, 128], [128 * VP, DC], [1, hi - lo]])
            ).then_inc(s_xb[0], 16)
            g.dma_start(out=WT_sb[:, :, :, :], in_=bass.AP(
                WTp, 0, [[H_ * DC * 128, 128], [DC * 128, H_], [128, DC], [1, 128]]
            )).then_inc(s_ldwt, 16)
            g.dma_start(out=eye_sb[:, :], in_=eyep[:, :]).then_inc(s_lde, 16)
            for bN in range(1, NBLK):
                lo, hi = bN * VBLK, min(VP, bN * VBLK + VBLK)
                g.dma_start(out=xT_sb[:, :, lo:hi], in_=bass.AP(
                    xT, lo, [[VP, 128], [128 * VP, DC], [1, hi - lo]])
                ).then_inc(s_xb[bN], 16)
            g.dma_start(out=blk_sb[:, :], in_=blkp[:, :]).then_inc(s_ld2, 16)
            g.wait_ge(s_dvi, 1)
            g.dma_start(out=bass.AP(tbl_loc, VP * RW, [[RW, cfg.ZPAD], [1, RW]]),
                        in_=zero_sb[:, :]).then_inc(s_zr, 16)
            g.wait_ge(s_zr, 16)
            g.wait_ge(s_sd[0], 16 * cum_par[-1][0])
            if cum_par[-1][1]:
                g.wait_ge(s_sd[1], 16 * cum_par[-1][1])
            if skip_ag:
                g.dma_start(out=bass.AP(tbl, 0, [[RW, VPAD], [1, RW]]),
                            in_=tbl_loc[:, :]).then_inc(s_cc, 16)
                g.wait_ge(s_cc, 16)
            else:
                g.collective_compute(
                    "AllGather", mybir.AluOpType.bypass,
                    replica_groups=[list(range(cfg.ncores))],
                    ins=[tbl_loc[:, :]], outs=[tbl[:, :]],
                ).then_inc(s_cc)
                g.wait_ge(s_cc, 1)
            g.wait_ge(s_idx, 16)
            for c in range(NCH):
                if c >= 2:
                    g.wait_ge(s_mm, c - 1)
                for hf in range(2):
                    g.dma_gather(
                        out_ap=gtile[c % 2][:, hf * 8:hf * 8 + 8, :],
                        in_ap=tbl[:, :],
                        idxs_ap=idx_sb[:, c * 128 + hf * 64:c * 128 + hf * 64 + 64],
                        num_idxs=NE // 2,
                        num_idxs_reg=NE // 2,
                        elem_size=RW,
                    ).then_inc(s_g[c % 2], 16)

        @block.tensor
        def _(pe):
            def transp(t):
                pe.wait_ge(s_sb, t + 1)          # sB_sb[t%2] ready (DVE done)
                if t >= 2:
                    pe.wait_ge(s_st, t - 1)      # ps_C[t%2] s-cols free (DVE)
                for j in range(NJ):
                    e = pe.transpose(ps_C[t % 2][:, j * 128:(j + 1) * 128],
                                     sB_sb[t % 2][:, j * 128:(j + 1) * 128],
                                     eye_sb[:, :])
                e.then_inc(s_pc, 1)

            pe.wait_ge(s_ldb, 16)
            pe.wait_ge(s_dvi, 1)
            pe.matmul(ps_misc[0][:, 0:O_], ones4[:, :], b_sb[:, :],
                      start=True, stop=True).then_inc(s_bini, 1)
            pe.wait_ge(s_ldwt, 16)
            pe.wait_ge(s_lda, 16)
            for c in range(DC):
                for h in range(H_):
                    e = pe.matmul(ps_misc[1][:, c * H_ + h:c * H_ + h + 1],
                                  WT_sb[:, h, c, :], aT_sb[:, h:h + 1],
                                  start=True, stop=True)
            e.then_inc(s_a3, 1)
            pe.wait_ge(s_ld, 16)                 # W_sb
            pe.wait_ge(s_lds, 16)                # sel_sb
            pe.wait_ge(s_lde, 16)                # eye_sb

            def attn(b):
                pe.wait_ge(s_xb[b], 16)
                vl = b * VBLK
                for c in range(DC):
                    mm = pe.matmul(ps_misc[b % 2][0:4, 0:VBLK],
                                   a3_sb[:, c * H_:(c + 1) * H_],
                                   xT_sb[:, c, vl:vl + VBLK],
                                   start=(c == 0), stop=(c == DC - 1))
                mm.then_inc(s_attn, 1)

            pe.wait_ge(s_bcp, 1)                 # ps_misc[0] binit copied out
            pe.wait_ge(s_a3c, 1)                 # a3_sb ready (+ps_misc[1] free)
            attn(0)
            for it in range(NIT):
                bN, h = divmod(it, H_)
                vlo = bN * VBLK
                if it >= 2:
                    transp(it - 2)
                if h == 3 and bN + 1 < NBLK:
                    if bN >= 1:
                        pe.wait_ge(s_w, bN)      # attn region read (exp bN-1)
                        pe.wait_ge(s_st, 4 * bN)  # pw cols of block bN-1 copied
                    attn(bN + 1)
                if it >= 2:
                    pe.wait_ge(s_tbsb, it - 1)   # ps_tB[it%2] free (ACT copied)
                mm = pe.matmul(ps_tB[it % 2][:, 0:VBLK], W_sb[:, h, 0, :],
                               xT_sb[:, 0, vlo:vlo + VBLK],
                               start=True, stop=(DC == 1))
                for c in range(1, DC):
                    mm = pe.matmul(ps_tB[it % 2][:, 0:VBLK], W_sb[:, h, c, :],
                                   xT_sb[:, c, vlo:vlo + VBLK],
                                   start=False, stop=(c == DC - 1))
                mm.then_inc(s_tb, 1)
                pe.wait_ge(s_w, bN + 1)          # w4[bN%2] ready (exp done)
                if it >= 2:
                    pe.wait_ge(s_sb, it - 1)     # ps_wbc[it%2] free (DVE)
                if it >= 8:
                    pe.wait_ge(s_st, it - 7)     # ps_misc[bN%2] pw cols copied
                pe.matmul(ps_wbc[it % 2][:, 0:VBLK],
                          sel_sb[:, h * 128:(h + 1) * 128],
                          w4[bN % 2][:, :], start=True, stop=True)
                for j in range(NJ):
                    e = pe.matmul(
                        ps_misc[bN % 2][:, 4 * h + j:4 * h + j + 1],
                        w4[bN % 2][:, j * 128:(j + 1) * 128],
                        sel_sb[:, h * 128:h * 128 + 1], start=True, stop=True)
                e.then_inc(s_pw, 1)
            transp(NIT - 2)
            transp(NIT - 1)
            pe.wait_ge(s_ld2, 16)
            for c in range(NCH):
                pe.wait_ge(s_g[c % 2], 32 * (c // 2 + 1))
                if c >= 2:
                    pe.wait_ge(s_ep, c - 1)      # ps_A/ps_B2[c%2] free (DVE)
                for gi in range(16):
                    pe.matmul(ps_A[c % 2][:, 0:SW],
                              blk_sb[:, 120 - 8 * gi:248 - 8 * gi],
                              gtile[c % 2][:, gi, 0:SW],
                              start=(gi == 0), stop=(gi == 15))
                for gi in range(16):
                    e = pe.matmul(ps_B2[c % 2][:, 0:H_],
                                  blk_sb[:, 120 - 8 * gi:248 - 8 * gi],
                                  gtile[c % 2][:, gi, SW:SW + H_],
                                  start=(gi == 0), stop=(gi == 15))
                e.then_inc(s_mm, 1)

        @block.scalar
        def _(s):
            s.wait_ge(s_a3, 1)
            s.activation(a3_sb[:, :], ps_misc[1][:, 0:DC * H_],
                         mybir.ActivationFunctionType.Copy).then_inc(s_a3c, 1)
            for it in range(NIT):
                bN, h = divmod(it, H_)
                if h == 0:
                    s.wait_ge(s_attn, bN + 1)
                    if bN >= 2:
                        s.wait_ge(s_pw, 4 * bN - 4)  # w4[bN%2] free (PE read)
                    s.activation(w4[bN % 2][:, :],
                                 ps_misc[bN % 2][0:4, 0:VBLK],
                                 mybir.ActivationFunctionType.Exp
                                 ).then_inc(s_w, 1)
                s.wait_ge(s_tb, it + 1)
                if it >= 2:
                    s.wait_ge(s_sb, it - 1)      # tB_sb[it%2] free (DVE read)
                s.activation(tB_sb[it % 2][:, :], ps_tB[it % 2][:, 0:VBLK],
                             mybir.ActivationFunctionType.Copy).then_inc(s_tbsb, 1)
            for c in range(NCH):
                s.wait_ge(s_ep, c + 1)
                if c >= 2:
                    s.wait_ge(s_o[c % 2], 16 * (c // 2))  # ostage free
                s.activation(ostage[c % 2][:, :], acc_sb[c % 2][:, :],
                             mybir.ActivationFunctionType.Relu,
                             scale=1.0 / H_).then_inc(s_rel, 1)

        @block.vector
        def _(v):
            def copies(t):
                b, h = divmod(t, H_)
                v.wait_ge(s_pc, t + 1)           # ps_C[t%2] s-cols ready
                if h == 0 and b >= 2:
                    v.wait_ge(s_sd[b % 2], 16 * cum_par[b - 2][b % 2])
                v.tensor_copy(
                    stage[b % 2][:, :, h * O_:(h + 1) * O_],
                    bass.AP(ps_C[t % 2], 0, [[1024, 128], [128, NJ], [1, 128]]))
                v.tensor_copy(
                    stage[b % 2][:, :, SW + h:SW + h + 1],
                    bass.AP(ps_misc[b % 2], 4 * h, [[512, 128], [1, NJ], [1, 1]])
                ).then_inc(s_st, 1)

            v.memset(ones4[:, :], 1.0)
            v.memset(stage[0][:, :, SW + H_:RW], 0.0)
            v.memset(stage[1][:, :, SW + H_:RW], 0.0)
            if VPF > VP:
                v.memset(xT_sb[:, :, VP:VPF], 0.0)
            v.memset(zero_sb[:, :], 0.0).then_inc(s_dvi, 1)
            v.wait_ge(s_bini, 1)
            v.tensor_copy(binit_sb[:, :], ps_misc[0][:, 0:O_]).then_inc(s_bcp, 1)
            for it in range(NIT):
                if it >= 1:
                    copies(it - 1)
                v.wait_ge(s_tbsb, it + 1)        # tB_sb[it%2] ready
                v.wait_ge(s_pw, it + 1)          # ps_wbc[it%2] ready
                if it >= 2:
                    v.wait_ge(s_pc, it - 1)      # sB_sb[it%2] free (PE read)
                v.tensor_tensor(sB_sb[it % 2][:, :], tB_sb[it % 2][:, :],
                                ps_wbc[it % 2][:, 0:VBLK],
                                mybir.AluOpType.mult).then_inc(s_sb, 1)
            copies(NIT - 1)
            for c in range(NCH):
                v.wait_ge(s_mm, c + 1)
                if c >= 2:
                    v.wait_ge(s_rel, c - 1)      # acc_sb[c%2] free (ACT read)
                v.tensor_scalar_max(den_sb[c % 2][:, :], ps_B2[c % 2][:, 0:H_],
                                    1e-30)
                v.drain()
                v.reciprocal(rcp_sb[c % 2][:, :], den_sb[c % 2][:, :])
                v.drain()
                v.scalar_tensor_tensor(
                    acc_sb[c % 2][:, :], ps_A[c % 2][:, 0:O_],
                    rcp_sb[c % 2][:, 0:1], binit_sb[:, :],
                    mybir.AluOpType.mult, mybir.AluOpType.add)
                for h in range(1, H_):
                    v.drain()
                    e = v.scalar_tensor_tensor(
                        acc_sb[c % 2][:, :], ps_A[c % 2][:, h * O_:(h + 1) * O_],
                        rcp_sb[c % 2][:, h:h + 1], acc_sb[c % 2][:, :],
                        mybir.AluOpType.mult, mybir.AluOpType.add)
                e.then_inc(s_ep, 1)

        @block.sync
        def _(sy):
            sy.dma_start(out=idx_sb[:, :], in_=idxp[:, :]).then_inc(s_idx, 16)
            for bN in range(NBLK):
                sy.wait_ge(s_st, H_ * (bN + 1))
                lo, hi = bN * VBLK, min(VP, bN * VBLK + VBLK)
                full_j, rem = (hi - lo) // 128, (hi - lo) % 128
                if full_j:
                    sy.dma_start(
                        out=bass.AP(tbl_loc, lo * RW,
                                    [[RW, 128], [128 * RW, full_j], [1, RW]]),
                        in_=stage[bN % 2][:, 0:full_j, :]).then_inc(s_sd[bN % 2], 16)
                if rem:
                    sy.dma_start(
                        out=bass.AP(tbl_loc, (lo + full_j * 128) * RW,
                                    [[RW, rem], [1, RW]]),
                        in_=stage[bN % 2][0:rem, full_j, :]).then_inc(s_sd[bN % 2], 16)
            for c in range(NCH):
                sy.wait_ge(s_rel, c + 1)
                lo = c * 128
                rows = min(128, VP - lo)
                sy.dma_start(out=bass.AP(out_ext, lo * O_, [[O_, rows], [1, O_]]),
                             in_=ostage[c % 2][0:rows, :]).then_inc(s_o[c % 2], 16)
            sy.wait_ge(s_o[0], 16 * ((NCH + 1) // 2))
            if NCH > 1:
                sy.wait_ge(s_o[1], 16 * (NCH // 2))

    sctx.close()
    ctx.close()
    nc.compile()
    return nc


def prep_core_inputs(cfg: Cfg, x, W, a, b, adj_lst, r):
    """Host-side shard/layout prep for core r (index/layout only, no math)."""
    VP, VPAD, NCH = cfg.VP, cfg.VPAD, cfg.NCHUNK
    xs = np.ascontiguousarray(x[r * VP:(r + 1) * VP].T)
    adj = adj_lst[r * VP:(r + 1) * VP]
    rows = np.where(adj == cfg.V, VP,
                    (adj // VP) * VPAD + (adj % VP)).astype(np.int32)
    rows_p = np.full((NCH * 128, cfg.K), VP, np.int32)
    rows_p[:VP] = rows
    # gtile[p, g, :] = tbl[flat[g*128+p]]; edge (node 8g+i, k) at p=16i+k.
    # dma_gather flat order i reads idxs_sb[i%16, i//16] (16-part wrap,
    # replicated to all 8 gpsimd cores); two 1024-idx ops per chunk.
    lin = rows_p.reshape(NCH, 16, 128)           # [c, g, p] (p = i*16 + k)
    F = lin.reshape(NCH, 2, 64, 16)              # [c, half, s, q]: i = s*16+q
    B = F.transpose(0, 1, 3, 2)                  # [c, half, q, s]
    idx = np.ascontiguousarray(
        np.tile(B, (1, 1, 8, 1)).transpose(2, 0, 1, 3).reshape(128, NCH * 128)
    ).astype(np.int16)
    blkm = np.zeros((128, 248), np.float32)
    blkm[np.arange(128), 120 + np.arange(128) // 16] = 1.0
    sel = np.zeros((cfg.H, cfg.H * 128), np.float32)
    for h in range(cfg.H):
        sel[h, h * 128:(h + 1) * 128] = 1.0
    W_ = np.asarray(W, np.float32)
    WT = np.ascontiguousarray(
        W_.transpose(2, 0, 1).reshape(cfg.O, cfg.H, cfg.DC, 128))
    return {
        "xT": xs.astype(np.float32), "W": W_, "WT": WT,
        "aT": np.ascontiguousarray(np.asarray(a, np.float32).T),
        "b": np.asarray(b, np.float32),
        "eye": np.eye(128, dtype=np.float32), "blk": blkm, "idx": idx,
        "sel": sel,
    }


_GRAPH_CACHE = {}


def kernel(x, W, a, b, adj_lst, mask_index, _cfg=None, _trace=False):
    cfg = _cfg or Cfg()
    x = np.asarray(x)
    adj_lst = np.asarray(adj_lst)
    assert int(mask_index) == cfg.V
    key = (cfg.V, cfg.ncores)
    if key not in _GRAPH_CACHE:
        _GRAPH_CACHE[key] = build_graph(cfg)
    nc = _GRAPH_CACHE[key]
    in_maps = [prep_core_inputs(cfg, x, W, a, b, adj_lst, r)
               for r in range(cfg.ncores)]
    res = run_bass_kernel_spmd(nc, in_maps, list(range(cfg.ncores)),
                               trace=_trace)
    out = np.concatenate([res.results[r]["out"] for r in range(cfg.ncores)], 0)
    kernel._last_exec_ns = res.exec_time_ns
    return out


# revision 60
# speedup vs baseline: 1.2499x; 1.0030x over previous
"""Distributed GAT (nn_AdjGAT) kernel for 8 TRN2 NeuronCores.

Math: the reference's per-edge softmax logit depends ONLY on the source
node (scores[h,v,k] = attn[h, adj[v,k]]), so
    softmax_k -> w[n_k] / sum_j w[n_j],  w = exp(attn)
    head[h,v] = (sum_k w[h,n_k] * t[h,n_k]) / (sum_k w[h,n_k])
    out = relu(mean_h(head + b[h]))
Phase 1 (per core): build node-major table rows [w*t_0..w*t_3 | w_0..w_3]
(bf16, padded to 640 elems = 1280B).  attn is computed independently of t
via a3 = W_h @ a_h (device-precomputed), attn = a3^T x per 512-node block
for all 4 heads at once.  AllGather the row table.
Phase 2: per 128-node chunk, two 1024-row dma_gather ops fetch the 2048
neighbor rows (SWDGE descriptor generation on GpSimd is the bottleneck,
~8ns/row), then block-diagonal-ones matmuls sum each node's K=16 rows
(numerator AND denominator), then normalize/relu.  Padding slots point at
a hardwired zero row (w=0).
"""

import math
from contextlib import ExitStack

import numpy as np

import concourse.bass as bass
import concourse.bacc as bacc
import concourse.mybir as mybir
from concourse import library_config
from concourse.bass_utils import run_bass_kernel_spmd

F32 = mybir.dt.float32
BF16 = mybir.dt.bfloat16
I16 = mybir.dt.int16
I32 = mybir.dt.int32

V, D, K, O, H = 20000, 256, 16, 128, 4
NCORES = 8


class Cfg:
    def __init__(self, V=V, D=D, K=K, O=O, H=H, ncores=NCORES):
        assert V % ncores == 0
        self.V, self.D, self.K, self.O, self.H, self.ncores = V, D, K, O, H, ncores
        self.VP = V // ncores
        self.ZPAD = 12
        self.VPAD = self.VP + self.ZPAD
        self.VT = self.VPAD * ncores
        self.RW = H * O + 128          # 640 bf16 elems: [s (512) | w (4) | pad]
        self.NCHUNK = math.ceil(self.VP / 128)
        self.VBLK = 512 if self.VP >= 512 else 128 * math.ceil(self.VP / 128)
        self.NBLK = math.ceil(self.VP / self.VBLK)
        self.NJ = self.VBLK // 128
        self.VPF = self.NBLK * self.VBLK
        self.DC = D // 128
        self.VPH1 = self.VP // 2          # table half-1 rows per core
        self.VPH2 = self.VPAD - self.VPH1
        assert D % 128 == 0 and O == 128 and K == 16 and self.DC >= 1
        assert self.VT < 32000


def build_graph(cfg: Cfg, skip_ag=False):
    nc = bacc.Bacc()
    VP, VPAD, VT, RW, H_, O_, DC = (
        cfg.VP, cfg.VPAD, cfg.VT, cfg.RW, cfg.H, cfg.O, cfg.DC)
    NCH, NBLK, VBLK, NJ, VPF = cfg.NCHUNK, cfg.NBLK, cfg.VBLK, cfg.NJ, cfg.VPF
    SW = H_ * O_
    NE = 128 * 16
    NIT = NBLK * H_

    xT = nc.declare_dram_parameter("xT", [cfg.D, VP], F32, isOutput=False)
    Wp = nc.declare_dram_parameter("W", [H_, cfg.D, O_], F32, isOutput=False)
    WTp = nc.declare_dram_parameter("WT", [O_, H_, DC, 128], F32, isOutput=False)
    aTp = nc.declare_dram_parameter("aT", [O_, H_], F32, isOutput=False)
    bp = nc.declare_dram_parameter("b", [H_, O_], F32, isOutput=False)
    eyep = nc.declare_dram_parameter("eye", [128, 128], F32, isOutput=False)
    selp = nc.declare_dram_parameter("sel", [H_, H_ * 128], F32, isOutput=False)
    blkp = nc.declare_dram_parameter("blk", [128, 248], F32, isOutput=False)
    idxp = nc.declare_dram_parameter("idx", [128, NCH * 128], I16, isOutput=False)
    out_ext = nc.declare_dram_parameter("out", [VP, O_], F32, isOutput=True)

    tbl_loc = nc.dram_tensor("tbl_loc", [VPAD, RW], BF16)
    tbl = nc.dram_tensor("tbl", [VT, RW], BF16, addr_space="Shared")

    ctx = ExitStack()
    sb = lambda name, shape, dt: ctx.enter_context(nc.sbuf_tensor(name, shape, dt))
    xT_sb = sb("xT_sb", [128, DC, VPF], BF16)
    W_sb = sb("W_sb", [128, H_, DC, O_], BF16)
    WT_sb = sb("WT_sb", [128, H_, DC, 128], BF16)
    aT_sb = sb("aT_sb", [128, H_], BF16)
    a3_sb = sb("a3_sb", [128, DC * H_], BF16)
    b_sb = sb("b_sb", [H_, O_], BF16)
    eye_sb = sb("eye_sb", [128, 128], BF16)
    blk_sb = sb("blk_sb", [128, 248], BF16)
    idx_sb = sb("idx_sb", [128, NCH * 128], I16)
    ones4 = sb("ones4", [H_, O_], BF16)
    zero_sb = sb("zero_sb", [cfg.ZPAD, RW], BF16)
    w4 = [sb(f"w4_{i}", [H_, VBLK], BF16) for i in range(2)]
    sel_sb = sb("sel_sb", [H_, H_ * 128], BF16)
    tB_sb = [sb(f"tB_sb{i}", [128, VBLK], BF16) for i in range(2)]
    sB_sb = [sb(f"sB_sb{i}", [128, VBLK], BF16) for i in range(2)]
    stage = [sb(f"stage{i}", [128, NJ, RW], BF16) for i in range(2)]
    binit_sb = sb("binit_sb", [128, O_], F32)
    gtile = [sb(f"gtile{i}", [128, 16, RW], BF16) for i in range(2)]
    den_sb = [sb(f"den{i}", [128, H_], F32) for i in range(2)]
    rcp_sb = [sb(f"rcp{i}", [128, H_], F32) for i in range(2)]
    acc_sb = [sb(f"acc{i}", [128, O_], F32) for i in range(2)]
    ostage = [sb(f"ostage{i}", [128, O_], F32) for i in range(2)]

    # phase-1 PSUM (freed before phase-2 allocs; full 2KB banks each)
    ph1 = ExitStack()
    psa = lambda name, shape, dt: ph1.enter_context(nc.psum_tensor(name, shape, dt))
    ps_tB = [psa(f"ps_tB{i}", [128, 512], F32) for i in range(2)]
    ps_wbc = [psa(f"ps_wbc{i}", [128, 512], F32) for i in range(2)]
    ps_misc = [psa(f"ps_misc{i}", [128, 512], F32) for i in range(2)]
    ps_C = [psa(f"ps_C{i}", [128, 1024], BF16) for i in range(2)]
    ph1.close()
    ph2 = ExitStack()
    ps_A = [ph2.enter_context(nc.psum_tensor(f"ps_A{i}", [128, 512], F32))
            for i in range(2)]
    ps_B2 = [ph2.enter_context(nc.psum_tensor(f"ps_B2{i}", [128, 512], F32))
             for i in range(2)]
    ph2.close()

    blk_ndma = []
    for bN in range(NBLK):
        lo, hi = bN * VBLK, min(VP, bN * VBLK + VBLK)
        blk_ndma.append((1 if (hi - lo) // 128 else 0) + (1 if (hi - lo) % 128 else 0))
    VPH1 = cfg.VPH1
    nblk1 = math.ceil(VPH1 / VBLK)

    sctx = ExitStack()
    sem = lambda n: sctx.enter_context(nc.semaphore(n))
    (s_ld, s_ld2, s_idx, s_dvi, s_bini, s_bcp, s_a3, s_a3c, s_tb, s_tbsb,
     s_attn, s_w, s_pw, s_sb, s_pc, s_w4t, s_st, s_zr, s_cc, s_mm, s_ep,
     s_rel) = [sem(n) for n in (
        "s_ld", "s_ld2", "s_idx", "s_dvi", "s_bini", "s_bcp", "s_a3",
        "s_a3c", "s_tb", "s_tbsb", "s_attn", "s_w", "s_pw", "s_sb", "s_pc",
        "s_w4t", "s_st", "s_zr", "s_cc", "s_mm", "s_ep", "s_rel")]
    s_xb = [sem(f"s_xb{i}") for i in range(NBLK)]
    s_ldb, s_lda, s_lds, s_ldwt, s_lde = [sem(n) for n in (
        "s_ldb", "s_lda", "s_lds", "s_ldwt", "s_lde")]
    s_g = [sem("s_g0"), sem("s_g1")]
    s_sd = [sem("s_sd0"), sem("s_sd1")]
    s_o = [sem("s_o0"), sem("s_o1")]
    # cumulative stage-DMA counts per parity (for safe per-parity waits)
    cum_par = []
    c0 = c1 = 0
    for bN in range(NBLK):
        if bN % 2 == 0:
            c0 += blk_ndma[bN]
        else:
            c1 += blk_ndma[bN]
        cum_par.append((c0, c1))
    with nc.Block() as block:
        @block.gpsimd
        def _(g):
            g.dma_start(out=b_sb[:, :], in_=bass.AP(
                bp, 0, [[O_, H_], [1, O_]])).then_inc(s_ldb, 16)
            g.dma_start(out=aT_sb[:, :], in_=bass.AP(
                aTp, 0, [[H_, 128], [1, H_]])).then_inc(s_lda, 16)
            g.dma_start(out=sel_sb[:, :], in_=selp[:, :]).then_inc(s_lds, 16)
            g.dma_start(out=W_sb[:, :, :, :], in_=bass.AP(
                Wp, 0, [[O_, 128], [cfg.D * O_, H_], [128 * O_, DC], [1, O_]]
            )).then_inc(s_ld, 16)
            lo, hi = 0, min(VP, VBLK)
            g.dma_start(out=xT_sb[:, :, lo:hi], in_=bass.AP(
                xT, lo, [[VP, 128], [128 * VP, DC], [1, hi - lo]])
            ).then_inc(s_xb[0], 16)
            g.dma_start(out=WT_sb[:, :, :, :], in_=bass.AP(
                WTp, 0, [[H_ * DC * 128, 128], [DC * 128, H_], [128, DC], [1, 128]]
            )).then_inc(s_ldwt, 16)
            g.dma_start(out=eye_sb[:, :], in_=eyep[:, :]).then_inc(s_lde, 16)
            for bN in range(1, NBLK):
                lo, hi = bN * VBLK, min(VP, bN * VBLK + VBLK)
                g.dma_start(out=xT_sb[:, :, lo:hi], in_=bass.AP(
                    xT, lo, [[VP, 128], [128 * VP, DC], [1, hi - lo]])
                ).then_inc(s_xb[bN], 16)
            g.dma_start(out=blk_sb[:, :], in_=blkp[:, :]).then_inc(s_ld2, 16)
            g.wait_ge(s_dvi, 1)
            g.dma_start(out=bass.AP(tbl_loc, VP * RW, [[RW, cfg.ZPAD], [1, RW]]),
                        in_=zero_sb[:, :]).then_inc(s_zr, 16)
            g.wait_ge(s_zr, 16)
            # half-1 AllGather fires once blocks [0, nblk1) are staged
            c0h = sum(blk_ndma[b] for b in range(nblk1) if b % 2 == 0)
            c1h = sum(blk_ndma[b] for b in range(nblk1) if b % 2 == 1)
            if c0h:
                g.wait_ge(s_sd[0], 16 * c0h)
            if c1h:
                g.wait_ge(s_sd[1], 16 * c1h)
            NC = cfg.ncores
            if skip_ag:
                g.dma_start(out=tbl[0:NC * VPH1, :],
                            in_=tbl_loc[0:VPH1, :]).then_inc(s_cc, 16)
            else:
                g.collective_compute(
                    "AllGather", mybir.AluOpType.bypass,
                    replica_groups=[list(range(cfg.ncores))],
                    ins=[tbl_loc[0:VPH1, :]], outs=[tbl[0:NC * VPH1, :]],
                ).then_inc(s_cc)
            g.wait_ge(s_sd[0], 16 * cum_par[-1][0])
            if cum_par[-1][1]:
                g.wait_ge(s_sd[1], 16 * cum_par[-1][1])
            if skip_ag:
                g.dma_start(out=tbl[NC * VPH1:VT, :],
                            in_=tbl_loc[VPH1:VPAD, :]).then_inc(s_cc, 16)
                g.wait_ge(s_cc, 32)
            else:
                g.collective_compute(
                    "AllGather", mybir.AluOpType.bypass,
                    replica_groups=[list(range(cfg.ncores))],
                    ins=[tbl_loc[VPH1:VPAD, :]], outs=[tbl[NC * VPH1:VT, :]],
                ).then_inc(s_cc)
                g.wait_ge(s_cc, 2)
            g.wait_ge(s_idx, 16)
            for c in range(NCH):
                if c >= 2:
                    g.wait_ge(s_mm, c - 1)
                for hf in range(2):
                    g.dma_gather(
                        out_ap=gtile[c % 2][:, hf * 8:hf * 8 + 8, :],
                        in_ap=tbl[:, :],
                        idxs_ap=idx_sb[:, c * 128 + hf * 64:c * 128 + hf * 64 + 64],
                        num_idxs=NE // 2,
                        num_idxs_reg=NE // 2,
                        elem_size=RW,
                    ).then_inc(s_g[c % 2], 16)

        @block.tensor
        def _(pe):
            def transp(t):
                pe.wait_ge(s_sb, t + 1)          # sB_sb[t%2] ready (DVE done)
                if t >= 2:
                    pe.wait_ge(s_st, t - 1)      # ps_C[t%2] s-cols free (DVE)
                for j in range(NJ):
                    e = pe.transpose(ps_C[t % 2][:, j * 128:(j + 1) * 128],
                                     sB_sb[t % 2][:, j * 128:(j + 1) * 128],
                                     eye_sb[:, :])
                e.then_inc(s_pc, 1)

            pe.wait_ge(s_ldb, 16)
            pe.wait_ge(s_dvi, 1)
            pe.matmul(ps_misc[0][:, 0:O_], ones4[:, :], b_sb[:, :],
                      start=True, stop=True).then_inc(s_bini, 1)
            pe.wait_ge(s_ldwt, 16)
            pe.wait_ge(s_lda, 16)
            for c in range(DC):
                for h in range(H_):
                    e = pe.matmul(ps_misc[1][:, c * H_ + h:c * H_ + h + 1],
                                  WT_sb[:, h, c, :], aT_sb[:, h:h + 1],
                                  start=True, stop=True)
            e.then_inc(s_a3, 1)
            pe.wait_ge(s_ld, 16)                 # W_sb
            pe.wait_ge(s_lds, 16)                # sel_sb
            pe.wait_ge(s_lde, 16)                # eye_sb

            def attn(b):
                pe.wait_ge(s_xb[b], 16)
                vl = b * VBLK
                for c in range(DC):
                    mm = pe.matmul(ps_misc[b % 2][0:4, 0:VBLK],
                                   a3_sb[:, c * H_:(c + 1) * H_],
                                   xT_sb[:, c, vl:vl + VBLK],
                                   start=(c == 0), stop=(c == DC - 1))
                mm.then_inc(s_attn, 1)

            pe.wait_ge(s_bcp, 1)                 # ps_misc[0] binit copied out
            pe.wait_ge(s_a3c, 1)                 # a3_sb ready (+ps_misc[1] free)
            attn(0)
            for it in range(NIT):
                bN, h = divmod(it, H_)
                vlo = bN * VBLK
                if it >= 2:
                    transp(it - 2)
                if h == 3 and bN + 1 < NBLK:
                    if bN >= 1:
                        pe.wait_ge(s_w, bN)      # attn region read (exp bN-1)
                        pe.wait_ge(s_st, 4 * bN)  # pw cols of block bN-1 copied
                    attn(bN + 1)
                if it >= 2:
                    pe.wait_ge(s_tbsb, it - 1)   # ps_tB[it%2] free (ACT copied)
                mm = pe.matmul(ps_tB[it % 2][:, 0:VBLK], W_sb[:, h, 0, :],
                               xT_sb[:, 0, vlo:vlo + VBLK],
                               start=True, stop=(DC == 1))
                for c in range(1, DC):
                    mm = pe.matmul(ps_tB[it % 2][:, 0:VBLK], W_sb[:, h, c, :],
                                   xT_sb[:, c, vlo:vlo + VBLK],
                                   start=False, stop=(c == DC - 1))
                mm.then_inc(s_tb, 1)
                pe.wait_ge(s_w, bN + 1)          # w4[bN%2] ready (exp done)
                if it >= 2:
                    pe.wait_ge(s_sb, it - 1)     # ps_wbc[it%2] free (DVE)
                if it >= 8:
                    pe.wait_ge(s_st, it - 7)     # ps_misc[bN%2] pw cols copied
                pe.matmul(ps_wbc[it % 2][:, 0:VBLK],
                          sel_sb[:, h * 128:(h + 1) * 128],
                          w4[bN % 2][:, :], start=True, stop=True)
                for j in range(NJ):
                    e = pe.matmul(
                        ps_misc[bN % 2][:, 4 * h + j:4 * h + j + 1],
                        w4[bN % 2][:, j * 128:(j + 1) * 128],
                        sel_sb[:, h * 128:h * 128 + 1], start=True, stop=True)
                e.then_inc(s_pw, 1)
            transp(NIT - 2)
            transp(NIT - 1)
            pe.wait_ge(s_ld2, 16)
            for c in range(NCH):
                pe.wait_ge(s_g[c % 2], 32 * (c // 2 + 1))
                if c >= 2:
                    pe.wait_ge(s_ep, c - 1)      # ps_A/ps_B2[c%2] free (DVE)
                for gi in range(16):
                    pe.matmul(ps_A[c % 2][:, 0:SW],
                              blk_sb[:, 120 - 8 * gi:248 - 8 * gi],
                              gtile[c % 2][:, gi, 0:SW],
                              start=(gi == 0), stop=(gi == 15))
                for gi in range(16):
                    e = pe.matmul(ps_B2[c % 2][:, 0:H_],
                                  blk_sb[:, 120 - 8 * gi:248 - 8 * gi],
                                  gtile[c % 2][:, gi, SW:SW + H_],
                                  start=(gi == 0), stop=(gi == 15))
                e.then_inc(s_mm, 1)

        @block.scalar
        def _(s):
            s.wait_ge(s_a3, 1)
            s.activation(a3_sb[:, :], ps_misc[1][:, 0:DC * H_],
                         mybir.ActivationFunctionType.Copy).then_inc(s_a3c, 1)
            for it in range(NIT):
                bN, h = divmod(it, H_)
                if h == 0:
                    s.wait_ge(s_attn, bN + 1)
                    if bN >= 2:
                        s.wait_ge(s_pw, 4 * bN - 4)  # w4[bN%2] free (PE read)
                    s.activation(w4[bN % 2][:, :],
                                 ps_misc[bN % 2][0:4, 0:VBLK],
                                 mybir.ActivationFunctionType.Exp
                                 ).then_inc(s_w, 1)
                s.wait_ge(s_tb, it + 1)
                if it >= 2:
                    s.wait_ge(s_sb, it - 1)      # tB_sb[it%2] free (DVE read)
                s.activation(tB_sb[it % 2][:, :], ps_tB[it % 2][:, 0:VBLK],
                             mybir.ActivationFunctionType.Copy).then_inc(s_tbsb, 1)
            for c in range(NCH):
                s.wait_ge(s_ep, c + 1)
                if c >= 2:
                    s.wait_ge(s_o[c % 2], 16 * (c // 2))  # ostage free
                s.activation(ostage[c % 2][:, :], acc_sb[c % 2][:, :],
                             mybir.ActivationFunctionType.Relu,
                             scale=1.0 / H_).then_inc(s_rel, 1)

        @block.vector
        def _(v):
            def copies(t):
                b, h = divmod(t, H_)
                v.wait_ge(s_pc, t + 1)           # ps_C[t%2] s-cols ready
                if h == 0 and b >= 2:
                    v.wait_ge(s_sd[b % 2], 16 * cum_par[b - 2][b % 2])
                v.tensor_copy(
                    stage[b % 2][:, :, h * O_:(h + 1) * O_],
                    bass.AP(ps_C[t % 2], 0, [[1024, 128], [128, NJ], [1, 128]]))
                v.tensor_copy(
                    stage[b % 2][:, :, SW + h:SW + h + 1],
                    bass.AP(ps_misc[b % 2], 4 * h, [[512, 128], [1, NJ], [1, 1]])
                ).then_inc(s_st, 1)

            v.memset(ones4[:, :], 1.0)
            v.memset(stage[0][:, :, SW + H_:RW], 0.0)
            v.memset(stage[1][:, :, SW + H_:RW], 0.0)
            if VPF > VP:
                v.memset(xT_sb[:, :, VP:VPF], 0.0)
            v.memset(zero_sb[:, :], 0.0).then_inc(s_dvi, 1)
            v.wait_ge(s_bini, 1)
            v.tensor_copy(binit_sb[:, :], ps_misc[0][:, 0:O_]).then_inc(s_bcp, 1)
            for it in range(NIT):
                if it >= 1:
                    copies(it - 1)
                v.wait_ge(s_tbsb, it + 1)        # tB_sb[it%2] ready
                v.wait_ge(s_pw, it + 1)          # ps_wbc[it%2] ready
                if it >= 2:
                    v.wait_ge(s_pc, it - 1)      # sB_sb[it%2] free (PE read)
                v.tensor_tensor(sB_sb[it % 2][:, :], tB_sb[it % 2][:, :],
                                ps_wbc[it % 2][:, 0:VBLK],
                                mybir.AluOpType.mult).then_inc(s_sb, 1)
            copies(NIT - 1)
            for c in range(NCH):
                v.wait_ge(s_mm, c + 1)
                if c >= 2:
                    v.wait_ge(s_rel, c - 1)      # acc_sb[c%2] free (ACT read)
                v.tensor_scalar_max(den_sb[c % 2][:, :], ps_B2[c % 2][:, 0:H_],
                                    1e-30)
                v.drain()
                v.reciprocal(rcp_sb[c % 2][:, :], den_sb[c % 2][:, :])
                v.drain()
                v.scalar_tensor_tensor(
                    acc_sb[c % 2][:, :], ps_A[c % 2][:, 0:O_],
                    rcp_sb[c % 2][:, 0:1], binit_sb[:, :],
                    mybir.AluOpType.mult, mybir.AluOpType.add)
                for h in range(1, H_):
                    v.drain()
                    e = v.scalar_tensor_tensor(
                        acc_sb[c % 2][:, :], ps_A[c % 2][:, h * O_:(h + 1) * O_],
                        rcp_sb[c % 2][:, h:h + 1], acc_sb[c % 2][:, :],
                        mybir.AluOpType.mult, mybir.AluOpType.add)
                e.then_inc(s_ep, 1)

        @block.sync
        def _(sy):
            sy.dma_start(out=idx_sb[:, :], in_=idxp[:, :]).then_inc(s_idx, 16)
            for bN in range(NBLK):
                sy.wait_ge(s_st, H_ * (bN + 1))
                lo, hi = bN * VBLK, min(VP, bN * VBLK + VBLK)
                full_j, rem = (hi - lo) // 128, (hi - lo) % 128
                if full_j:
                    sy.dma_start(
                        out=bass.AP(tbl_loc, lo * RW,
                                    [[RW, 128], [128 * RW, full_j], [1, RW]]),
                        in_=stage[bN % 2][:, 0:full_j, :]).then_inc(s_sd[bN % 2], 16)
                if rem:
                    sy.dma_start(
                        out=bass.AP(tbl_loc, (lo + full_j * 128) * RW,
                                    [[RW, rem], [1, RW]]),
                        in_=stage[bN % 2][0:rem, full_j, :]).then_inc(s_sd[bN % 2], 16)
            for c in range(NCH):
                sy.wait_ge(s_rel, c + 1)
                lo = c * 128
                rows = min(128, VP - lo)
                sy.dma_start(out=bass.AP(out_ext, lo * O_, [[O_, rows], [1, O_]]),
                             in_=ostage[c % 2][0:rows, :]).then_inc(s_o[c % 2], 16)
            sy.wait_ge(s_o[0], 16 * ((NCH + 1) // 2))
            if NCH > 1:
                sy.wait_ge(s_o[1], 16 * (NCH // 2))

    sctx.close()
    ctx.close()
    nc.compile()
    return nc


def prep_core_inputs(cfg: Cfg, x, W, a, b, adj_lst, r):
    """Host-side shard/layout prep for core r (index/layout only, no math)."""
    VP, VPAD, NCH = cfg.VP, cfg.VPAD, cfg.NCHUNK
    xs = np.ascontiguousarray(x[r * VP:(r + 1) * VP].T)
    adj = adj_lst[r * VP:(r + 1) * VP]
    # table row ids for the 2-half AllGather layout:
    #   [core-major half-1 rows | core-major half-2 rows (+ zero pad)]
    NC, H1, H2 = cfg.ncores, cfg.VPH1, cfg.VPH2
    rr, q = adj // VP, adj % VP
    rows = np.where(q < H1, rr * H1 + q,
                    NC * H1 + rr * H2 + (q - H1))
    rows = np.where(adj == cfg.V, NC * H1 + (VP - H1), rows).astype(np.int32)
    rows_p = np.full((NCH * 128, cfg.K), NC * H1 + (VP - H1), np.int32)
    rows_p[:VP] = rows
    # gtile[p, g, :] = tbl[flat[g*128+p]]; edge (node 8g+i, k) at p=16i+k.
    # dma_gather flat order i reads idxs_sb[i%16, i//16] (16-part wrap,
    # replicated to all 8 gpsimd cores); two 1024-idx ops per chunk.
    lin = rows_p.reshape(NCH, 16, 128)           # [c, g, p] (p = i*16 + k)
    F = lin.reshape(NCH, 2, 64, 16)              # [c, half, s, q]: i = s*16+q
    B = F.transpose(0, 1, 3, 2)                  # [c, half, q, s]
    idx = np.ascontiguousarray(
        np.tile(B, (1, 1, 8, 1)).transpose(2, 0, 1, 3).reshape(128, NCH * 128)
    ).astype(np.int16)
    blkm = np.zeros((128, 248), np.float32)
    blkm[np.arange(128), 120 + np.arange(128) // 16] = 1.0
    sel = np.zeros((cfg.H, cfg.H * 128), np.float32)
    for h in range(cfg.H):
        sel[h, h * 128:(h + 1) * 128] = 1.0
    W_ = np.asarray(W, np.float32)
    WT = np.ascontiguousarray(
        W_.transpose(2, 0, 1).reshape(cfg.O, cfg.H, cfg.DC, 128))
    return {
        "xT": xs.astype(np.float32), "W": W_, "WT": WT,
        "aT": np.ascontiguousarray(np.asarray(a, np.float32).T),
        "b": np.asarray(b, np.float32),
        "eye": np.eye(128, dtype=np.float32), "blk": blkm, "idx": idx,
        "sel": sel,
    }


_GRAPH_CACHE = {}


def kernel(x, W, a, b, adj_lst, mask_index, _cfg=None, _trace=False):
    cfg = _cfg or Cfg()
    x = np.asarray(x)
    adj_lst = np.asarray(adj_lst)
    assert int(mask_index) == cfg.V
    key = (cfg.V, cfg.ncores)
    if key not in _GRAPH_CACHE:
        _GRAPH_CACHE[key] = build_graph(cfg)
    nc = _GRAPH_CACHE[key]
    in_maps = [prep_core_inputs(cfg, x, W, a, b, adj_lst, r)
               for r in range(cfg.ncores)]
    res = run_bass_kernel_spmd(nc, in_maps, list(range(cfg.ncores)),
                               trace=_trace)
    out = np.concatenate([res.results[r]["out"] for r in range(cfg.ncores)], 0)
    kernel._last_exec_ns = res.exec_time_ns
    return out


# revision 65
# speedup vs baseline: 1.2889x; 1.0312x over previous
"""Distributed GAT (nn_AdjGAT) kernel for 8 TRN2 NeuronCores.

Math: the reference's per-edge softmax logit depends ONLY on the source
node (scores[h,v,k] = attn[h, adj[v,k]]), so
    softmax_k -> w[n_k] / sum_j w[n_j],  w = exp(attn)
    head[h,v] = (sum_k w[h,n_k] * t[h,n_k]) / (sum_k w[h,n_k])
    out = relu(mean_h(head + b[h]))
Phase 1 (per core): build node-major table rows [w*t_0..w*t_3 | w_0..w_3]
(bf16, padded to 640 elems = 1280B).  attn is computed independently of t
via a3 = W_h @ a_h (device-precomputed), attn = a3^T x per 512-node block
for all 4 heads at once.  AllGather the row table.
Phase 2: per 128-node chunk, two 1024-row dma_gather ops fetch the 2048
neighbor rows (SWDGE descriptor generation on GpSimd is the bottleneck,
~8ns/row), then block-diagonal-ones matmuls sum each node's K=16 rows
(numerator AND denominator), then normalize/relu.  Padding slots point at
a hardwired zero row (w=0).
"""

import math
from contextlib import ExitStack

import numpy as np

import concourse.bass as bass
import concourse.bacc as bacc
import concourse.mybir as mybir
from concourse import library_config
from concourse.bass_utils import run_bass_kernel_spmd

F32 = mybir.dt.float32
BF16 = mybir.dt.bfloat16
I16 = mybir.dt.int16
I32 = mybir.dt.int32

V, D, K, O, H = 20000, 256, 16, 128, 4
NCORES = 8


class Cfg:
    def __init__(self, V=V, D=D, K=K, O=O, H=H, ncores=NCORES):
        assert V % ncores == 0
        self.V, self.D, self.K, self.O, self.H, self.ncores = V, D, K, O, H, ncores
        self.VP = V // ncores
        self.ZPAD = 12
        self.VPAD = self.VP + self.ZPAD
        self.VT = self.VPAD * ncores
        self.RW = H * O + 128          # 640 bf16 elems: [s (512) | w (4) | pad]
        self.NCHUNK = math.ceil(self.VP / 128)
        self.VBLK = 512 if self.VP >= 512 else 128 * math.ceil(self.VP / 128)
        self.NBLK = math.ceil(self.VP / self.VBLK)
        self.NJ = self.VBLK // 128
        self.VPF = self.NBLK * self.VBLK
        self.DC = D // 128
        self.VPH1 = self.VP // 2          # table half-1 rows per core
        self.VPH2 = self.VPAD - self.VPH1
        assert D % 128 == 0 and O == 128 and K == 16 and self.DC >= 1
        assert self.VT < 32000


def build_graph(cfg: Cfg, skip_ag=False):
    nc = bacc.Bacc()
    VP, VPAD, VT, RW, H_, O_, DC = (
        cfg.VP, cfg.VPAD, cfg.VT, cfg.RW, cfg.H, cfg.O, cfg.DC)
    NCH, NBLK, VBLK, NJ, VPF = cfg.NCHUNK, cfg.NBLK, cfg.VBLK, cfg.NJ, cfg.VPF
    SW = H_ * O_
    NE = 128 * 16
    NIT = NBLK * H_

    xT = nc.declare_dram_parameter("xT", [cfg.D, VP], F32, isOutput=False)
    Wp = nc.declare_dram_parameter("W", [H_, cfg.D, O_], F32, isOutput=False)
    WTp = nc.declare_dram_parameter("WT", [O_, H_, DC, 128], F32, isOutput=False)
    aTp = nc.declare_dram_parameter("aT", [O_, H_], F32, isOutput=False)
    bp = nc.declare_dram_parameter("b", [H_, O_], F32, isOutput=False)
    eyep = nc.declare_dram_parameter("eye", [128, 128], F32, isOutput=False)
    selp = nc.declare_dram_parameter("sel", [H_, H_ * 128], F32, isOutput=False)
    blkp = nc.declare_dram_parameter("blk", [128, 248], F32, isOutput=False)
    idxp = nc.declare_dram_parameter("idx", [128, NCH * 128], I16, isOutput=False)
    out_ext = nc.declare_dram_parameter("out", [VP, O_], F32, isOutput=True)

    tbl_loc = nc.dram_tensor("tbl_loc", [VPAD, RW], BF16)
    tbl = nc.dram_tensor("tbl", [VT, RW], BF16, addr_space="Shared")

    ctx = ExitStack()
    sb = lambda name, shape, dt: ctx.enter_context(nc.sbuf_tensor(name, shape, dt))
    xT_sb = sb("xT_sb", [128, DC, VPF], BF16)
    W_sb = sb("W_sb", [128, H_, DC, O_], BF16)
    WT_sb = sb("WT_sb", [128, H_, DC, 128], BF16)
    aT_sb = sb("aT_sb", [128, H_], BF16)
    a3_sb = sb("a3_sb", [128, DC * H_], BF16)
    b_sb = sb("b_sb", [H_, O_], BF16)
    eye_sb = sb("eye_sb", [128, 128], BF16)
    blk_sb = sb("blk_sb", [128, 248], BF16)
    idx_sb = sb("idx_sb", [128, NCH * 128], I16)
    ones4 = sb("ones4", [H_, O_], BF16)
    zero_sb = sb("zero_sb", [cfg.ZPAD, RW], BF16)
    w4 = [sb(f"w4_{i}", [H_, VBLK], BF16) for i in range(2)]
    sel_sb = sb("sel_sb", [H_, H_ * 128], BF16)
    tB_sb = [sb(f"tB_sb{i}", [128, VBLK], BF16) for i in range(2)]
    sB_sb = [sb(f"sB_sb{i}", [128, VBLK], BF16) for i in range(2)]
    stage = [sb(f"stage{i}", [128, NJ, RW], BF16) for i in range(2)]
    binit_sb = sb("binit_sb", [128, O_], F32)
    gtile = [sb(f"gtile{i}", [128, 16, RW], BF16) for i in range(2)]
    den_sb = [sb(f"den{i}", [128, H_], F32) for i in range(2)]
    rcp_sb = [sb(f"rcp{i}", [128, H_], F32) for i in range(2)]
    acc_sb = [sb(f"acc{i}", [128, O_], F32) for i in range(2)]
    ostage = [sb(f"ostage{i}", [128, O_], F32) for i in range(2)]

    # phase-1 PSUM (freed before phase-2 allocs; full 2KB banks each)
    ph1 = ExitStack()
    psa = lambda name, shape, dt: ph1.enter_context(nc.psum_tensor(name, shape, dt))
    ps_tB = [psa(f"ps_tB{i}", [128, 512], F32) for i in range(2)]
    ps_wbc = [psa(f"ps_wbc{i}", [128, 512], F32) for i in range(2)]
    ps_misc = [psa(f"ps_misc{i}", [128, 512], F32) for i in range(2)]
    ps_C = [psa(f"ps_C{i}", [128, 1024], BF16) for i in range(2)]
    ph1.close()
    ph2 = ExitStack()
    ps_A = [ph2.enter_context(nc.psum_tensor(f"ps_A{i}", [128, 512], F32))
            for i in range(2)]
    ps_B2 = [ph2.enter_context(nc.psum_tensor(f"ps_B2{i}", [128, 512], F32))
             for i in range(2)]
    ph2.close()

    blk_ndma = []
    for bN in range(NBLK):
        lo, hi = bN * VBLK, min(VP, bN * VBLK + VBLK)
        blk_ndma.append((1 if (hi - lo) // 128 else 0) + (1 if (hi - lo) % 128 else 0))
    VPH1 = cfg.VPH1
    nblk1 = math.ceil(VPH1 / VBLK)

    sctx = ExitStack()
    sem = lambda n: sctx.enter_context(nc.semaphore(n))
    (s_ld, s_ld2, s_idx, s_dvi, s_bini, s_bcp, s_a3, s_a3c, s_tb, s_tbsb,
     s_attn, s_w, s_pw, s_sb, s_pc, s_w4t, s_st, s_zr, s_cc, s_mm, s_ep,
     s_rel) = [sem(n) for n in (
        "s_ld", "s_ld2", "s_idx", "s_dvi", "s_bini", "s_bcp", "s_a3",
        "s_a3c", "s_tb", "s_tbsb", "s_attn", "s_w", "s_pw", "s_sb", "s_pc",
        "s_w4t", "s_st", "s_zr", "s_cc", "s_mm", "s_ep", "s_rel")]
    s_xb = [sem(f"s_xb{i}") for i in range(NBLK)]
    s_ldb, s_lda, s_lds, s_ldwt, s_lde, s_ldw2 = [sem(n) for n in (
        "s_ldb", "s_lda", "s_lds", "s_ldwt", "s_lde", "s_ldw2")]
    s_g = [sem("s_g0"), sem("s_g1")]
    s_sd = [sem("s_sd0"), sem("s_sd1")]
    s_o = [sem("s_o0"), sem("s_o1")]
    # cumulative stage-DMA counts per parity (for safe per-parity waits)
    cum_par = []
    c0 = c1 = 0
    for bN in range(NBLK):
        if bN % 2 == 0:
            c0 += blk_ndma[bN]
        else:
            c1 += blk_ndma[bN]
        cum_par.append((c0, c1))
    with nc.Block() as block:
        @block.gpsimd
        def _(g):
            g.dma_start(out=b_sb[:, :], in_=bass.AP(
                bp, 0, [[O_, H_], [1, O_]])).then_inc(s_ldb, 16)
            g.dma_start(out=aT_sb[:, :], in_=bass.AP(
                aTp, 0, [[H_, 128], [1, H_]])).then_inc(s_lda, 16)
            g.dma_start(out=sel_sb[:, :], in_=selp[:, :]).then_inc(s_lds, 16)
            g.dma_start(out=W_sb[:, 0:2, :, :], in_=bass.AP(
                Wp, 0, [[O_, 128], [cfg.D * O_, 2], [128 * O_, DC], [1, O_]]
            )).then_inc(s_ld, 16)
            lo, hi = 0, min(VP, VBLK)
            g.dma_start(out=xT_sb[:, :, lo:hi], in_=bass.AP(
                xT, lo, [[VP, 128], [128 * VP, DC], [1, hi - lo]])
            ).then_inc(s_xb[0], 16)
            g.dma_start(out=W_sb[:, 2:4, :, :], in_=bass.AP(
                Wp, 2 * cfg.D * O_, [[O_, 128], [cfg.D * O_, 2], [128 * O_, DC],
                                     [1, O_]])).then_inc(s_ldw2, 16)
            g.dma_start(out=WT_sb[:, :, :, :], in_=bass.AP(
                WTp, 0, [[H_ * DC * 128, 128], [DC * 128, H_], [128, DC], [1, 128]]
            )).then_inc(s_ldwt, 16)
            g.dma_start(out=eye_sb[:, :], in_=eyep[:, :]).then_inc(s_lde, 16)
            for bN in range(1, NBLK):
                lo, hi = bN * VBLK, min(VP, bN * VBLK + VBLK)
                g.dma_start(out=xT_sb[:, :, lo:hi], in_=bass.AP(
                    xT, lo, [[VP, 128], [128 * VP, DC], [1, hi - lo]])
                ).then_inc(s_xb[bN], 16)
            g.dma_start(out=blk_sb[:, :], in_=blkp[:, :]).then_inc(s_ld2, 16)
            g.wait_ge(s_dvi, 1)
            g.dma_start(out=bass.AP(tbl_loc, VP * RW, [[RW, cfg.ZPAD], [1, RW]]),
                        in_=zero_sb[:, :]).then_inc(s_zr, 16)
            g.wait_ge(s_zr, 16)
            # preload the dma_gather ucode library while waiting on stage/AG
            g.load_library(library_config.mlp)
            # half-1 AllGather fires once blocks [0, nblk1) are staged
            c0h = sum(blk_ndma[b] for b in range(nblk1) if b % 2 == 0)
            c1h = sum(blk_ndma[b] for b in range(nblk1) if b % 2 == 1)
            if c0h:
                g.wait_ge(s_sd[0], 16 * c0h)
            if c1h:
                g.wait_ge(s_sd[1], 16 * c1h)
            NC = cfg.ncores
            if skip_ag:
                g.dma_start(out=tbl[0:NC * VPH1, :],
                            in_=tbl_loc[0:VPH1, :]).then_inc(s_cc, 16)
            else:
                g.collective_compute(
                    "AllGather", mybir.AluOpType.bypass,
                    replica_groups=[list(range(cfg.ncores))],
                    ins=[tbl_loc[0:VPH1, :]], outs=[tbl[0:NC * VPH1, :]],
                ).then_inc(s_cc)
            g.wait_ge(s_sd[0], 16 * cum_par[-1][0])
            if cum_par[-1][1]:
                g.wait_ge(s_sd[1], 16 * cum_par[-1][1])
            if skip_ag:
                g.dma_start(out=tbl[NC * VPH1:VT, :],
                            in_=tbl_loc[VPH1:VPAD, :]).then_inc(s_cc, 16)
                g.wait_ge(s_cc, 32)
            else:
                g.collective_compute(
                    "AllGather", mybir.AluOpType.bypass,
                    replica_groups=[list(range(cfg.ncores))],
                    ins=[tbl_loc[VPH1:VPAD, :]], outs=[tbl[NC * VPH1:VT, :]],
                ).then_inc(s_cc)
                g.wait_ge(s_cc, 2)
            g.wait_ge(s_idx, 16)
            for c in range(NCH):
                if c >= 2:
                    g.wait_ge(s_mm, c - 1)
                for hf in range(2):
                    g.dma_gather(
                        out_ap=gtile[c % 2][:, hf * 8:hf * 8 + 8, :],
                        in_ap=tbl[:, :],
                        idxs_ap=idx_sb[:, c * 128 + hf * 64:c * 128 + hf * 64 + 64],
                        num_idxs=NE // 2,
                        num_idxs_reg=NE // 2,
                        elem_size=RW,
                    ).then_inc(s_g[c % 2], 16)

        @block.tensor
        def _(pe):
            def transp(t):
                pe.wait_ge(s_sb, t + 1)          # sB_sb[t%2] ready (DVE done)
                if t >= 2:
                    pe.wait_ge(s_st, t - 1)      # ps_C[t%2] s-cols free (DVE)
                for j in range(NJ):
                    e = pe.transpose(ps_C[t % 2][:, j * 128:(j + 1) * 128],
                                     sB_sb[t % 2][:, j * 128:(j + 1) * 128],
                                     eye_sb[:, :])
                e.then_inc(s_pc, 1)

            pe.wait_ge(s_ldb, 16)
            pe.wait_ge(s_dvi, 1)
            pe.matmul(ps_misc[0][:, 0:O_], ones4[:, :], b_sb[:, :],
                      start=True, stop=True).then_inc(s_bini, 1)
            pe.wait_ge(s_ldwt, 16)
            pe.wait_ge(s_lda, 16)
            for c in range(DC):
                for h in range(H_):
                    e = pe.matmul(ps_misc[1][:, c * H_ + h:c * H_ + h + 1],
                                  WT_sb[:, h, c, :], aT_sb[:, h:h + 1],
                                  start=True, stop=True)
            e.then_inc(s_a3, 1)
            pe.wait_ge(s_ld, 16)                 # W_sb
            pe.wait_ge(s_lds, 16)                # sel_sb
            pe.wait_ge(s_lde, 16)                # eye_sb

            def attn(b):
                pe.wait_ge(s_xb[b], 16)
                vl = b * VBLK
                for c in range(DC):
                    mm = pe.matmul(ps_misc[b % 2][0:4, 0:VBLK],
                                   a3_sb[:, c * H_:(c + 1) * H_],
                                   xT_sb[:, c, vl:vl + VBLK],
                                   start=(c == 0), stop=(c == DC - 1))
                mm.then_inc(s_attn, 1)

            pe.wait_ge(s_bcp, 1)                 # ps_misc[0] binit copied out
            pe.wait_ge(s_a3c, 1)                 # a3_sb ready (+ps_misc[1] free)
            attn(0)
            for it in range(NIT):
                bN, h = divmod(it, H_)
                vlo = bN * VBLK
                if it >= 2:
                    transp(it - 2)
                if h == 3 and bN + 1 < NBLK:
                    if bN >= 1:
                        pe.wait_ge(s_w, bN)      # attn region read (exp bN-1)
                        pe.wait_ge(s_st, 4 * bN)  # pw cols of block bN-1 copied
                    attn(bN + 1)
                if it == 2:
                    pe.wait_ge(s_ldw2, 16)       # W_sb heads 2-3 loaded
                if it >= 2:
                    pe.wait_ge(s_tbsb, it - 1)   # ps_tB[it%2] free (ACT copied)
                mm = pe.matmul(ps_tB[it % 2][:, 0:VBLK], W_sb[:, h, 0, :],
                               xT_sb[:, 0, vlo:vlo + VBLK],
                               start=True, stop=(DC == 1))
                for c in range(1, DC):
                    mm = pe.matmul(ps_tB[it % 2][:, 0:VBLK], W_sb[:, h, c, :],
                                   xT_sb[:, c, vlo:vlo + VBLK],
                                   start=False, stop=(c == DC - 1))
                mm.then_inc(s_tb, 1)
                pe.wait_ge(s_w, bN + 1)          # w4[bN%2] ready (exp done)
                if it >= 2:
                    pe.wait_ge(s_sb, it - 1)     # ps_wbc[it%2] free (DVE)
                if it >= 8:
                    pe.wait_ge(s_st, it - 7)     # ps_misc[bN%2] pw cols copied
                pe.matmul(ps_wbc[it % 2][:, 0:VBLK],
                          sel_sb[:, h * 128:(h + 1) * 128],
                          w4[bN % 2][:, :], start=True, stop=True)
                for j in range(NJ):
                    e = pe.matmul(
                        ps_misc[bN % 2][:, 4 * h + j:4 * h + j + 1],
                        w4[bN % 2][:, j * 128:(j + 1) * 128],
                        sel_sb[:, h * 128:h * 128 + 1], start=True, stop=True)
                e.then_inc(s_pw, 1)
            transp(NIT - 2)
            transp(NIT - 1)
            pe.wait_ge(s_ld2, 16)
            for c in range(NCH):
                pe.wait_ge(s_g[c % 2], 16 * (2 * (c // 2) + 1))
                if c >= 2:
                    pe.wait_ge(s_ep, c - 1)      # ps_A/ps_B2[c%2] free (DVE)
                for gi in range(16):
                    if gi == 8:
                        pe.wait_ge(s_g[c % 2], 32 * (c // 2 + 1))
                    pe.matmul(ps_A[c % 2][:, 0:SW],
                              blk_sb[:, 120 - 8 * gi:248 - 8 * gi],
                              gtile[c % 2][:, gi, 0:SW],
                              start=(gi == 0), stop=(gi == 15))
                for gi in range(16):
                    e = pe.matmul(ps_B2[c % 2][:, 0:H_],
                                  blk_sb[:, 120 - 8 * gi:248 - 8 * gi],
                                  gtile[c % 2][:, gi, SW:SW + H_],
                                  start=(gi == 0), stop=(gi == 15))
                e.then_inc(s_mm, 1)

        @block.scalar
        def _(s):
            s.wait_ge(s_a3, 1)
            s.activation(a3_sb[:, :], ps_misc[1][:, 0:DC * H_],
                         mybir.ActivationFunctionType.Copy).then_inc(s_a3c, 1)
            for it in range(NIT):
                bN, h = divmod(it, H_)
                if h == 0:
                    s.wait_ge(s_attn, bN + 1)
                    if bN >= 2:
                        s.wait_ge(s_pw, 4 * bN - 4)  # w4[bN%2] free (PE read)
                    s.activation(w4[bN % 2][:, :],
                                 ps_misc[bN % 2][0:4, 0:VBLK],
                                 mybir.ActivationFunctionType.Exp
                                 ).then_inc(s_w, 1)
                s.wait_ge(s_tb, it + 1)
                if it >= 2:
                    s.wait_ge(s_sb, it - 1)      # tB_sb[it%2] free (DVE read)
                s.activation(tB_sb[it % 2][:, :], ps_tB[it % 2][:, 0:VBLK],
                             mybir.ActivationFunctionType.Copy).then_inc(s_tbsb, 1)
            for c in range(NCH):
                s.wait_ge(s_ep, c + 1)
                if c >= 2:
                    s.wait_ge(s_o[c % 2], 16 * (c // 2))  # ostage free
                s.activation(ostage[c % 2][:, :], acc_sb[c % 2][:, :],
                             mybir.ActivationFunctionType.Relu,
                             scale=1.0 / H_).then_inc(s_rel, 1)

        @block.vector
        def _(v):
            def copies(t):
                b, h = divmod(t, H_)
                v.wait_ge(s_pc, t + 1)           # ps_C[t%2] s-cols ready
                if h == 0 and b >= 2:
                    v.wait_ge(s_sd[b % 2], 16 * cum_par[b - 2][b % 2])
                v.tensor_copy(
                    stage[b % 2][:, :, h * O_:(h + 1) * O_],
                    bass.AP(ps_C[t % 2], 0, [[1024, 128], [128, NJ], [1, 128]]))
                v.tensor_copy(
                    stage[b % 2][:, :, SW + h:SW + h + 1],
                    bass.AP(ps_misc[b % 2], 4 * h, [[512, 128], [1, NJ], [1, 1]])
                ).then_inc(s_st, 1)

            v.memset(ones4[:, :], 1.0)
            v.memset(stage[0][:, :, SW + H_:RW], 0.0)
            v.memset(stage[1][:, :, SW + H_:RW], 0.0)
            if VPF > VP:
                v.memset(xT_sb[:, :, VP:VPF], 0.0)
            v.memset(zero_sb[:, :], 0.0).then_inc(s_dvi, 1)
            v.wait_ge(s_bini, 1)
            v.tensor_copy(binit_sb[:, :], ps_misc[0][:, 0:O_]).then_inc(s_bcp, 1)
            for it in range(NIT):
                if it >= 1:
                    copies(it - 1)
                v.wait_ge(s_tbsb, it + 1)        # tB_sb[it%2] ready
                v.wait_ge(s_pw, it + 1)          # ps_wbc[it%2] ready
                if it >= 2:
                    v.wait_ge(s_pc, it - 1)      # sB_sb[it%2] free (PE read)
                v.tensor_tensor(sB_sb[it % 2][:, :], tB_sb[it % 2][:, :],
                                ps_wbc[it % 2][:, 0:VBLK],
                                mybir.AluOpType.mult).then_inc(s_sb, 1)
            copies(NIT - 1)
            for c in range(NCH):
                v.wait_ge(s_mm, c + 1)
                if c >= 2:
                    v.wait_ge(s_rel, c - 1)      # acc_sb[c%2] free (ACT read)
                v.tensor_scalar_max(den_sb[c % 2][:, :], ps_B2[c % 2][:, 0:H_],
                                    1e-30)
                v.drain()
                v.reciprocal(rcp_sb[c % 2][:, :], den_sb[c % 2][:, :])
                v.drain()
                v.scalar_tensor_tensor(
                    acc_sb[c % 2][:, :], ps_A[c % 2][:, 0:O_],
                    rcp_sb[c % 2][:, 0:1], binit_sb[:, :],
                    mybir.AluOpType.mult, mybir.AluOpType.add)
                for h in range(1, H_):
                    v.drain()
                    e = v.scalar_tensor_tensor(
                        acc_sb[c % 2][:, :], ps_A[c % 2][:, h * O_:(h + 1) * O_],
                        rcp_sb[c % 2][:, h:h + 1], acc_sb[c % 2][:, :],
                        mybir.AluOpType.mult, mybir.AluOpType.add)
                e.then_inc(s_ep, 1)

        @block.sync
        def _(sy):
            sy.dma_start(out=idx_sb[:, :], in_=idxp[:, :]).then_inc(s_idx, 16)
            for bN in range(NBLK):
                sy.wait_ge(s_st, H_ * (bN + 1))
                lo, hi = bN * VBLK, min(VP, bN * VBLK + VBLK)
                full_j, rem = (hi - lo) // 128, (hi - lo) % 128
                if full_j:
                    sy.dma_start(
                        out=bass.AP(tbl_loc, lo * RW,
                                    [[RW, 128], [128 * RW, full_j], [1, RW]]),
                        in_=stage[bN % 2][:, 0:full_j, :]).then_inc(s_sd[bN % 2], 16)
                if rem:
                    sy.dma_start(
                        out=bass.AP(tbl_loc, (lo + full_j * 128) * RW,
                                    [[RW, rem], [1, RW]]),
                        in_=stage[bN % 2][0:rem, full_j, :]).then_inc(s_sd[bN % 2], 16)
            for c in range(NCH):
                sy.wait_ge(s_rel, c + 1)
                lo = c * 128
                rows = min(128, VP - lo)
                sy.dma_start(out=bass.AP(out_ext, lo * O_, [[O_, rows], [1, O_]]),
                             in_=ostage[c % 2][0:rows, :]).then_inc(s_o[c % 2], 16)
            sy.wait_ge(s_o[0], 16 * ((NCH + 1) // 2))
            if NCH > 1:
                sy.wait_ge(s_o[1], 16 * (NCH // 2))

    sctx.close()
    ctx.close()
    nc.compile()
    return nc


def prep_core_inputs(cfg: Cfg, x, W, a, b, adj_lst, r):
    """Host-side shard/layout prep for core r (index/layout only, no math)."""
    VP, VPAD, NCH = cfg.VP, cfg.VPAD, cfg.NCHUNK
    xs = np.ascontiguousarray(x[r * VP:(r + 1) * VP].T)
    adj = adj_lst[r * VP:(r + 1) * VP]
    # table row ids for the 2-half AllGather layout:
    #   [core-major half-1 rows | core-major half-2 rows (+ zero pad)]
    NC, H1, H2 = cfg.ncores, cfg.VPH1, cfg.VPH2
    rr, q = adj // VP, adj % VP
    rows = np.where(q < H1, rr * H1 + q,
                    NC * H1 + rr * H2 + (q - H1))
    rows = np.where(adj == cfg.V, NC * H1 + (VP - H1), rows).astype(np.int32)
    rows_p = np.full((NCH * 128, cfg.K), NC * H1 + (VP - H1), np.int32)
    rows_p[:VP] = rows
    # gtile[p, g, :] = tbl[flat[g*128+p]]; edge (node 8g+i, k) at p=16i+k.
    # dma_gather flat order i reads idxs_sb[i%16, i//16] (16-part wrap,
    # replicated to all 8 gpsimd cores); two 1024-idx ops per chunk.
    lin = rows_p.reshape(NCH, 16, 128)           # [c, g, p] (p = i*16 + k)
    F = lin.reshape(NCH, 2, 64, 16)              # [c, half, s, q]: i = s*16+q
    B = F.transpose(0, 1, 3, 2)                  # [c, half, q, s]
    idx = np.ascontiguousarray(
        np.tile(B, (1, 1, 8, 1)).transpose(2, 0, 1, 3).reshape(128, NCH * 128)
    ).astype(np.int16)
    blkm = np.zeros((128, 248), np.float32)
    blkm[np.arange(128), 120 + np.arange(128) // 16] = 1.0
    sel = np.zeros((cfg.H, cfg.H * 128), np.float32)
    for h in range(cfg.H):
        sel[h, h * 128:(h + 1) * 128] = 1.0
    W_ = np.asarray(W, np.float32)
    WT = np.ascontiguousarray(
        W_.transpose(2, 0, 1).reshape(cfg.O, cfg.H, cfg.DC, 128))
    return {
        "xT": xs.astype(np.float32), "W": W_, "WT": WT,
        "aT": np.ascontiguousarray(np.asarray(a, np.float32).T),
        "b": np.asarray(b, np.float32),
        "eye": np.eye(128, dtype=np.float32), "blk": blkm, "idx": idx,
        "sel": sel,
    }


_GRAPH_CACHE = {}


def kernel(x, W, a, b, adj_lst, mask_index, _cfg=None, _trace=False):
    cfg = _cfg or Cfg()
    x = np.asarray(x)
    adj_lst = np.asarray(adj_lst)
    assert int(mask_index) == cfg.V
    key = (cfg.V, cfg.ncores)
    if key not in _GRAPH_CACHE:
        _GRAPH_CACHE[key] = build_graph(cfg)
    nc = _GRAPH_CACHE[key]
    in_maps = [prep_core_inputs(cfg, x, W, a, b, adj_lst, r)
               for r in range(cfg.ncores)]
    res = run_bass_kernel_spmd(nc, in_maps, list(range(cfg.ncores)),
                               trace=_trace)
    out = np.concatenate([res.results[r]["out"] for r in range(cfg.ncores)], 0)
    kernel._last_exec_ns = res.exec_time_ns
    return out


# revision 76
# speedup vs baseline: 1.3146x; 1.0199x over previous
"""Distributed GAT (nn_AdjGAT) kernel for 8 TRN2 NeuronCores.

Math: the reference's per-edge softmax logit depends ONLY on the source
node (scores[h,v,k] = attn[h, adj[v,k]]), so
    softmax_k -> w[n_k] / sum_j w[n_j],  w = exp(attn)
    head[h,v] = (sum_k w[h,n_k] * t[h,n_k]) / (sum_k w[h,n_k])
    out = relu(mean_h(head + b[h]))
Phase 1 (per core): build node-major table rows [w*t_0..w*t_3 | w_0..w_3]
(bf16, padded to 640 elems = 1280B).  attn is computed independently of t
via a3 = W_h @ a_h (device-precomputed), attn = a3^T x per 512-node block
for all 4 heads at once.  AllGather the row table.
Phase 2: per 128-node chunk, two 1024-row dma_gather ops fetch the 2048
neighbor rows (SWDGE descriptor generation on GpSimd is the bottleneck,
~8ns/row), then block-diagonal-ones matmuls sum each node's K=16 rows
(numerator AND denominator), then normalize/relu.  Padding slots point at
a hardwired zero row (w=0).
"""

import math
from contextlib import ExitStack

import numpy as np

import concourse.bass as bass
import concourse.bacc as bacc
import concourse.mybir as mybir
from concourse import library_config
from concourse.bass_utils import run_bass_kernel_spmd

F32 = mybir.dt.float32
BF16 = mybir.dt.bfloat16
I16 = mybir.dt.int16
I32 = mybir.dt.int32

V, D, K, O, H = 20000, 256, 16, 128, 4
NCORES = 8


class Cfg:
    def __init__(self, V=V, D=D, K=K, O=O, H=H, ncores=NCORES):
        assert V % ncores == 0
        self.V, self.D, self.K, self.O, self.H, self.ncores = V, D, K, O, H, ncores
        self.VP = V // ncores
        self.ZPAD = 12
        self.VPAD = self.VP + self.ZPAD
        self.VT = self.VPAD * ncores
        self.RW = H * O + 128          # 640 bf16 elems: [s (512) | w (4) | pad]
        self.NCHUNK = math.ceil(self.VP / 128)
        self.VBLK = 512 if self.VP >= 512 else 128 * math.ceil(self.VP / 128)
        self.NBLK = math.ceil(self.VP / self.VBLK)
        self.NJ = self.VBLK // 128
        self.VPF = self.NBLK * self.VBLK
        self.DC = D // 128
        self.VPH1 = self.VP // 2          # table half-1 rows per core
        self.VPH2 = self.VPAD - self.VPH1
        assert D % 128 == 0 and O == 128 and K == 16 and self.DC >= 1
        assert self.VT < 32000


def build_graph(cfg: Cfg, skip_ag=False):
    # 96KB/partition SWDGE ring: holds 6 prepared 1024-desc gather ops
    nc = bacc.Bacc(dynamic_dma_scratch_size=98304)
    VP, VPAD, VT, RW, H_, O_, DC = (
        cfg.VP, cfg.VPAD, cfg.VT, cfg.RW, cfg.H, cfg.O, cfg.DC)
    NCH, NBLK, VBLK, NJ, VPF = cfg.NCHUNK, cfg.NBLK, cfg.VBLK, cfg.NJ, cfg.VPF
    SW = H_ * O_
    NE = 128 * 16
    NIT = NBLK * H_

    xT = nc.declare_dram_parameter("xT", [cfg.D, VP], F32, isOutput=False)
    Wp = nc.declare_dram_parameter("W", [H_, cfg.D, O_], F32, isOutput=False)
    WTp = nc.declare_dram_parameter("WT", [O_, H_, DC, 128], F32, isOutput=False)
    aTp = nc.declare_dram_parameter("aT", [O_, H_], F32, isOutput=False)
    bp = nc.declare_dram_parameter("b", [H_, O_], F32, isOutput=False)
    eyep = nc.declare_dram_parameter("eye", [128, 128], F32, isOutput=False)
    selp = nc.declare_dram_parameter("sel", [H_, H_ * 128], F32, isOutput=False)
    blkp = nc.declare_dram_parameter("blk", [128, 248], F32, isOutput=False)
    idxp = nc.declare_dram_parameter("idx", [128, NCH * 128], I16, isOutput=False)
    out_ext = nc.declare_dram_parameter("out", [VP, O_], F32, isOutput=True)

    tbl_loc = nc.dram_tensor("tbl_loc", [VPAD, RW], BF16)
    tbl = nc.dram_tensor("tbl", [VT, RW], BF16, addr_space="Shared")

    ctx = ExitStack()
    sb = lambda name, shape, dt: ctx.enter_context(nc.sbuf_tensor(name, shape, dt))
    xT_sb = sb("xT_sb", [128, DC, VPF], BF16)
    W_sb = sb("W_sb", [128, H_, DC, O_], BF16)
    WT_sb = sb("WT_sb", [128, H_, DC, 128], BF16)
    aT_sb = sb("aT_sb", [128, H_], BF16)
    a3_sb = sb("a3_sb", [128, DC * H_], BF16)
    b_sb = sb("b_sb", [H_, O_], BF16)
    eye_sb = sb("eye_sb", [128, 128], BF16)
    blk_sb = sb("blk_sb", [128, 248], BF16)
    idx_sb = sb("idx_sb", [128, NCH * 128], I16)
    ones4 = sb("ones4", [H_, O_], BF16)
    zero_sb = sb("zero_sb", [cfg.ZPAD, RW], BF16)
    w4 = [sb(f"w4_{i}", [H_, VBLK], BF16) for i in range(2)]
    sel_sb = sb("sel_sb", [H_, H_ * 128], BF16)
    tB_sb = [sb(f"tB_sb{i}", [128, VBLK], BF16) for i in range(2)]
    sB_sb = [sb(f"sB_sb{i}", [128, VBLK], BF16) for i in range(2)]
    stage = [sb(f"stage{i}", [128, NJ, RW], BF16) for i in range(2)]
    binit_sb = sb("binit_sb", [128, O_], F32)
    gtile = [sb(f"gtile{i}", [128, 16, RW], BF16) for i in range(3)]
    den_sb = [sb(f"den{i}", [128, H_], F32) for i in range(2)]
    rcp_sb = [sb(f"rcp{i}", [128, H_], F32) for i in range(2)]
    acc_sb = [sb(f"acc{i}", [128, O_], F32) for i in range(2)]
    ostage = [sb(f"ostage{i}", [128, O_], F32) for i in range(2)]

    # phase-1 PSUM (freed before phase-2 allocs; full 2KB banks each)
    ph1 = ExitStack()
    psa = lambda name, shape, dt: ph1.enter_context(nc.psum_tensor(name, shape, dt))
    ps_tB = [psa(f"ps_tB{i}", [128, 512], F32) for i in range(2)]
    ps_wbc = [psa(f"ps_wbc{i}", [128, 512], F32) for i in range(2)]
    ps_misc = [psa(f"ps_misc{i}", [128, 512], F32) for i in range(2)]
    ps_C = [psa(f"ps_C{i}", [128, 1024], BF16) for i in range(2)]
    ph1.close()
    ph2 = ExitStack()
    ps_A = [ph2.enter_context(nc.psum_tensor(f"ps_A{i}", [128, 512], F32))
            for i in range(2)]
    ps_B2 = [ph2.enter_context(nc.psum_tensor(f"ps_B2{i}", [128, 512], F32))
             for i in range(2)]
    ph2.close()

    blk_ndma = []
    for bN in range(NBLK):
        lo, hi = bN * VBLK, min(VP, bN * VBLK + VBLK)
        blk_ndma.append((1 if (hi - lo) // 128 else 0) + (1 if (hi - lo) % 128 else 0))
    VPH1 = cfg.VPH1
    nblk1 = math.ceil(VPH1 / VBLK)

    sctx = ExitStack()
    sem = lambda n: sctx.enter_context(nc.semaphore(n))
    (s_ld, s_ld2, s_idx, s_dvi, s_bini, s_bcp, s_a3, s_a3c, s_tb, s_tbsb,
     s_attn, s_w, s_pw, s_sb, s_pc, s_w4t, s_st, s_zr, s_cc, s_mm, s_ep,
     s_rel) = [sem(n) for n in (
        "s_ld", "s_ld2", "s_idx", "s_dvi", "s_bini", "s_bcp", "s_a3",
        "s_a3c", "s_tb", "s_tbsb", "s_attn", "s_w", "s_pw", "s_sb", "s_pc",
        "s_w4t", "s_st", "s_zr", "s_cc", "s_mm", "s_ep", "s_rel")]
    s_xb = [sem(f"s_xb{i}") for i in range(NBLK)]
    s_ldb, s_lda, s_lds, s_ldwt, s_lde, s_ldw2 = [sem(n) for n in (
        "s_ldb", "s_lda", "s_lds", "s_ldwt", "s_lde", "s_ldw2")]
    NPC = min(3, NCH)                    # chunks gathered via prepare/trigger
    s_gp = [sem(f"s_gp{i}") for i in range(NPC)]
    s_prep = sem("s_prep")
    s_g = [sem(f"s_g{i}") for i in range(4)]
    s_gB = [sem(f"s_gB{i}") for i in range(4)]
    s_sd = [sem("s_sd0"), sem("s_sd1")]
    s_o = [sem("s_o0"), sem("s_o1")]
    # cumulative stage-DMA counts per parity (for safe per-parity waits)
    cum_par = []
    c0 = c1 = 0
    for bN in range(NBLK):
        if bN % 2 == 0:
            c0 += blk_ndma[bN]
        else:
            c1 += blk_ndma[bN]
        cum_par.append((c0, c1))
    with nc.Block() as block:
        @block.gpsimd
        def _(g):
            g.dma_start(out=b_sb[:, :], in_=bass.AP(
                bp, 0, [[O_, H_], [1, O_]])).then_inc(s_ldb, 16)
            g.dma_start(out=aT_sb[:, :], in_=bass.AP(
                aTp, 0, [[H_, 128], [1, H_]])).then_inc(s_lda, 16)
            g.dma_start(out=sel_sb[:, :], in_=selp[:, :]).then_inc(s_lds, 16)
            g.dma_start(out=W_sb[:, 0:2, :, :], in_=bass.AP(
                Wp, 0, [[O_, 128], [cfg.D * O_, 2], [128 * O_, DC], [1, O_]]
            )).then_inc(s_ld, 16)
            lo, hi = 0, min(VP, VBLK)
            g.dma_start(out=xT_sb[:, :, lo:hi], in_=bass.AP(
                xT, lo, [[VP, 128], [128 * VP, DC], [1, hi - lo]])
            ).then_inc(s_xb[0], 16)
            g.dma_start(out=W_sb[:, 2:4, :, :], in_=bass.AP(
                Wp, 2 * cfg.D * O_, [[O_, 128], [cfg.D * O_, 2], [128 * O_, DC],
                                     [1, O_]])).then_inc(s_ldw2, 16)
            g.dma_start(out=WT_sb[:, :, :, :], in_=bass.AP(
                WTp, 0, [[H_ * DC * 128, 128], [DC * 128, H_], [128, DC], [1, 128]]
            )).then_inc(s_ldwt, 16)
            g.dma_start(out=eye_sb[:, :], in_=eyep[:, :]).then_inc(s_lde, 16)
            for bN in range(1, NBLK):
                lo, hi = bN * VBLK, min(VP, bN * VBLK + VBLK)
                g.dma_start(out=xT_sb[:, :, lo:hi], in_=bass.AP(
                    xT, lo, [[VP, 128], [128 * VP, DC], [1, hi - lo]])
                ).then_inc(s_xb[bN], 16)
            g.dma_start(out=blk_sb[:, :], in_=blkp[:, :]).then_inc(s_ld2, 16)
            g.wait_ge(s_dvi, 1)
            g.dma_start(out=bass.AP(tbl_loc, VP * RW, [[RW, cfg.ZPAD], [1, RW]]),
                        in_=zero_sb[:, :]).then_inc(s_zr, 16)
            g.wait_ge(s_zr, 16)
            # preload the dma_gather ucode library while waiting on stage/AG
            g.load_library(library_config.mlp)
            # half-1 AllGather fires once blocks [0, nblk1) are staged
            c0h = sum(blk_ndma[b] for b in range(nblk1) if b % 2 == 0)
            c1h = sum(blk_ndma[b] for b in range(nblk1) if b % 2 == 1)
            if c0h:
                g.wait_ge(s_sd[0], 16 * c0h)
            if c1h:
                g.wait_ge(s_sd[1], 16 * c1h)
            NC = cfg.ncores
            if skip_ag:
                g.dma_start(out=tbl[0:NC * VPH1, :],
                            in_=tbl_loc[0:VPH1, :]).then_inc(s_cc, 16)
            else:
                g.collective_compute(
                    "AllGather", mybir.AluOpType.bypass,
                    replica_groups=[list(range(cfg.ncores))],
                    ins=[tbl_loc[0:VPH1, :]], outs=[tbl[0:NC * VPH1, :]],
                ).then_inc(s_cc)
            g.wait_ge(s_sd[0], 16 * cum_par[-1][0])
            if cum_par[-1][1]:
                g.wait_ge(s_sd[1], 16 * cum_par[-1][1])
            if skip_ag:
                g.dma_start(out=tbl[NC * VPH1:VT, :],
                            in_=tbl_loc[VPH1:VPAD, :]).then_inc(s_cc, 16)
            else:
                g.collective_compute(
                    "AllGather", mybir.AluOpType.bypass,
                    replica_groups=[list(range(cfg.ncores))],
                    ins=[tbl_loc[VPH1:VPAD, :]], outs=[tbl[NC * VPH1:VT, :]],
                ).then_inc(s_cc)
            g.wait_ge(s_idx, 16)
            # pre-generate descriptors for the first NPC chunks while the
            # AllGather is in flight; fire them the moment the table lands
            for c in range(NPC):
                for hf in range(2):
                    g.dma_gather(
                        out_ap=gtile[c][:, hf * 8:hf * 8 + 8, :],
                        in_ap=tbl[:, :],
                        idxs_ap=idx_sb[:, c * 128 + hf * 64:c * 128 + hf * 64 + 64],
                        num_idxs=NE // 2,
                        num_idxs_reg=NE // 2,
                        elem_size=RW,
                        prepare_only=True,
                        sem=s_gp[c],
                    ).then_inc(s_prep, 1)
            g.wait_ge(s_prep, 2 * NPC)
            g.wait_ge(s_cc, 32 if skip_ag else 2)
            g.trigger_dma(count=2 * NPC)
            for c in range(NPC, NCH):
                if c >= 3:
                    g.wait_ge(s_mm, c - 2)       # gtile[c%3] free (PE chunk c-3)
                for hf in range(2):
                    g.dma_gather(
                        out_ap=gtile[c % 3][:, hf * 8:hf * 8 + 8, :],
                        in_ap=tbl[:, :],
                        idxs_ap=idx_sb[:, c * 128 + hf * 64:c * 128 + hf * 64 + 64],
                        num_idxs=NE // 2,
                        num_idxs_reg=NE // 2,
                        elem_size=RW,
                    ).then_inc((s_g if hf == 0 else s_gB)[c % 4], 16)

        @block.tensor
        def _(pe):
            def transp(t):
                pe.wait_ge(s_sb, t + 1)          # sB_sb[t%2] ready (DVE done)
                if t >= 2:
                    pe.wait_ge(s_st, t - 1)      # ps_C[t%2] s-cols free (DVE)
                for j in range(NJ):
                    e = pe.transpose(ps_C[t % 2][:, j * 128:(j + 1) * 128],
                                     sB_sb[t % 2][:, j * 128:(j + 1) * 128],
                                     eye_sb[:, :])
                e.then_inc(s_pc, 1)

            pe.wait_ge(s_ldb, 16)
            pe.wait_ge(s_dvi, 1)
            pe.matmul(ps_misc[0][:, 0:O_], ones4[:, :], b_sb[:, :],
                      start=True, stop=True).then_inc(s_bini, 1)
            pe.wait_ge(s_ldwt, 16)
            pe.wait_ge(s_lda, 16)
            for c in range(DC):
                for h in range(H_):
                    e = pe.matmul(ps_misc[1][:, c * H_ + h:c * H_ + h + 1],
                                  WT_sb[:, h, c, :], aT_sb[:, h:h + 1],
                                  start=True, stop=True)
            e.then_inc(s_a3, 1)
            pe.wait_ge(s_ld, 16)                 # W_sb
            pe.wait_ge(s_lds, 16)                # sel_sb
            pe.wait_ge(s_lde, 16)                # eye_sb

            def attn(b):
                pe.wait_ge(s_xb[b], 16)
                vl = b * VBLK
                for c in range(DC):
                    mm = pe.matmul(ps_misc[b % 2][0:4, 0:VBLK],
                                   a3_sb[:, c * H_:(c + 1) * H_],
                                   xT_sb[:, c, vl:vl + VBLK],
                                   start=(c == 0), stop=(c == DC - 1))
                mm.then_inc(s_attn, 1)

            pe.wait_ge(s_bcp, 1)                 # ps_misc[0] binit copied out
            pe.wait_ge(s_a3c, 1)                 # a3_sb ready (+ps_misc[1] free)
            attn(0)
            for it in range(NIT):
                bN, h = divmod(it, H_)
                vlo = bN * VBLK
                if it >= 2:
                    transp(it - 2)
                if h == 3 and bN + 1 < NBLK:
                    if bN >= 1:
                        pe.wait_ge(s_w, bN)      # attn region read (exp bN-1)
                        pe.wait_ge(s_st, 4 * bN)  # pw cols of block bN-1 copied
                    attn(bN + 1)
                if it == 2:
                    pe.wait_ge(s_ldw2, 16)       # W_sb heads 2-3 loaded
                if it >= 2:
                    pe.wait_ge(s_tbsb, it - 1)   # ps_tB[it%2] free (ACT copied)
                mm = pe.matmul(ps_tB[it % 2][:, 0:VBLK], W_sb[:, h, 0, :],
                               xT_sb[:, 0, vlo:vlo + VBLK],
                               start=True, stop=(DC == 1))
                for c in range(1, DC):
                    mm = pe.matmul(ps_tB[it % 2][:, 0:VBLK], W_sb[:, h, c, :],
                                   xT_sb[:, c, vlo:vlo + VBLK],
                                   start=False, stop=(c == DC - 1))
                mm.then_inc(s_tb, 1)
                pe.wait_ge(s_w, bN + 1)          # w4[bN%2] ready (exp done)
                if it >= 2:
                    pe.wait_ge(s_sb, it - 1)     # ps_wbc[it%2] free (DVE)
                if it >= 8:
                    pe.wait_ge(s_st, it - 7)     # ps_misc[bN%2] pw cols copied
                pe.matmul(ps_wbc[it % 2][:, 0:VBLK],
                          sel_sb[:, h * 128:(h + 1) * 128],
                          w4[bN % 2][:, :], start=True, stop=True)
                for j in range(NJ):
                    e = pe.matmul(
                        ps_misc[bN % 2][:, 4 * h + j:4 * h + j + 1],
                        w4[bN % 2][:, j * 128:(j + 1) * 128],
                        sel_sb[:, h * 128:h * 128 + 1], start=True, stop=True)
                e.then_inc(s_pw, 1)
            transp(NIT - 2)
            transp(NIT - 1)
            pe.wait_ge(s_ld2, 16)
            for c in range(NCH):
                m = (c - NPC) // 4 + 1
                if c < NPC:
                    pe.wait_ge(s_gp[c], 32)
                else:
                    pe.wait_ge(s_g[c % 4], 16 * m)
                if c >= 2:
                    pe.wait_ge(s_ep, c - 1)      # ps_A/ps_B2[c%2] free (DVE)
                for gi in range(16):
                    if gi == 8 and c >= NPC:
                        pe.wait_ge(s_gB[c % 4], 16 * m)
                    pe.matmul(ps_A[c % 2][:, 0:SW],
                              blk_sb[:, 120 - 8 * gi:248 - 8 * gi],
                              gtile[c % 3][:, gi, 0:SW],
                              start=(gi == 0), stop=(gi == 15))
                for gi in range(16):
                    e = pe.matmul(ps_B2[c % 2][:, 0:H_],
                                  blk_sb[:, 120 - 8 * gi:248 - 8 * gi],
                                  gtile[c % 3][:, gi, SW:SW + H_],
                                  start=(gi == 0), stop=(gi == 15))
                e.then_inc(s_mm, 1)

        @block.scalar
        def _(s):
            s.wait_ge(s_a3, 1)
            s.activation(a3_sb[:, :], ps_misc[1][:, 0:DC * H_],
                         mybir.ActivationFunctionType.Copy).then_inc(s_a3c, 1)
            for it in range(NIT):
                bN, h = divmod(it, H_)
                if h == 0:
                    s.wait_ge(s_attn, bN + 1)
                    if bN >= 2:
                        s.wait_ge(s_pw, 4 * bN - 4)  # w4[bN%2] free (PE read)
                    s.activation(w4[bN % 2][:, :],
                                 ps_misc[bN % 2][0:4, 0:VBLK],
                                 mybir.ActivationFunctionType.Exp
                                 ).then_inc(s_w, 1)
                s.wait_ge(s_tb, it + 1)
                if it >= 2:
                    s.wait_ge(s_sb, it - 1)      # tB_sb[it%2] free (DVE read)
                s.activation(tB_sb[it % 2][:, :], ps_tB[it % 2][:, 0:VBLK],
                             mybir.ActivationFunctionType.Copy).then_inc(s_tbsb, 1)
            for c in range(NCH):
                s.wait_ge(s_ep, c + 1)
                if c >= 2:
                    s.wait_ge(s_o[c % 2], 16 * (c // 2))  # ostage free
                s.activation(ostage[c % 2][:, :], acc_sb[c % 2][:, :],
                             mybir.ActivationFunctionType.Relu,
                             scale=1.0 / H_).then_inc(s_rel, 1)

        @block.vector
        def _(v):
            def copies(t):
                b, h = divmod(t, H_)
                v.wait_ge(s_pc, t + 1)           # ps_C[t%2] s-cols ready
                if h == 0 and b >= 2:
                    v.wait_ge(s_sd[b % 2], 16 * cum_par[b - 2][b % 2])
                v.tensor_copy(
                    stage[b % 2][:, :, h * O_:(h + 1) * O_],
                    bass.AP(ps_C[t % 2], 0, [[1024, 128], [128, NJ], [1, 128]]))
                v.tensor_copy(
                    stage[b % 2][:, :, SW + h:SW + h + 1],
                    bass.AP(ps_misc[b % 2], 4 * h, [[512, 128], [1, NJ], [1, 1]])
                ).then_inc(s_st, 1)

            v.memset(ones4[:, :], 1.0)
            v.memset(stage[0][:, :, SW + H_:RW], 0.0)
            v.memset(stage[1][:, :, SW + H_:RW], 0.0)
            if VPF > VP:
                v.memset(xT_sb[:, :, VP:VPF], 0.0)
            v.memset(zero_sb[:, :], 0.0).then_inc(s_dvi, 1)
            v.wait_ge(s_bini, 1)
            v.tensor_copy(binit_sb[:, :], ps_misc[0][:, 0:O_]).then_inc(s_bcp, 1)
            for it in range(NIT):
                if it >= 1:
                    copies(it - 1)
                v.wait_ge(s_tbsb, it + 1)        # tB_sb[it%2] ready
                v.wait_ge(s_pw, it + 1)          # ps_wbc[it%2] ready
                if it >= 2:
                    v.wait_ge(s_pc, it - 1)      # sB_sb[it%2] free (PE read)
                v.tensor_tensor(sB_sb[it % 2][:, :], tB_sb[it % 2][:, :],
                                ps_wbc[it % 2][:, 0:VBLK],
                                mybir.AluOpType.mult).then_inc(s_sb, 1)
            copies(NIT - 1)
            for c in range(NCH):
                v.wait_ge(s_mm, c + 1)
                if c >= 2:
                    v.wait_ge(s_rel, c - 1)      # acc_sb[c%2] free (ACT read)
                v.tensor_scalar_max(den_sb[c % 2][:, :], ps_B2[c % 2][:, 0:H_],
                                    1e-30)
                v.drain()
                v.reciprocal(rcp_sb[c % 2][:, :], den_sb[c % 2][:, :])
                v.drain()
                v.scalar_tensor_tensor(
                    acc_sb[c % 2][:, :], ps_A[c % 2][:, 0:O_],
                    rcp_sb[c % 2][:, 0:1], binit_sb[:, :],
                    mybir.AluOpType.mult, mybir.AluOpType.add)
                for h in range(1, H_):
                    v.drain()
                    e = v.scalar_tensor_tensor(
                        acc_sb[c % 2][:, :], ps_A[c % 2][:, h * O_:(h + 1) * O_],
                        rcp_sb[c % 2][:, h:h + 1], acc_sb[c % 2][:, :],
                        mybir.AluOpType.mult, mybir.AluOpType.add)
                e.then_inc(s_ep, 1)

        @block.sync
        def _(sy):
            sy.dma_start(out=idx_sb[:, :], in_=idxp[:, :]).then_inc(s_idx, 16)
            for bN in range(NBLK):
                sy.wait_ge(s_st, H_ * (bN + 1))
                lo, hi = bN * VBLK, min(VP, bN * VBLK + VBLK)
                full_j, rem = (hi - lo) // 128, (hi - lo) % 128
                if full_j:
                    sy.dma_start(
                        out=bass.AP(tbl_loc, lo * RW,
                                    [[RW, 128], [128 * RW, full_j], [1, RW]]),
                        in_=stage[bN % 2][:, 0:full_j, :]).then_inc(s_sd[bN % 2], 16)
                if rem:
                    sy.dma_start(
                        out=bass.AP(tbl_loc, (lo + full_j * 128) * RW,
                                    [[RW, rem], [1, RW]]),
                        in_=stage[bN % 2][0:rem, full_j, :]).then_inc(s_sd[bN % 2], 16)
            for c in range(NCH):
                sy.wait_ge(s_rel, c + 1)
                lo = c * 128
                rows = min(128, VP - lo)
                sy.dma_start(out=bass.AP(out_ext, lo * O_, [[O_, rows], [1, O_]]),
                             in_=ostage[c % 2][0:rows, :]).then_inc(s_o[c % 2], 16)
            sy.wait_ge(s_o[0], 16 * ((NCH + 1) // 2))
            if NCH > 1:
                sy.wait_ge(s_o[1], 16 * (NCH // 2))

    sctx.close()
    ctx.close()
    nc.compile()
    return nc


def prep_core_inputs(cfg: Cfg, x, W, a, b, adj_lst, r):
    """Host-side shard/layout prep for core r (index/layout only, no math)."""
    VP, VPAD, NCH = cfg.VP, cfg.VPAD, cfg.NCHUNK
    xs = np.ascontiguousarray(x[r * VP:(r + 1) * VP].T)
    adj = adj_lst[r * VP:(r + 1) * VP]
    # table row ids for the 2-half AllGather layout:
    #   [core-major half-1 rows | core-major half-2 rows (+ zero pad)]
    NC, H1, H2 = cfg.ncores, cfg.VPH1, cfg.VPH2
    rr, q = adj // VP, adj % VP
    rows = np.where(q < H1, rr * H1 + q,
                    NC * H1 + rr * H2 + (q - H1))
    rows = np.where(adj == cfg.V, NC * H1 + (VP - H1), rows).astype(np.int32)
    rows_p = np.full((NCH * 128, cfg.K), NC * H1 + (VP - H1), np.int32)
    rows_p[:VP] = rows
    # gtile[p, g, :] = tbl[flat[g*128+p]]; edge (node 8g+i, k) at p=16i+k.
    # dma_gather flat order i reads idxs_sb[i%16, i//16] (16-part wrap,
    # replicated to all 8 gpsimd cores); two 1024-idx ops per chunk.
    lin = rows_p.reshape(NCH, 16, 128)           # [c, g, p] (p = i*16 + k)
    F = lin.reshape(NCH, 2, 64, 16)              # [c, half, s, q]: i = s*16+q
    B = F.transpose(0, 1, 3, 2)                  # [c, half, q, s]
    idx = np.ascontiguousarray(
        np.tile(B, (1, 1, 8, 1)).transpose(2, 0, 1, 3).reshape(128, NCH * 128)
    ).astype(np.int16)
    blkm = np.zeros((128, 248), np.float32)
    blkm[np.arange(128), 120 + np.arange(128) // 16] = 1.0
    sel = np.zeros((cfg.H, cfg.H * 128), np.float32)
    for h in range(cfg.H):
        sel[h, h * 128:(h + 1) * 128] = 1.0
    W_ = np.asarray(W, np.float32)
    WT = np.ascontiguousarray(
        W_.transpose(2, 0, 1).reshape(cfg.O, cfg.H, cfg.DC, 128))
    return {
        "xT": xs.astype(np.float32), "W": W_, "WT": WT,
        "aT": np.ascontiguousarray(np.asarray(a, np.float32).T),
        "b": np.asarray(b, np.float32),
        "eye": np.eye(128, dtype=np.float32), "blk": blkm, "idx": idx,
        "sel": sel,
    }


_GRAPH_CACHE = {}


def kernel(x, W, a, b, adj_lst, mask_index, _cfg=None, _trace=False):
    cfg = _cfg or Cfg()
    x = np.asarray(x)
    adj_lst = np.asarray(adj_lst)
    assert int(mask_index) == cfg.V
    key = (cfg.V, cfg.ncores)
    if key not in _GRAPH_CACHE:
        _GRAPH_CACHE[key] = build_graph(cfg)
    nc = _GRAPH_CACHE[key]
    in_maps = [prep_core_inputs(cfg, x, W, a, b, adj_lst, r)
               for r in range(cfg.ncores)]
    res = run_bass_kernel_spmd(nc, in_maps, list(range(cfg.ncores)),
                               trace=_trace)
    out = np.concatenate([res.results[r]["out"] for r in range(cfg.ncores)], 0)
    kernel._last_exec_ns = res.exec_time_ns
    return out
